# revision 1
# baseline (speedup 1.0000x reference)
"""Causal self-attention (dense transformer) on 8 trn2 NeuronCores.

Reference semantics (note the headless reshape):
  x_proj = x @ Wqkv + bqkv                     # [B, T, 3C]
  q = x_proj[:, :, :C].reshape(B, H, T, hd)    # direct reshape, no transpose!
Because of the direct reshape, head h consumes the contiguous row block
x_proj[b, h*128:(h+1)*128, :] reinterpreted as [T, hd].  So sharding by
(batch, head-group) makes QKV projection + attention fully core-local;
only the output projection is a row-parallel partial sum, reduced on host.

Shapes (hardcoded): B=2, T=2048, C=1024, n_head=16, hd=64, 8 cores.
Core c: batch b=c//4, quarter q=c%4 -> x rows [512q, 512q+512), heads 4q..4q+3.

Device layout tricks:
- Q,K columns of x_proj computed in TRANSPOSED orientation (lhsT=Wqkv tile,
  rhs=x^T tile): the per-head Q^T/K^T [hd, T] layouts fall out of the PSUM
  eviction with a stride-16 destination AP (no PE transposes at all).  Bias
  is folded into the eviction (tensor_scalar add, per-partition scalar).
- V columns computed in natural orientation, bounced through a DRAM scratch
  and gathered back as [s, hd] tiles (re-partition); a ones column is
  appended so P@[V|1] also yields the softmax denominator row.
- exp on ACT reads 2-bank PSUM S^T tiles directly, 1/sqrt(hd) folded into
  the activation scale; causal masking via gpsimd affine_select (in-place).
- softmax normalization: reciprocal of denom row, broadcast via a K=1 PE
  outer product, multiply on DVE during Y eviction.
- all matmul operands are float32r (~13-bit mantissa, 1 cyc/row on PE).
"""

import os

import numpy as np

os.environ.setdefault("NEURON_RT_RESET_CORES", "1")

import concourse.bacc as bacc
import concourse.mybir as mybir
import concourse.tile as tile
from concourse.bass_utils import run_bass_kernel_spmd

dt = mybir.dt
AF = mybir.ActivationFunctionType
OP = mybir.AluOpType

B, T, C = 2, 2048, 1024
NH, HD = 16, 64
N_CORES = 8
HPC = 4          # heads per core
RPC = 512        # x rows per core
SCALE = 1.0 / 8.0   # 1/sqrt(hd), folded into the exp activation


def build_program():
    nc = bacc.Bacc("TRN2", target_bir_lowering=False, debug=False,
                   num_devices=N_CORES)

    # ---- DRAM I/O (per core) ----
    xT = nc.dram_tensor("xT", [128, 8, RPC], dt.float32r, kind="ExternalInput")
    wq = nc.dram_tensor("wq", [16, 128, 8 * 128], dt.float32r, kind="ExternalInput")
    wv = nc.dram_tensor("wv", [2, 128, 8 * 512], dt.float32r, kind="ExternalInput")
    bqk = nc.dram_tensor("bqk", [1, 2048], dt.float32r, kind="ExternalInput")
    bv = nc.dram_tensor("bv", [1, 1024], dt.float32r, kind="ExternalInput")
    wp = nc.dram_tensor("wp", [128, 2 * 1024], dt.float32r, kind="ExternalInput")
    bp = nc.dram_tensor("bp", [128, 1024], dt.float32, kind="ExternalInput")
    ones512 = nc.dram_tensor("ones512", [1, 512], dt.float32r, kind="ExternalInput")
    ones16 = nc.dram_tensor("ones16", [128, 16], dt.float32r, kind="ExternalInput")
    out_d = nc.dram_tensor("out", [T, C], dt.float32, kind="ExternalOutput")

    with tile.TileContext(nc) as tc:
        with tc.tile_pool(name="persist", bufs=1) as pp, \
             tc.tile_pool(name="drampool", bufs=1, space="DRAM") as dp:
            vscr = [dp.tile([128, 1024], dt.float32r, tag=f"vscr{h}",
                            name=f"vscr{h}") for h in range(HPC)]

            xt = pp.tile([128, 8, RPC], dt.float32r, tag="xt")
            bqk_sb = pp.tile([1, 2048], dt.float32r, tag="bqk")
            bv_sb = pp.tile([1, 1024], dt.float32r, tag="bv")
            onesr = pp.tile([1, 512], dt.float32r, tag="onesr")
            ones16_sb = pp.tile([128, 16], dt.float32r, tag="ones16")
            wp_sb = pp.tile([128, 2, 1024], dt.float32r, tag="wp")
            bp_sb = pp.tile([128, 1024], dt.float32, tag="bp")

            qt_all = pp.tile([64, HPC * T], dt.float32r, tag="qt_all")
            kt_all = pp.tile([64, HPC * T], dt.float32r, tag="kt_all")
            vn = [pp.tile([128, 16 * 65], dt.float32r, tag=f"vn{h}", name=f"vn{h}")
                  for h in range(HPC)]            # per-head [V | 1] s-tiles
            yt = [pp.tile([128, T], dt.float32r, tag=f"yt{p}", name=f"yt{p}")
                  for p in range(2)]

            # attention pools opened early: first S/exp groups are hoisted
            # into phase 1 so ACT warms up while PE finishes the V part
            with tc.tile_pool(name="ptpool", bufs=10) as ptp, \
                 tc.tile_pool(name="ps2", bufs=2, space="PSUM") as ps2:

                def emit_sexp(h, j, sp):
                    """S^T matmuls for an s-pair + exp + causal mask."""
                    ssp = ps2.tile([128, 1024], dt.float32, tag="spsum",
                                   name=f"ssp{h}{j}{sp}")
                    for half in range(2):
                        i = 2 * sp + half
                        nc.tensor.matmul(
                            ssp[:, 512 * half:512 * (half + 1)],
                            kt_all[:, T * h + 128 * i:T * h + 128 * (i + 1)],
                            qt_all[:, T * h + 512 * j:T * h + 512 * (j + 1)],
                            start=True, stop=True)
                    pt = ptp.tile([128, 1024], dt.float32r, tag="pt",
                                  name=f"pt{h}{j}{sp}")
                    nc.scalar.activation(pt[:], ssp[:], AF.Exp, scale=SCALE)
                    for half in range(2):
                        i = 2 * sp + half
                        if i >= 4 * j:  # diagonal band: causal mask
                            nc.gpsimd.affine_select(
                                out=pt[:, 512 * half:512 * (half + 1)],
                                in_=pt[:, 512 * half:512 * (half + 1)],
                                compare_op=OP.is_ge, fill=0.0,
                                base=512 * j - 128 * i,
                                channel_multiplier=-1,
                                pattern=[[1, 512]])
                    return pt

                # ================= Phase 1: QKV projection =================
                with tc.tile_pool(name="wstream", bufs=2) as ws, \
                     tc.tile_pool(name="ps1", bufs=2, space="PSUM") as ps1:
                    # --- Q,K in transposed orientation: x_proj^T j-tiles ---
                    for m in range(16):
                        wqt = ws.tile([128, 8, 128], dt.float32r, tag="wqt")
                        nc.sync.dma_start(wqt[:], wq[m].rearrange(
                            "p (k j) -> p k j", k=8))
                        if m == 0:
                            for k in range(8):
                                nc.sync.dma_start(xt[:, k, :], xT[:, k, :])
                            nc.sync.dma_start(bqk_sb[:], bqk[:])
                            nc.sync.dma_start(bv_sb[:], bv[:])
                            nc.sync.dma_start(onesr[:], ones512[:])
                            nc.sync.dma_start(ones16_sb[:], ones16[:])
                        ps = ps1.tile([128, RPC], dt.float32, tag="psqk")
                        for k in range(8):
                            nc.tensor.matmul(ps[:], wqt[:, k, :], xt[:, k, :],
                                             start=(k == 0), stop=False)
                        nc.tensor.matmul(ps[:], bqk_sb[:, 128 * m:128 * (m + 1)],
                                         onesr[:, 0:RPC], start=False, stop=True)
                        # evict with bias + stride-16 shuffle into Q^T / K^T
                        # free index = 2048h + 16rh + (gp+par): one strided AP
                        # covers all 4 heads (source free r = 128h + rh aligns)
                        dest = qt_all if m < 8 else kt_all
                        gp = 2 * (m % 8)
                        for par in range(2):
                            nc.scalar.activation(
                                dest[:, gp + par:HPC * T:16],
                                ps[64 * par:64 * par + 64, :],
                                AF.Copy, scale=1.0)

                    # hoisted S/exp for (j=3, h=0,1): keeps ACT busy during V
                    hoisted = {(0, sp): emit_sexp(0, 3, sp) for sp in range(8)}
                    hoisted.update({(1, sp): emit_sexp(1, 3, sp) for sp in range(3)})

                    # --- V in natural orientation -> DRAM scratch ---
                    # (virtual-time delay: let the wq stream own DMA bandwidth
                    # so attention can start as early as possible)
                    tc.tile_set_cur_wait(0.024)
                    for jv in range(2):
                        wvt = ws.tile([128, 8, 512], dt.float32r, tag="wvt",
                                      bufs=1, name=f"wvt{jv}")
                        for kh in range(2):
                            nc.sync.dma_start(
                                wvt[:, 4 * kh:4 * kh + 4, :],
                                wv[jv, :, 2048 * kh:2048 * (kh + 1)].rearrange(
                                    "p (k j) -> p k j", k=4))
                        for h in range(HPC):
                            ps = ps1.tile([128, 512], dt.float32, tag="psv", bufs=2)
                            for k in range(8):
                                nc.tensor.matmul(
                                    ps[:], xt[:, k, 128 * h:128 * (h + 1)],
                                    wvt[:, k, :], start=(k == 0), stop=False)
                            nc.tensor.matmul(ps[:], onesr[:, 0:128],
                                             bv_sb[:, 512 * jv:512 * (jv + 1)],
                                             start=False, stop=True)
                            vsb = ws.tile([128, 512], dt.float32r, tag="vsb",
                                          bufs=1)
                            nc.vector.tensor_copy(vsb[:], ps[:])
                            nc.sync.dma_start(
                                vscr[h][:, 512 * jv:512 * (jv + 1)], vsb[:])

                nc.sync.dma_start(wp_sb[:], wp.rearrange("p (t c) -> p t c", t=2))
                nc.sync.dma_start(bp_sb[:], bp[:])
                tc.tile_set_cur_wait(0.0)

                # --- gather V natural [s, d] + ones cols (one DMA per head:
                # src AP [[1024,8],[64,16],[8192,16],[1,64]] over the flat
                # scratch; dest free dims (i:65-stride, d)) ---
                for h in range(HPC):
                    src_ap = vscr[h][:].rearrange(
                        "(i r) (g d) -> (r g) i d", r=8, d=64)
                    dst_ap = vn[h][:].rearrange("p (i e) -> p i e", e=65)[:, :, 0:64]
                    nc.sync.dma_start(dst_ap, src_ap)
                    nc.sync.dma_start(vn[h][:, 64:16 * 65:65], ones16_sb[:])

                # ===== Phase 2+3: attention (j desc) + fused projection =====
                with tc.tile_pool(name="misc", bufs=2) as mp, \
                     tc.tile_pool(name="osb", bufs=3) as osbp, \
                     tc.tile_pool(name="psy", bufs=2, space="PSUM") as psy, \
                     tc.tile_pool(name="ps3", bufs=2, space="PSUM") as ps3:

                    def emit_pv(h, sp, pt, yps, n_st):
                        for half in range(2):
                            i = 2 * sp + half
                            nc.tensor.matmul(
                                yps[:], vn[h][:, 65 * i:65 * i + 65],
                                pt[:, 512 * half:512 * (half + 1)],
                                start=(i == 0), stop=(i == n_st - 1))

                    def make_norm(h, j, yps):
                        def norm():
                            den = mp.tile([1, 512], dt.float32r, tag="den",
                                          name=f"den{h}{j}")
                            nc.vector.tensor_copy(den[:], yps[64:65, :])
                            rec = mp.tile([1, 512], dt.float32r, tag="rec",
                                          name=f"rec{h}{j}")
                            with nc.allow_low_precision(reason="softmax recip"):
                                nc.vector.reciprocal(rec[:], den[:])
                            bcp = ps3.tile([128, 512], dt.float32, tag="px",
                                           name=f"bcp{h}{j}")[0:64, :]
                            nc.tensor.matmul(bcp[:], onesr[:, 0:64], rec[:],
                                             start=True, stop=True)
                            bcs = mp.tile([64, 512], dt.float32, tag="bcs",
                                          name=f"bcs{h}{j}")
                            nc.vector.tensor_copy(bcs[:], bcp[:])
                            nc.vector.tensor_tensor(
                                yt[h // 2][64 * (h % 2):64 * (h % 2) + 64,
                                           512 * j:512 * (j + 1)],
                                yps[0:64, :], bcs[:], op=OP.mult)
                        return norm

                    def make_proj_one(j, tt, cc, last=False):
                        def proj():
                            if last and (tt + cc) % 2 == 0:
                                pw = ps2.tile([128, 1024], dt.float32,
                                              tag="spsum", name=f"pow{tt}{cc}")
                                po = pw[:, 0:512]
                            else:
                                po = ps3.tile([128, 512], dt.float32,
                                              tag="px", name=f"po{tt}{cc}")
                            nc.tensor.matmul(
                                po[:], yt[0][:, 128 * tt:128 * (tt + 1)],
                                wp_sb[:, 0, 512 * cc:512 * (cc + 1)],
                                start=True, stop=False)
                            nc.tensor.matmul(
                                po[:], yt[1][:, 128 * tt:128 * (tt + 1)],
                                wp_sb[:, 1, 512 * cc:512 * (cc + 1)],
                                start=False, stop=True)
                            ot = osbp.tile([128, 512], dt.float32,
                                           tag="ot", name=f"ot{tt}{cc}")
                            nc.vector.tensor_tensor(
                                ot[:], po[:],
                                bp_sb[:, 512 * cc:512 * (cc + 1)], op=OP.add)
                            nc.sync.dma_start(
                                out_d[128 * tt:128 * (tt + 1),
                                      512 * cc:512 * (cc + 1)], ot[:])
                        return proj

                    pending = []   # small deferred closures, drip-fed
                    for jx, j in enumerate([3, 2, 1, 0]):
                        for h in range(HPC):
                            n_st = 4 * j + 4        # s-tiles needed (causal)
                            yps = psy.tile([65, 512], dt.float32, tag="ypsum",
                                           name=f"yps{h}{j}")
                            prev = None
                            for sp in range(n_st // 2):
                                if jx == 0 and (h, sp) in hoisted:
                                    pt = hoisted[(h, sp)]
                                else:
                                    pt = emit_sexp(h, j, sp)
                                if prev is not None:
                                    psp, pt_prev = prev
                                    emit_pv(h, psp, pt_prev, yps, n_st)
                                if sp >= min(2, n_st // 2 - 1) and pending:
                                    pending.pop(0)()
                                prev = (sp, pt)
                            psp, pt_prev = prev
                            emit_pv(h, psp, pt_prev, yps, n_st)
                            pending.append(make_norm(h, j, yps))
                        for tt in range(4 * j, 4 * j + 4):
                            for cc in range(2):
                                pending.append(
                                    make_proj_one(j, tt, cc, last=(jx == 3)))
                    for fn in pending:
                        fn()
    nc.compile()
    return nc


_NC_CACHE = None


def _get_program():
    global _NC_CACHE
    if _NC_CACHE is None:
        _NC_CACHE = build_program()
    return _NC_CACHE


def _prep_core_inputs(x, Wqkv, bqkv, Wproj, bproj):
    """Build the 8 per-core input dicts (host-side shard + layout prep)."""
    x = np.asarray(x, dtype=np.float32)
    Wqkv = np.ascontiguousarray(np.asarray(Wqkv, dtype=np.float32))
    bqkv = np.asarray(bqkv, dtype=np.float32)
    Wproj = np.asarray(Wproj, dtype=np.float32)
    bproj = np.asarray(bproj, dtype=np.float32)

    wq_np = np.ascontiguousarray(
        Wqkv[:, :2048].reshape(8, 128, 16, 128).transpose(2, 1, 0, 3)
        .reshape(16, 128, 8 * 128))
    wv_np = np.ascontiguousarray(
        Wqkv[:, 2048:].reshape(8, 128, 2, 512).transpose(2, 1, 0, 3)
        .reshape(2, 128, 8 * 512))
    bqk_np = np.ascontiguousarray(bqkv[:2048].reshape(1, 2048))
    bv_np = np.ascontiguousarray(bqkv[2048:].reshape(1, 1024))
    ones512_np = np.ones((1, 512), np.float32)
    ones16_np = np.ones((128, 16), np.float32)
    bp_rep = np.broadcast_to(bproj, (128, C)).copy()
    bp_zero = np.zeros((128, C), np.float32)

    in_maps = []
    for c in range(N_CORES):
        b, q = divmod(c, 4)
        xT_np = np.ascontiguousarray(
            x[b, RPC * q:RPC * (q + 1), :].reshape(RPC, 8, 128)
            .transpose(2, 1, 0))
        wp_np = np.ascontiguousarray(
            Wproj[256 * q:256 * (q + 1), :].reshape(2, 128, 1024)
            .transpose(1, 0, 2).reshape(128, 2048))
        in_maps.append({
            "xT": xT_np, "wq": wq_np, "wv": wv_np, "bqk": bqk_np,
            "bv": bv_np, "wp": wp_np,
            "bp": bp_rep if q == 0 else bp_zero,
            "ones512": ones512_np, "ones16": ones16_np,
        })
    return in_maps


def kernel(x, Wqkv, bqkv, Wproj, bproj):
    nc = _get_program()
    in_maps = _prep_core_inputs(x, Wqkv, bqkv, Wproj, bproj)
    res = run_bass_kernel_spmd(nc, in_maps, list(range(N_CORES)))
    out = np.zeros((B, T, C), dtype=np.float32)
    for c in range(N_CORES):
        out[c // 4] += res.results[c]["out"]
    return out



# revision 12
# speedup vs baseline: 1.1656x; 1.1656x over previous
"""Causal self-attention (dense transformer) on 8 trn2 NeuronCores.

Reference semantics (note the headless reshape):
  x_proj = x @ Wqkv + bqkv                     # [B, T, 3C]
  q = x_proj[:, :, :C].reshape(B, H, T, hd)    # direct reshape, no transpose!
Because of the direct reshape, head h consumes the contiguous row block
x_proj[b, h*128:(h+1)*128, :] reinterpreted as [T, hd].  So sharding by
(batch, head-group) makes QKV projection + attention fully core-local;
only the output projection is a row-parallel partial sum, reduced on host.

Shapes (hardcoded): B=2, T=2048, C=1024, n_head=16, hd=64, 8 cores.
Core c: batch b=c//4, quarter q=c%4 -> x rows [512q, 512q+512), heads 4q..4q+3.

v2 design notes (cost model: matmul cost = out-free-size cycles, K and
partition count free; bf16 1 cyc/row at any width):
- everything bf16 on SBUF/DRAM (PSUM fp32); halves DMA vs f32r.
- Q,K computed transposed (as v1) with bias folded into the ACT eviction
  (per-partition bias AP, func=Identity) - no bias matmuls.
- V computed natural, evicted bf16, re-partitioned to [s, hd] tiles via an
  SBUF->SBUF gather DMA (no DRAM bounce).  V bias is NOT added: with
  P@[V|1] giving unnormalized y and the denominator, y/den + bv equals the
  exact result, so bv folds into the final per-partition eviction add.
- S^T tiles [s=128, q<=512] in psum; diagonal-band tiles narrowed to the
  causally needed q-width; exp on ACT (scale=1/8) straight from 2-bank
  psum into bf16 pt tiles; only the exact-diagonal 128x128 sub-block needs
  an affine_select mask (uniform f>=p predicate).
- P@V in natural-Y orientation: out yn[q=128, 65] accumulates over s-tiles
  with lhsT = pt 128-col slices -> 65 cycles per (s-tile, q-block) instead
  of 512 (the 8x win).  Above-diagonal sub-blocks are skipped entirely.
- normalization: batched reciprocal of the 4 q-block denominators, DVE
  per-partition scale eviction to bf16, PE-transpose (identity matmul) to
  Y^T, evicted with the V-bias per-partition add.
- output projection bf16 from Y^T tiles; partials DMA'd bf16; host does
  the 4-way reduction + bproj in fp32.
"""

import os

import numpy as np
import ml_dtypes

os.environ.setdefault("NEURON_RT_RESET_CORES", "1")

import concourse.bacc as bacc
import concourse.mybir as mybir
import concourse.tile as tile
from concourse import masks
from concourse.bass_utils import run_bass_kernel_spmd

dt = mybir.dt
AF = mybir.ActivationFunctionType
OP = mybir.AluOpType
BF16 = np.dtype(ml_dtypes.bfloat16)

B, T, C = 2, 2048, 1024
NH, HD = 16, 64
N_CORES = 8
HPC = 4          # heads per core
RPC = 512        # x rows per core
SCALE = 1.0 / 8.0   # 1/sqrt(hd), folded into the exp activation


def _pair_members(j, u):
    """s-tile pair u of query block j: list of (i, qoff, psum_off, width).

    Widths stay >= 256 so f32r matmuls keep 1 cyc/row; the last diagonal
    tile (needs only 128 q) is computed 256 wide, its dead q-block skipped
    in P@V (above the diagonal, never read).
    """
    if u < 2 * j:
        return [(2 * u, 0, 0, 512), (2 * u + 1, 0, 512, 512)]
    if u == 2 * j:
        return [(4 * j, 0, 0, 512), (4 * j + 1, 128, 512, 384)]
    return [(4 * j + 2, 256, 0, 256), (4 * j + 3, 256, 256, 256)]


def build_program():
    nc = bacc.Bacc("TRN2", target_bir_lowering=False, debug=False,
                   num_devices=N_CORES)

    # ---- DRAM I/O (per core) ----
    xT = nc.dram_tensor("xT", [128, 8, RPC], dt.float32r, kind="ExternalInput")
    wq = nc.dram_tensor("wq", [16, 128, 8 * 128], dt.float32r, kind="ExternalInput")
    wv = nc.dram_tensor("wv", [2, 128, 8 * 512], dt.float32r, kind="ExternalInput")
    bqk = nc.dram_tensor("bqk", [128, 16], dt.float32, kind="ExternalInput")
    bvn = nc.dram_tensor("bvn", [1, 1024], dt.float32r, kind="ExternalInput")
    wp = nc.dram_tensor("wp", [128, 2 * 1024], dt.bfloat16, kind="ExternalInput")
    out_d = nc.dram_tensor("out", [T, C], dt.bfloat16, kind="ExternalOutput")

    with tile.TileContext(nc) as tc:
        with tc.tile_pool(name="persist", bufs=1) as pp, \
             tc.tile_pool(name="drampool", bufs=1, space="DRAM") as dp:
            vscr = [dp.tile([128, 2, 512], dt.bfloat16, tag=f"vscr{h}",
                            name=f"vscr{h}") for h in range(HPC)]
            xt = pp.tile([128, 8, RPC], dt.float32r, tag="xt")
            bqk_sb = pp.tile([128, 16], dt.float32, tag="bqk")
            bvn_sb = pp.tile([1, 1024], dt.float32r, tag="bvn")
            onesr = pp.tile([1, 128], dt.float32, tag="onesr")
            wp_sb = pp.tile([128, 2, 1024], dt.bfloat16, tag="wp")
            ident = pp.tile([128, 128], dt.bfloat16, tag="ident")

            qt_all = pp.tile([64, HPC * T], dt.float32r, tag="qt_all")
            kt_all = pp.tile([64, HPC * T], dt.float32r, tag="kt_all")
            vn = pp.tile([128, HPC, 16, 65], dt.bfloat16, tag="vn")
            yt = pp.tile([128, 2, T], dt.bfloat16, tag="yt")

            masks.make_identity(nc, ident[:])
            nc.gpsimd.memset(onesr[:], 1.0)
            nc.gpsimd.memset(vn[:, :, :, 64], 1.0)

            # attention pools opened early: first S/exp pairs are hoisted
            # into phase 1 so ACT warms up while PE finishes the V part
            with tc.tile_pool(name="ptpool", bufs=12) as ptp, \
                 tc.tile_pool(name="ps2", bufs=2, space="PSUM") as ps2:

                def emit_sexp(h, j, u):
                    """S^T matmuls for pair u + exp + exact-diagonal mask."""
                    mem = _pair_members(j, u)
                    tot = mem[-1][2] + mem[-1][3]
                    # pair B's two members share psum bank 0: one accumulation
                    # group (start on first, stop on second; first-touch of a
                    # pending-zero region overwrites, so no accumulation mix)
                    same_bank = mem[-1][2] + mem[-1][3] <= 512
                    ssp = ps2.tile([128, 1024], dt.float32, tag="ssp",
                                   name=f"ssp{h}{j}{u}")
                    for mi, (i, qoff, off, w) in enumerate(mem):
                        nc.tensor.matmul(
                            ssp[:, off:off + w],
                            kt_all[:, T * h + 128 * i:T * h + 128 * (i + 1)],
                            qt_all[:, T * h + 512 * j + qoff:
                                   T * h + 512 * j + qoff + w],
                            start=(not same_bank) or mi == 0,
                            stop=(not same_bank) or mi == len(mem) - 1)
                    pt = ptp.tile([128, 1024], dt.bfloat16, tag="pt",
                                  name=f"pt{h}{j}{u}")
                    nc.scalar.activation(pt[:, 0:tot], ssp[:, 0:tot],
                                         AF.Exp, scale=SCALE)
                    if u >= 2 * j:  # diagonal band: mask the exact diagonal
                        for (i, qoff, off, w) in mem:
                            dcol = off + 128 * (i - 4 * j) - qoff
                            nc.gpsimd.affine_select(
                                out=pt[:, dcol:dcol + 128],
                                in_=pt[:, dcol:dcol + 128],
                                compare_op=OP.is_ge, fill=0.0,
                                base=0, channel_multiplier=-1,
                                pattern=[[1, 128]])
                    return pt, mem

                # ================= Phase 1: QKV projection =================
                with tc.tile_pool(name="wstream", bufs=2) as ws, \
                     tc.tile_pool(name="vstage", bufs=3) as vst, \
                     tc.tile_pool(name="ps1", bufs=2, space="PSUM") as ps1:
                    # --- Q,K in transposed orientation: x_proj^T m-tiles ---
                    for m in range(16):
                        if m == 0:
                            nc.sync.dma_start(xt[:, 0, :], xT[:, 0, :])
                        wqt = ws.tile([128, 8, 128], dt.float32r, tag="wqt")
                        nc.sync.dma_start(wqt[:], wq[m].rearrange(
                            "p (k j) -> p k j", k=8))
                        if m == 0:
                            nc.sync.dma_start(xt[:, 1:4, :], xT[:, 1:4, :])
                            nc.sync.dma_start(xt[:, 4:8, :], xT[:, 4:8, :])
                            nc.sync.dma_start(bqk_sb[:], bqk[:])
                            nc.sync.dma_start(bvn_sb[:], bvn[:])
                        ps = ps1.tile([128, RPC], dt.float32, tag="psqk")
                        for k in range(8):
                            nc.tensor.matmul(ps[:], wqt[:, k, :], xt[:, k, :],
                                             start=(k == 0), stop=(k == 7))
                        # evict with bias + stride-16 shuffle into Q^T / K^T
                        # free index = 2048h + 16rh + (gp+par): one strided AP
                        dest = qt_all if m < 8 else kt_all
                        gp = 2 * (m % 8)
                        for par in range(2):
                            nc.scalar.activation(
                                dest[:, gp + par:HPC * T:16],
                                ps[64 * par:64 * par + 64, :],
                                AF.Identity, scale=1.0,
                                bias=bqk_sb[64 * par:64 * par + 64, m:m + 1])

                    # --- V in natural orientation, h-outer so each head's
                    # [s, hd] gather DMA can start as early as possible ---
                    wvt = [ws.tile([128, 8, 512], dt.float32r, tag="wvt",
                                   name=f"wvt{jv}") for jv in range(2)]
                    for jv in range(2):
                        nc.sync.dma_start(wvt[jv][:], wv[jv].rearrange(
                            "p (k j) -> p k j", k=8))
                    nc.sync.dma_start(wp_sb[:], wp.rearrange(
                        "p (t c) -> p t c", t=2))

                    hoisted = {}
                    for h in range(HPC):
                        vsb = vst.tile([128, 2, 512], dt.bfloat16, tag="vsb",
                                       name=f"vsb{h}")
                        for jv in range(2):
                            ps = ps1.tile([128, 512], dt.float32, tag="psv")
                            for k in range(8):
                                nc.tensor.matmul(
                                    ps[:], xt[:, k, 128 * h:128 * (h + 1)],
                                    wvt[jv][:, k, :],
                                    start=(k == 0), stop=False)
                            nc.tensor.matmul(
                                ps[:], onesr[:].bitcast(dt.float32r),
                                bvn_sb[:, 512 * jv:512 * (jv + 1)],
                                start=False, stop=True)
                            nc.vector.tensor_copy(vsb[:, jv, :], ps[:])
                        # re-partition to [s, hd] tiles via a DRAM bounce
                        # (the gather's source AP mixes partition bits into
                        # free dims, which SBUF addressing can't express):
                        # s = 16*rr + g, g = 8*jv + g2 -> partition (r jv g2)
                        nc.sync.dma_start(vscr[h][:], vsb[:])
                        src = vscr[h][:].rearrange(
                            "(i r) jv (g2 d) -> (r jv g2) i d", r=8, d=64)
                        nc.sync.dma_start(vn[:, h, :, 0:64], src)
                        # hoisted S/exp for (h=0/1, j=3): keeps ACT busy
                        if h == 2:
                            for u in range(8):
                                hoisted[(0, u)] = emit_sexp(0, 3, u)
                        elif h == 3:
                            for u in range(3):
                                hoisted[(1, u)] = emit_sexp(1, 3, u)

                # ===== Phase 2+3: attention (j desc) + fused projection =====
                with tc.tile_pool(name="misc", bufs=3) as mp, \
                     tc.tile_pool(name="osb", bufs=3) as osbp, \
                     tc.tile_pool(name="psy", bufs=2, space="PSUM") as psy, \
                     tc.tile_pool(name="pst", bufs=1, space="PSUM") as pstp, \
                     tc.tile_pool(name="ps3", bufs=1, space="PSUM") as ps3:

                    def emit_pv(h, j, pt, mem, yn, first, last):
                        # yn's 4 q-block accumulators share one psum bank:
                        # ONE group for the whole (h, j) unit — start marks
                        # the bank pending-zero, each slice's first touch
                        # overwrites, stop on the final matmul only.
                        for mi, (i, qoff, off, w) in enumerate(mem):
                            qbs = list(range(max(qoff // 128, i - 4 * j), 4))
                            for qi, qb in enumerate(qbs):
                                col = off + 128 * qb - qoff
                                nc.tensor.matmul(
                                    yn[:, qb, :], pt[:, col:col + 128],
                                    vn[:, h, i, :],
                                    start=first and mi == 0 and qi == 0,
                                    stop=(last and mi == len(mem) - 1
                                          and qi == len(qbs) - 1))

                    def make_norm(h, j, yn):
                        def norm():
                            rin = mp.tile([128, 4], dt.float32, tag="rin",
                                          name=f"rin{h}{j}")
                            nc.vector.tensor_copy(rin[:], yn[:, :, 64])
                            rcp = mp.tile([128, 4], dt.float32, tag="rcp",
                                          name=f"rcp{h}{j}")
                            with nc.allow_low_precision(reason="softmax recip"):
                                nc.vector.reciprocal(rcp[:], rin[:])
                            yb = mp.tile([128, 4, 64], dt.bfloat16, tag="yb",
                                         name=f"yb{h}{j}")
                            for qb in range(4):
                                nc.vector.tensor_scalar(
                                    out=yb[:, qb, :], in0=yn[:, qb, 0:64],
                                    scalar1=rcp[:, qb:qb + 1], scalar2=None,
                                    op0=OP.mult)
                            ytr = pstp.tile([64, 512], dt.bfloat16, tag="ytr",
                                            name=f"ytr{h}{j}")
                            for qb in range(4):
                                nc.tensor.transpose(
                                    ytr[:, 128 * qb:128 * (qb + 1)],
                                    yb[:, qb, :], ident[:])
                            nc.vector.tensor_copy(
                                yt[64 * (h % 2):64 * (h % 2) + 64,
                                   h // 2, 512 * j:512 * (j + 1)], ytr[:])
                        return norm

                    def make_proj_one(j, tt, cc, last=False):
                        def proj():
                            if last:
                                po = ps2.tile([128, 1024], dt.float32,
                                              tag="ssp",
                                              name=f"pol{tt}{cc}")[:, 0:512]
                            else:
                                po = ps3.tile([128, 512], dt.float32,
                                              tag="px", name=f"po{tt}{cc}")
                            for p in range(2):
                                nc.tensor.matmul(
                                    po[:], yt[:, p, 128 * tt:128 * (tt + 1)],
                                    wp_sb[:, p, 512 * cc:512 * (cc + 1)],
                                    start=(p == 0), stop=(p == 1))
                            ot = osbp.tile([128, 512], dt.bfloat16,
                                           tag="ot", name=f"ot{tt}{cc}")
                            nc.vector.tensor_copy(ot[:], po[:])
                            nc.sync.dma_start(
                                out_d[128 * tt:128 * (tt + 1),
                                      512 * cc:512 * (cc + 1)], ot[:])
                        return proj

                    pending = []   # small deferred closures, drip-fed
                    for jx, j in enumerate([3, 2, 1, 0]):
                        for h in range(HPC):
                            yn = psy.tile([128, 4, 65], dt.float32, tag="yn",
                                          name=f"yn{h}{j}")
                            n_u = 2 * j + 2
                            prev = None
                            for u in range(n_u):
                                if jx == 0 and (h, u) in hoisted:
                                    pt, mem = hoisted[(h, u)]
                                else:
                                    pt, mem = emit_sexp(h, j, u)
                                if prev is not None:
                                    emit_pv(h, j, prev[0], prev[1], yn,
                                            first=(u == 1), last=False)
                                if u >= 1 and pending:
                                    pending.pop(0)()
                                prev = (pt, mem)
                            emit_pv(h, j, prev[0], prev[1], yn,
                                    first=(n_u == 1), last=True)
                            pending.append(make_norm(h, j, yn))
                        for tt in range(4 * j, 4 * j + 4):
                            for cc in range(2):
                                pending.append(
                                    make_proj_one(j, tt, cc, last=(jx == 3)))
                    for fn in pending:
                        fn()
    nc.compile()
    return nc


_NC_CACHE = None


def _get_program():
    global _NC_CACHE
    if _NC_CACHE is None:
        _NC_CACHE = build_program()
    return _NC_CACHE


def _prep_core_inputs(x, Wqkv, bqkv, Wproj, bproj):
    """Build the 8 per-core input dicts (host-side shard + layout prep)."""
    x = np.asarray(x, dtype=np.float32)
    Wqkv = np.ascontiguousarray(np.asarray(Wqkv, dtype=np.float32))
    bqkv = np.asarray(bqkv, dtype=np.float32)
    Wproj = np.asarray(Wproj, dtype=np.float32)

    wq_np = np.ascontiguousarray(
        Wqkv[:, :2048].reshape(8, 128, 16, 128).transpose(2, 1, 0, 3)
        .reshape(16, 128, 8 * 128))
    wv_np = np.ascontiguousarray(
        Wqkv[:, 2048:].reshape(8, 128, 2, 512).transpose(2, 1, 0, 3)
        .reshape(2, 128, 8 * 512))
    bqk_np = np.ascontiguousarray(bqkv[:2048].reshape(16, 128).T)
    bvn_np = np.ascontiguousarray(bqkv[2048:].reshape(1, 1024))

    in_maps = []
    for c in range(N_CORES):
        b, q = divmod(c, 4)
        xT_np = np.ascontiguousarray(
            x[b, RPC * q:RPC * (q + 1), :].reshape(RPC, 8, 128)
            .transpose(2, 1, 0))
        wp_np = np.ascontiguousarray(
            Wproj[256 * q:256 * (q + 1), :].reshape(2, 128, 1024)
            .transpose(1, 0, 2).reshape(128, 2048)).astype(BF16)
        in_maps.append({
            "xT": xT_np, "wq": wq_np, "wv": wv_np, "bqk": bqk_np,
            "bvn": bvn_np, "wp": wp_np,
        })
    return in_maps


def kernel(x, Wqkv, bqkv, Wproj, bproj):
    nc = _get_program()
    in_maps = _prep_core_inputs(x, Wqkv, bqkv, Wproj, bproj)
    res = run_bass_kernel_spmd(nc, in_maps, list(range(N_CORES)))
    out = np.zeros((B, T, C), dtype=np.float32)
    for c in range(N_CORES):
        out[c // 4] += res.results[c]["out"].astype(np.float32)
    out += np.asarray(bproj, dtype=np.float32)
    return out


# revision 27
# speedup vs baseline: 1.3994x; 1.2006x over previous
"""Causal self-attention (dense transformer) on 8 trn2 NeuronCores.

Reference semantics (note the headless reshape):
  x_proj = x @ Wqkv + bqkv                     # [B, T, 3C]
  q = x_proj[:, :, :C].reshape(B, H, T, hd)    # direct reshape, no transpose!
Because of the direct reshape, head h consumes the contiguous row block
x_proj[b, h*128:(h+1)*128, :] reinterpreted as [T, hd].  So sharding by
(batch, head-group) makes QKV projection + attention fully core-local;
only the output projection is a row-parallel partial sum, reduced on host.

Shapes (hardcoded): B=2, T=2048, C=1024, n_head=16, hd=64, 8 cores.
Core c: batch b=c//4, quarter q=c%4 -> x rows [512q, 512q+512), heads 4q..4q+3.

v4 design notes (cost model: matmul cost = out-free-size x cycles/row; K and
partition count are free; bf16 is 1 cyc/row at ANY width, f32r only >=256):
- bf16 everywhere (PSUM fp32).  Total error ~4e-3 vs the 2e-2 gate.
- QK projection is HEAD-BLOCKED (per (head, m-group-of-4) unit, 32 matmuls
  of 128-wide bf16 into one psum bank / one accumulation group), and the
  whole schedule is SOFTWARE-PIPELINED BY HEAD: head h's attention
  (S -> exp -> P@V) is emitted interleaved with head h+1's projection
  units, so the ACT engine (exp, the #2 load at ~70us) starts ~18us in and
  never waits for the full projection.
- order: QK(h0) | V(all heads, + S/exp of h0 woven) | QK(h1)+attn(h0) |
  QK(h2)+attn(h1) | QK(h3)+attn(h2) | attn(h3) | remaining query blocks
  j=2,1,0 with drip-fed normalization + projection closures.
- PSUM bank lifetimes telescope: QK pool (2) and V pool (2) + S pairs (4)
  early; V pool is then traded for the Y^T-transpose bank and the j=3 yn
  bank; the QK pool is traded for the j<=2 yn pool; the first yn bank is
  traded for the projection bank.  Always exactly 8 banks.
- eviction of q^T/k^T: DVE tensor_tensor add with a stride-0-broadcast
  per-(partition, m) bias AP; stride-16 shuffled dest APs.
- V natural with ones-row bias matmul (the V bias varies with s%16 via the
  headless reshape, so it must be added on x_proj columns), bf16-evicted,
  re-partitioned to [s, hd] tiles via a DRAM bounce.
- S^T tiles at causally-minimal widths (512/384/256/128); exp straight off
  2-bank psum with scale=1/8; only the exact-diagonal 128x128 block is
  masked, out-of-place into a small ptd tile (mask off the critical path).
- P@V natural-Y: yn[q=128, 4, 65] accumulates over s-tiles with lhsT =
  pt/ptd 128-col slices: 65 cycles per (s-tile, q-block) instead of 512.
  Above-diagonal blocks skipped; ones-column gives the denominator.
- normalization: batched reciprocal, one broadcast tensor_tensor eviction,
  PE-transpose (identity matmul) to Y^T.
- projection: 512-wide dripped units; the last query block runs 1024-wide
  units on the freed S-psum banks with ACT/DVE alternating evictions.
- host: 4-way partial reduction + bproj in fp32.
"""

import os

import numpy as np
import ml_dtypes

os.environ.setdefault("NEURON_RT_RESET_CORES", "1")

import concourse.bacc as bacc
import concourse.mybir as mybir
import concourse.tile as tile
from concourse import masks
from concourse.bass_utils import run_bass_kernel_spmd

dt = mybir.dt
AF = mybir.ActivationFunctionType
OP = mybir.AluOpType
BF16 = np.dtype(ml_dtypes.bfloat16)

B, T, C = 2, 2048, 1024
NH, HD = 16, 64
N_CORES = 8
HPC = 4          # heads per core
RPC = 512        # x rows per core
SCALE = 1.0 / 8.0   # 1/sqrt(hd), folded into the exp activation


def _pair_members(j, u):
    """s-tile pair u of query block j: list of (i, qoff, psum_off, width)."""
    if u < 2 * j:
        return [(2 * u, 0, 0, 512), (2 * u + 1, 0, 512, 512)]
    if u == 2 * j:
        return [(4 * j, 0, 0, 512), (4 * j + 1, 128, 512, 384)]
    return [(4 * j + 2, 256, 0, 256), (4 * j + 3, 384, 256, 128)]


def build_program():
    nc = bacc.Bacc("TRN2", target_bir_lowering=False, debug=False,
                   num_devices=N_CORES)

    # ---- DRAM I/O (per core) ----
    xT = nc.dram_tensor("xT", [128, 8, RPC], dt.bfloat16, kind="ExternalInput")
    wq = nc.dram_tensor("wq", [16, 128, 8 * 128], dt.bfloat16, kind="ExternalInput")
    wv = nc.dram_tensor("wv", [2, 128, 8 * 512], dt.bfloat16, kind="ExternalInput")
    bqk = nc.dram_tensor("bqk", [128, 16], dt.float32, kind="ExternalInput")
    bvn = nc.dram_tensor("bvn", [1, 1024], dt.bfloat16, kind="ExternalInput")
    wp = nc.dram_tensor("wp", [128, 2 * 1024], dt.bfloat16, kind="ExternalInput")
    out_d = nc.dram_tensor("out", [T, C], dt.bfloat16, kind="ExternalOutput")

    with tile.TileContext(nc) as tc:
        with tc.tile_pool(name="persist", bufs=1) as pp, \
             tc.tile_pool(name="drampool", bufs=1, space="DRAM") as dp:
            vscr = [dp.tile([128, 2, 512], dt.bfloat16, tag=f"vscr{h}",
                            name=f"vscr{h}") for h in range(HPC)]
            xt = pp.tile([128, 8, RPC], dt.bfloat16, tag="xt")
            bqk_sb = pp.tile([128, 16], dt.float32, tag="bqk")
            bvn_sb = pp.tile([1, 1024], dt.bfloat16, tag="bvn")
            onesr = pp.tile([1, 128], dt.bfloat16, tag="onesr")
            wp_sb = pp.tile([128, 2, 1024], dt.bfloat16, tag="wp")
            ident = pp.tile([128, 128], dt.bfloat16, tag="ident")

            qt_all = pp.tile([64, HPC * T], dt.bfloat16, tag="qt_all")
            kt_all = pp.tile([64, HPC * T], dt.bfloat16, tag="kt_all")
            vn = pp.tile([128, HPC, 16, 65], dt.bfloat16, tag="vn")
            yt = pp.tile([128, 2, T], dt.bfloat16, tag="yt")

            masks.make_identity(nc, ident[:])
            nc.gpsimd.memset(onesr[:], 1.0)
            nc.gpsimd.memset(vn[:, :, :, 64], 1.0)

            # p-state warmup: the PE clock ramp keys off the FIRST busy
            # time; burn it on the identity tile while input DMAs land
            with tc.tile_pool(name="warm", bufs=1, space="PSUM") as pw:
                wps = pw.tile([128, 128], dt.float32, tag="w")
                for _ in range(18):
                    nc.tensor.matmul(wps[:], ident[:], ident[:],
                                     start=True, stop=True)

            with tc.tile_pool(name="ptpool", bufs=26) as ptp, \
                 tc.tile_pool(name="ptdpool", bufs=14) as ptdp, \
                 tc.tile_pool(name="misc", bufs=3) as mp, \
                 tc.tile_pool(name="osb", bufs=2) as osbp, \
                 tc.tile_pool(name="wstream", bufs=1) as ws, \
                 tc.tile_pool(name="vstage", bufs=3) as vst, \
                 tc.tile_pool(name="ps2", bufs=2, space="PSUM") as ps2, \
                 tc.tile_pool(name="psA", bufs=2, space="PSUM") as psA, \
                 tc.tile_pool(name="psB", bufs=2, space="PSUM") as psB:

                # bank-reuse view allocators: psA's 2 banks serve the QK
                # units and later the j<=2 yn accumulators; psB's 2 banks
                # serve the V units and later the j=3 yn / Y^T-transpose /
                # projection tiles.  Always 8 banks total.
                def alloc_yn(h, j):
                    if j == 3 and h < 3:
                        tl = psB.tile([128, 512], dt.float32, tag="pvB",
                                      bufs=1, name=f"ynb{h}{j}")
                        return tl[:].rearrange("p (a b) -> p a b",
                                               a=4)[:, :, 0:65]
                    tl = psA.tile([128, 4, 128], dt.float32, tag="psqk",
                                  name=f"yna{h}{j}")
                    return tl[:, :, 0:65]

                def alloc_ytr(h, j):
                    tl = psB.tile([128, 512], dt.float32, tag="pvA",
                                  bufs=1, name=f"ytrt{h}{j}")
                    return tl[:].bitcast(dt.bfloat16)[0:64, 0:512]

                def alloc_po(tt, cc):
                    tl = psB.tile([128, 512], dt.float32, tag="pvB",
                                  bufs=1, name=f"pot{tt}{cc}")
                    return tl[:]

                # ---------------- input DMAs ----------------
                wqall = ws.tile([128, 16, 8, 128], dt.bfloat16, tag="wqall")
                nc.sync.dma_start(xt[:, 0:2, :], xT[:, 0:2, :])
                nc.sync.dma_start(xt[:, 2:8, :], xT[:, 2:8, :])
                for c4 in range(4):
                    nc.sync.dma_start(
                        wqall[:, 4 * c4:4 * c4 + 4],
                        wq[4 * c4:4 * c4 + 4].rearrange(
                            "m p (k j) -> p m k j", k=8))
                    if c4 == 0:
                        nc.sync.dma_start(bqk_sb[:], bqk[:])
                        nc.sync.dma_start(bvn_sb[:], bvn[:])
                # V/proj weights: virtual-time delayed so the wq stream owns
                # the DMA engine while it feeds PE; wv0 lands right as the
                # first V unit needs it
                wvt = [ws.tile([128, 8, 512], dt.bfloat16, tag="wvt",
                               bufs=2, name=f"wvt{jv}") for jv in range(2)]
                for jv in range(2):
                    tc.tile_set_cur_wait(0.0145 + 0.0035 * jv)
                    nc.sync.dma_start(wvt[jv][:], wv[jv].rearrange(
                        "p (k j) -> p k j", k=8))
                tc.tile_set_cur_wait(0.021)
                nc.sync.dma_start(wp_sb[:], wp.rearrange(
                    "p (t c) -> p t c", t=2))
                tc.tile_set_cur_wait(0.0)

                # ---------------- emission helpers ----------------
                def emit_sexp(h, j, u):
                    """S^T matmuls for pair u + exp + exact-diagonal mask."""
                    mem = _pair_members(j, u)
                    tot = mem[-1][2] + mem[-1][3]
                    same_bank = tot <= 512   # pair B: one group, start/stop split
                    ssp = ps2.tile([128, 1024], dt.float32, tag="ssp",
                                   name=f"ssp{h}{j}{u}")
                    for mi, (i, qoff, off, w) in enumerate(mem):
                        nc.tensor.matmul(
                            ssp[:, off:off + w],
                            kt_all[:, T * h + 128 * i:T * h + 128 * (i + 1)],
                            qt_all[:, T * h + 512 * j + qoff:
                                   T * h + 512 * j + qoff + w],
                            start=(not same_bank) or mi == 0,
                            stop=(not same_bank) or mi == len(mem) - 1)
                    pt = ptp.tile([128, 1024], dt.bfloat16, tag="pt",
                                  name=f"pt{h}{j}{u}")
                    nc.scalar.activation(pt[:, 0:tot], ssp[:, 0:tot],
                                         AF.Exp, scale=SCALE)
                    dmap = {}
                    if u >= 2 * j:  # mask the exact diagonal, out-of-place
                        for (i, qoff, off, w) in mem:
                            dcol = off + 128 * (i - 4 * j) - qoff
                            ptd = ptdp.tile([128, 128], dt.bfloat16,
                                            tag="ptd", name=f"ptd{h}{j}{u}{i}")
                            nc.gpsimd.affine_select(
                                out=ptd[:], in_=pt[:, dcol:dcol + 128],
                                compare_op=OP.is_ge, fill=0.0,
                                base=0, channel_multiplier=-1,
                                pattern=[[1, 128]])
                            dmap[i] = ptd
                    return pt, mem, dmap

                pending = []   # deferred (norm / proj) closures, drip-fed

                def drip(n=1):
                    npop = min(n + (len(pending) > 4) + (len(pending) > 8),
                               len(pending))
                    for _ in range(npop):
                        pending.pop(0)()

                def make_norm(h, j, yn, on_act=False):
                    def norm():
                        rin = mp.tile([128, 4], dt.float32, tag="rin",
                                      name=f"rin{h}{j}")
                        nc.vector.tensor_copy(rin[:], yn[:, :, 64])
                        rcp = mp.tile([128, 4], dt.float32, tag="rcp",
                                      name=f"rcp{h}{j}")
                        with nc.allow_low_precision(reason="softmax recip"):
                            nc.vector.reciprocal(rcp[:], rin[:])
                        yb = mp.tile([128, 4, 64], dt.bfloat16, tag="yb",
                                     name=f"yb{h}{j}")
                        nc.vector.tensor_tensor(
                            yb[:], yn[:, :, 0:64],
                            rcp[:, :, None].broadcast_to([128, 4, 64]),
                            op=OP.mult)
                        ytr = alloc_ytr(h, j)
                        for qb in range(4):
                            nc.tensor.transpose(
                                ytr[:, 128 * qb:128 * (qb + 1)],
                                yb[:, qb, :], ident[:])
                        dst = yt[64 * (h % 2):64 * (h % 2) + 64,
                                 h // 2, 512 * j:512 * (j + 1)]
                        if on_act:
                            nc.scalar.copy(dst, ytr)
                        else:
                            nc.vector.tensor_copy(dst, ytr)
                    return norm

                def make_proj_one(j, tt, cc, last=False):
                    def proj():
                        if last:
                            po = ps2.tile([128, 1024], dt.float32,
                                          tag="ssp", name=f"pol{tt}")
                            # one matmul per psum bank (a matmul must not
                            # cross a bank boundary)
                            for half in range(2):
                                for p in range(2):
                                    nc.tensor.matmul(
                                        po[:, 512 * half:512 * (half + 1)],
                                        yt[:, p, 128 * tt:128 * (tt + 1)],
                                        wp_sb[:, p, 512 * half:
                                              512 * (half + 1)],
                                        start=(p == 0), stop=(p == 1))
                            ot = osbp.tile([128, 1024], dt.bfloat16,
                                           tag="otw", name=f"otw{tt}")
                            if tt % 2 == 0:
                                nc.scalar.copy(ot[:], po[:])
                            else:
                                nc.vector.tensor_copy(ot[:], po[:])
                            nc.sync.dma_start(
                                out_d[128 * tt:128 * (tt + 1), :], ot[:])
                            return
                        po = alloc_po(tt, cc)
                        for p in range(2):
                            nc.tensor.matmul(
                                po, yt[:, p, 128 * tt:128 * (tt + 1)],
                                wp_sb[:, p, 512 * cc:512 * (cc + 1)],
                                start=(p == 0), stop=(p == 1))
                        ot = osbp.tile([128, 512], dt.bfloat16,
                                       tag="ot", name=f"ot{tt}{cc}")
                        nc.vector.tensor_copy(ot[:], po)
                        nc.sync.dma_start(
                            out_d[128 * tt:128 * (tt + 1),
                                  512 * cc:512 * (cc + 1)], ot[:])
                    return proj

                class HeadRun:
                    """Incremental emitter for one (head, query-block)."""

                    def __init__(self, h, j, on_act=False):
                        self.h, self.j = h, j
                        self.n_u = 2 * j + 2
                        self.units = []
                        self.np_ = 0
                        self.yn = None
                        self.on_act = on_act
                        self.done = False

                    def sexp(self, k=1):
                        for _ in range(k):
                            if len(self.units) >= self.n_u:
                                return
                            self.units.append(
                                emit_sexp(self.h, self.j, len(self.units)))

                    def pv(self, k=1):
                        ns = len(self.units)
                        lim = ns if ns == self.n_u else ns - 1
                        for _ in range(k):
                            if self.np_ >= lim:
                                break
                            if self.yn is None:
                                self.yn = alloc_yn(self.h, self.j)
                            u = self.np_
                            pt, mem, dmap = self.units[u]
                            for mi, (i, qoff, off, w) in enumerate(mem):
                                qbs = list(range(
                                    max(qoff // 128, i - 4 * self.j), 4))
                                for qi, qb in enumerate(qbs):
                                    if qb == i - 4 * self.j:
                                        lhs = dmap[i][:]
                                    else:
                                        col = off + 128 * qb - qoff
                                        lhs = pt[:, col:col + 128]
                                    nc.tensor.matmul(
                                        self.yn[:, qb, :], lhs,
                                        vn[:, self.h, i, :],
                                        start=(u == 0 and mi == 0
                                               and qi == 0),
                                        stop=(u == self.n_u - 1
                                              and mi == len(mem) - 1
                                              and qi == len(qbs) - 1))
                            self.units[u] = None
                            self.np_ += 1
                        if self.np_ == self.n_u and not self.done:
                            self.done = True
                            pending.append(make_norm(self.h, self.j, self.yn,
                                                     self.on_act))

                    def step(self):
                        self.sexp(1)
                        drip(1)
                        self.pv(1)

                    def run_all(self):
                        while not self.done:
                            self.step()


                def qk_unit(hh, mg):
                    ps = psA.tile([128, 4, 128], dt.float32, tag="psqk")
                    for mi in range(4):
                        m = 4 * mg + mi
                        for k in range(8):
                            nc.tensor.matmul(
                                ps[:, mi, :], wqall[:, m, k, :],
                                xt[:, k, 128 * hh:128 * (hh + 1)],
                                start=(mi == 0 and k == 0),
                                stop=(mi == 3 and k == 7))
                    dest = qt_all if mg < 2 else kt_all
                    dv = dest[:].rearrange("d (h rh g) -> d h rh g",
                                           rh=128, g=16)
                    for par in range(2):
                        gb = 8 * (mg % 2) + par
                        nc.vector.tensor_tensor(
                            dv[:, hh, :, gb:gb + 7:2],
                            ps[64 * par:64 * par + 64].rearrange(
                                "d mi rh -> d rh mi"),
                            bqk_sb[64 * par:64 * par + 64,
                                   4 * mg:4 * mg + 4][:, None, :]
                            .broadcast_to([64, 128, 4]),
                            op=OP.add)

                def v_head(h):
                    vsb = vst.tile([128, 2, 512], dt.bfloat16, tag="vsb",
                                   name=f"vsb{h}")
                    for jv in range(2):
                        ps = psB.tile([128, 512], dt.float32,
                                      tag=("pvA", "pvB")[jv], bufs=1)
                        for k in range(8):
                            nc.tensor.matmul(
                                ps[:], xt[:, k, 128 * h:128 * (h + 1)],
                                wvt[jv][:, k, :],
                                start=(k == 0), stop=False)
                        nc.tensor.matmul(
                            ps[:], onesr[:],
                            bvn_sb[:, 512 * jv:512 * (jv + 1)],
                            start=False, stop=True)
                        nc.vector.tensor_copy(vsb[:, jv, :], ps[:])
                    # re-partition to [s, hd] via a DRAM bounce (the gather's
                    # source AP mixes partition bits into free dims):
                    # s = 16*rr + g, g = 8*jv + g2 -> partition (r jv g2)
                    nc.sync.dma_start(vscr[h][:], vsb[:])
                    src = vscr[h][:].rearrange(
                        "(i r) jv (g2 d) -> (r jv g2) i d", r=8, d=64)
                    nc.sync.dma_start(vn[:, h, :, 0:64], src)

                # ---------------- the pipelined schedule ----------------
                runs = {}
                for h in range(HPC):
                    for j in range(4):
                        runs[(h, j)] = HeadRun(h, j, on_act=(j == 0))

                for mg in range(4):           # head 0 projection
                    qk_unit(0, mg)
                v_head(0)
                runs[(0, 3)].sexp(2)          # h0's exp starts during V
                v_head(1)
                runs[(0, 3)].sexp(2)
                v_head(2)
                runs[(0, 3)].sexp(2)
                v_head(3)
                runs[(0, 3)].sexp(2)

                for hh in range(1, HPC):      # heads 1-3 projection,
                    pr = runs[(hh - 1, 3)]    # interleaved with attn(hh-1)
                    for mg in range(4):
                        qk_unit(hh, mg)
                        pr.sexp(2)
                        drip(1)
                        pr.pv(3)
                        # keep ACT fed: pre-emit j=2 S/exp of done heads
                        runs[(hh - 1, 2)].sexp(1)
                    pr.pv(3)                  # finish the head
                    runs[(hh - 1, 2)].sexp(1)

                runs[(3, 3)].run_all()        # last head's big block

                for tt in range(12, 16):
                    pending.append(make_proj_one(3, tt, 0))
                    pending.append(make_proj_one(3, tt, 1))

                for jx, j in enumerate([2, 1, 0]):
                    for h in range(HPC):
                        runs[(h, j)].run_all()
                    for tt in range(4 * j, 4 * j + 4):
                        if j == 0:
                            pending.append(make_proj_one(j, tt, 0, last=True))
                        else:
                            for cc in range(2):
                                pending.append(make_proj_one(j, tt, cc))
                for fn in pending[:]:
                    pending.pop(0)()
    nc.compile()
    return nc


_NC_CACHE = None


def _get_program():
    global _NC_CACHE
    if _NC_CACHE is None:
        _NC_CACHE = build_program()
    return _NC_CACHE


def _prep_core_inputs(x, Wqkv, bqkv, Wproj, bproj):
    """Build the 8 per-core input dicts (host-side shard + layout prep)."""
    x = np.asarray(x, dtype=np.float32)
    Wqkv = np.ascontiguousarray(np.asarray(Wqkv, dtype=np.float32))
    bqkv = np.asarray(bqkv, dtype=np.float32)
    Wproj = np.asarray(Wproj, dtype=np.float32)

    wq_np = np.ascontiguousarray(
        Wqkv[:, :2048].reshape(8, 128, 16, 128).transpose(2, 1, 0, 3)
        .reshape(16, 128, 8 * 128)).astype(BF16)
    wv_np = np.ascontiguousarray(
        Wqkv[:, 2048:].reshape(8, 128, 2, 512).transpose(2, 1, 0, 3)
        .reshape(2, 128, 8 * 512)).astype(BF16)
    bqk_np = np.ascontiguousarray(bqkv[:2048].reshape(16, 128).T)
    bvn_np = np.ascontiguousarray(bqkv[2048:].reshape(1, 1024)).astype(BF16)

    in_maps = []
    for c in range(N_CORES):
        b, q = divmod(c, 4)
        xT_np = np.ascontiguousarray(
            x[b, RPC * q:RPC * (q + 1), :].reshape(RPC, 8, 128)
            .transpose(2, 1, 0)).astype(BF16)
        wp_np = np.ascontiguousarray(
            Wproj[256 * q:256 * (q + 1), :].reshape(2, 128, 1024)
            .transpose(1, 0, 2).reshape(128, 2048)).astype(BF16)
        in_maps.append({
            "xT": xT_np, "wq": wq_np, "wv": wv_np, "bqk": bqk_np,
            "bvn": bvn_np, "wp": wp_np,
        })
    return in_maps


def kernel(x, Wqkv, bqkv, Wproj, bproj):
    nc = _get_program()
    in_maps = _prep_core_inputs(x, Wqkv, bqkv, Wproj, bproj)
    res = run_bass_kernel_spmd(nc, in_maps, list(range(N_CORES)))
    out = np.zeros((B, T, C), dtype=np.float32)
    for c in range(N_CORES):
        out[c // 4] += res.results[c]["out"].astype(np.float32)
    out += np.asarray(bproj, dtype=np.float32)
    return out


# revision 32
# speedup vs baseline: 1.4205x; 1.0151x over previous
"""Causal self-attention (dense transformer) on 8 trn2 NeuronCores.

Reference semantics (note the headless reshape):
  x_proj = x @ Wqkv + bqkv                     # [B, T, 3C]
  q = x_proj[:, :, :C].reshape(B, H, T, hd)    # direct reshape, no transpose!
Because of the direct reshape, head h consumes the contiguous row block
x_proj[b, h*128:(h+1)*128, :] reinterpreted as [T, hd].  So sharding by
(batch, head-group) makes QKV projection + attention fully core-local;
only the output projection is a row-parallel partial sum, reduced on host.

Shapes (hardcoded): B=2, T=2048, C=1024, n_head=16, hd=64, 8 cores.
Core c: batch b=c//4, quarter q=c%4 -> x rows [512q, 512q+512), heads 4q..4q+3.

v4 design notes (cost model: matmul cost = out-free-size x cycles/row; K and
partition count are free; bf16 is 1 cyc/row at ANY width, f32r only >=256):
- bf16 everywhere (PSUM fp32).  Total error ~4e-3 vs the 2e-2 gate.
- QK projection is HEAD-BLOCKED (per (head, m-group-of-4) unit, 32 matmuls
  of 128-wide bf16 into one psum bank / one accumulation group), and the
  whole schedule is SOFTWARE-PIPELINED BY HEAD: head h's attention
  (S -> exp -> P@V) is emitted interleaved with head h+1's projection
  units, so the ACT engine (exp, the #2 load at ~70us) starts ~18us in and
  never waits for the full projection.
- order: QK(h0) | V(all heads, + S/exp of h0 woven) | QK(h1)+attn(h0) |
  QK(h2)+attn(h1) | QK(h3)+attn(h2) | attn(h3) | remaining query blocks
  j=2,1,0 with drip-fed normalization + projection closures.
- PSUM bank lifetimes telescope: QK pool (2) and V pool (2) + S pairs (4)
  early; V pool is then traded for the Y^T-transpose bank and the j=3 yn
  bank; the QK pool is traded for the j<=2 yn pool; the first yn bank is
  traded for the projection bank.  Always exactly 8 banks.
- eviction of q^T/k^T: DVE tensor_tensor add with a stride-0-broadcast
  per-(partition, m) bias AP; stride-16 shuffled dest APs.
- V natural with ones-row bias matmul (the V bias varies with s%16 via the
  headless reshape, so it must be added on x_proj columns), bf16-evicted,
  re-partitioned to [s, hd] tiles via a DRAM bounce.
- S^T tiles at causally-minimal widths (512/384/256/128); exp straight off
  2-bank psum with scale=1/8; only the exact-diagonal 128x128 block is
  masked, out-of-place into a small ptd tile (mask off the critical path).
- P@V natural-Y: yn[q=128, 4, 65] accumulates over s-tiles with lhsT =
  pt/ptd 128-col slices: 65 cycles per (s-tile, q-block) instead of 512.
  Above-diagonal blocks skipped; ones-column gives the denominator.
- normalization: batched reciprocal, one broadcast tensor_tensor eviction,
  PE-transpose (identity matmul) to Y^T.
- projection: 512-wide dripped units; the last query block runs 1024-wide
  units on the freed S-psum banks with ACT/DVE alternating evictions.
- host: 4-way partial reduction + bproj in fp32.
"""

import os

import numpy as np
import ml_dtypes

os.environ.setdefault("NEURON_RT_RESET_CORES", "1")

import concourse.bacc as bacc
import concourse.mybir as mybir
import concourse.tile as tile
from concourse import masks
from concourse.bass_utils import run_bass_kernel_spmd

dt = mybir.dt
AF = mybir.ActivationFunctionType
OP = mybir.AluOpType
BF16 = np.dtype(ml_dtypes.bfloat16)

B, T, C = 2, 2048, 1024
NH, HD = 16, 64
N_CORES = 8
HPC = 4          # heads per core
RPC = 512        # x rows per core
SCALE = 1.0 / 8.0   # 1/sqrt(hd), folded into the exp activation


def _pair_members(j, u):
    """s-tile pair u of query block j: list of (i, qoff, psum_off, width)."""
    if u < 2 * j:
        return [(2 * u, 0, 0, 512), (2 * u + 1, 0, 512, 512)]
    if u == 2 * j:
        return [(4 * j, 0, 0, 512), (4 * j + 1, 128, 512, 384)]
    return [(4 * j + 2, 256, 0, 256), (4 * j + 3, 384, 256, 128)]


def build_program():
    nc = bacc.Bacc("TRN2", target_bir_lowering=False, debug=False,
                   num_devices=N_CORES)

    # ---- DRAM I/O (per core) ----
    xT = nc.dram_tensor("xT", [128, 8, RPC], dt.bfloat16, kind="ExternalInput")
    wq = nc.dram_tensor("wq", [16, 128, 8 * 128], dt.bfloat16, kind="ExternalInput")
    wv = nc.dram_tensor("wv", [2, 128, 8 * 512], dt.bfloat16, kind="ExternalInput")
    bqk = nc.dram_tensor("bqk", [128, 16], dt.float32, kind="ExternalInput")
    bvn = nc.dram_tensor("bvn", [1, 1024], dt.bfloat16, kind="ExternalInput")
    wp = nc.dram_tensor("wp", [128, 2 * 1024], dt.bfloat16, kind="ExternalInput")
    out_d = nc.dram_tensor("out", [T, C], dt.bfloat16, kind="ExternalOutput")

    with tile.TileContext(nc) as tc:
        with tc.tile_pool(name="persist", bufs=1) as pp, \
             tc.tile_pool(name="drampool", bufs=1, space="DRAM") as dp:
            vscr = [dp.tile([128, 2, 512], dt.bfloat16, tag=f"vscr{h}",
                            name=f"vscr{h}") for h in range(HPC)]
            xt = pp.tile([128, 8, RPC], dt.bfloat16, tag="xt")
            bqk_sb = pp.tile([128, 16], dt.float32, tag="bqk")
            bvn_sb = pp.tile([1, 1024], dt.bfloat16, tag="bvn")
            onesr = pp.tile([1, 128], dt.bfloat16, tag="onesr")
            wp_sb = pp.tile([128, 2, 1024], dt.bfloat16, tag="wp")
            ident = pp.tile([128, 128], dt.bfloat16, tag="ident")

            qt_all = pp.tile([64, HPC * T], dt.bfloat16, tag="qt_all")
            kt_all = pp.tile([64, HPC * T], dt.bfloat16, tag="kt_all")
            vn = pp.tile([128, HPC, 16, 65], dt.bfloat16, tag="vn")
            yt = pp.tile([128, 2, T], dt.bfloat16, tag="yt")

            masks.make_identity(nc, ident[:])
            nc.gpsimd.memset(onesr[:], 1.0)
            nc.gpsimd.memset(vn[:, :, :, 64], 1.0)

            # p-state warmup: the PE clock ramp keys off the FIRST busy
            # time; burn it on the identity tile while input DMAs land
            with tc.tile_pool(name="warm", bufs=1, space="PSUM") as pw:
                wps = pw.tile([128, 128], dt.float32, tag="w")
                for _ in range(18):
                    nc.tensor.matmul(wps[:], ident[:], ident[:],
                                     start=True, stop=True)

            with tc.tile_pool(name="ptpool", bufs=30) as ptp, \
                 tc.tile_pool(name="ptdpool", bufs=14) as ptdp, \
                 tc.tile_pool(name="misc", bufs=3) as mp, \
                 tc.tile_pool(name="osb", bufs=3) as osbp, \
                 tc.tile_pool(name="wstream", bufs=1) as ws, \
                 tc.tile_pool(name="vstage", bufs=3) as vst, \
                 tc.tile_pool(name="ps2", bufs=2, space="PSUM") as ps2, \
                 tc.tile_pool(name="psA", bufs=2, space="PSUM") as psA, \
                 tc.tile_pool(name="psB", bufs=2, space="PSUM") as psB:

                # bank-reuse view allocators: psA's 2 banks serve the QK
                # units and later the j<=2 yn accumulators; psB's 2 banks
                # serve the V units and later the j=3 yn / Y^T-transpose /
                # projection tiles.  Always 8 banks total.
                def alloc_yn(h, j):
                    if j == 3 and h < 3:
                        tl = psB.tile([128, 512], dt.float32, tag="pvB",
                                      bufs=1, name=f"ynb{h}{j}")
                        return tl[:].rearrange("p (a b) -> p a b",
                                               a=4)[:, :, 0:65]
                    tl = psA.tile([128, 4, 128], dt.float32, tag="psqk",
                                  name=f"yna{h}{j}")
                    return tl[:, :, 0:65]

                def alloc_ytr(h, j):
                    tl = psB.tile([128, 512], dt.float32, tag="pvA",
                                  bufs=1, name=f"ytrt{h}{j}")
                    return tl[:].bitcast(dt.bfloat16)[0:64, 0:512]

                def alloc_po(tt, cc):
                    tl = psB.tile([128, 512], dt.float32, tag="pvB",
                                  bufs=1, name=f"pot{tt}{cc}")
                    return tl[:]

                # ---------------- input DMAs ----------------
                wqall = ws.tile([128, 16, 8, 128], dt.bfloat16, tag="wqall")
                nc.sync.dma_start(xt[:, 0:2, :], xT[:, 0:2, :])
                nc.sync.dma_start(xt[:, 2:8, :], xT[:, 2:8, :])
                for c4 in range(4):
                    nc.sync.dma_start(
                        wqall[:, 4 * c4:4 * c4 + 4],
                        wq[4 * c4:4 * c4 + 4].rearrange(
                            "m p (k j) -> p m k j", k=8))
                    if c4 == 0:
                        nc.sync.dma_start(bqk_sb[:], bqk[:])
                        nc.sync.dma_start(bvn_sb[:], bvn[:])
                # V/proj weights: virtual-time delayed so the wq stream owns
                # the DMA engine while it feeds PE; wv0 lands right as the
                # first V unit needs it
                wvt = [ws.tile([128, 8, 512], dt.bfloat16, tag="wvt",
                               bufs=2, name=f"wvt{jv}") for jv in range(2)]
                for jv in range(2):
                    tc.tile_set_cur_wait(0.0145 + 0.0035 * jv)
                    nc.sync.dma_start(wvt[jv][:], wv[jv].rearrange(
                        "p (k j) -> p k j", k=8))
                tc.tile_set_cur_wait(0.021)
                nc.sync.dma_start(wp_sb[:], wp.rearrange(
                    "p (t c) -> p t c", t=2))
                tc.tile_set_cur_wait(0.0)

                # ---------------- emission helpers ----------------
                def emit_sexp(h, j, u):
                    """S^T matmuls for pair u + exp + exact-diagonal mask."""
                    mem = _pair_members(j, u)
                    tot = mem[-1][2] + mem[-1][3]
                    same_bank = tot <= 512   # pair B: one group, start/stop split
                    ssp = ps2.tile([128, 1024], dt.float32, tag="ssp",
                                   name=f"ssp{h}{j}{u}")
                    for mi, (i, qoff, off, w) in enumerate(mem):
                        nc.tensor.matmul(
                            ssp[:, off:off + w],
                            kt_all[:, T * h + 128 * i:T * h + 128 * (i + 1)],
                            qt_all[:, T * h + 512 * j + qoff:
                                   T * h + 512 * j + qoff + w],
                            start=(not same_bank) or mi == 0,
                            stop=(not same_bank) or mi == len(mem) - 1)
                    pt = ptp.tile([128, 1024], dt.bfloat16, tag="pt",
                                  name=f"pt{h}{j}{u}")
                    nc.scalar.activation(pt[:, 0:tot], ssp[:, 0:tot],
                                         AF.Exp, scale=SCALE)
                    dmap = {}
                    if u >= 2 * j:  # mask the exact diagonal, out-of-place
                        for (i, qoff, off, w) in mem:
                            dcol = off + 128 * (i - 4 * j) - qoff
                            ptd = ptdp.tile([128, 128], dt.bfloat16,
                                            tag="ptd", name=f"ptd{h}{j}{u}{i}")
                            nc.gpsimd.affine_select(
                                out=ptd[:], in_=pt[:, dcol:dcol + 128],
                                compare_op=OP.is_ge, fill=0.0,
                                base=0, channel_multiplier=-1,
                                pattern=[[1, 128]])
                            dmap[i] = ptd
                    return pt, mem, dmap

                pending = []   # deferred (norm / proj) closures, drip-fed

                def drip(n=1):
                    npop = min(n + (len(pending) > 4) + (len(pending) > 8),
                               len(pending))
                    for _ in range(npop):
                        pending.pop(0)()

                def make_norm(h, j, yn, on_act=False):
                    def norm():
                        rin = mp.tile([128, 4], dt.float32, tag="rin",
                                      name=f"rin{h}{j}")
                        nc.vector.tensor_copy(rin[:], yn[:, :, 64])
                        rcp = mp.tile([128, 4], dt.float32, tag="rcp",
                                      name=f"rcp{h}{j}")
                        with nc.allow_low_precision(reason="softmax recip"):
                            nc.vector.reciprocal(rcp[:], rin[:])
                        yb = mp.tile([128, 4, 64], dt.bfloat16, tag="yb",
                                     name=f"yb{h}{j}")
                        nc.vector.tensor_tensor(
                            yb[:], yn[:, :, 0:64],
                            rcp[:, :, None].broadcast_to([128, 4, 64]),
                            op=OP.mult)
                        ytr = alloc_ytr(h, j)
                        for qb in range(4):
                            nc.tensor.transpose(
                                ytr[:, 128 * qb:128 * (qb + 1)],
                                yb[:, qb, :], ident[:])
                        dst = yt[64 * (h % 2):64 * (h % 2) + 64,
                                 h // 2, 512 * j:512 * (j + 1)]
                        if on_act:
                            nc.scalar.copy(dst, ytr)
                        else:
                            nc.vector.tensor_copy(dst, ytr)
                    return norm

                def make_proj_one(j, tt, cc, last=False):
                    def proj():
                        if last:
                            po = ps2.tile([128, 1024], dt.float32,
                                          tag="ssp", name=f"pol{tt}")
                            # one matmul per psum bank (a matmul must not
                            # cross a bank boundary)
                            for half in range(2):
                                for p in range(2):
                                    nc.tensor.matmul(
                                        po[:, 512 * half:512 * (half + 1)],
                                        yt[:, p, 128 * tt:128 * (tt + 1)],
                                        wp_sb[:, p, 512 * half:
                                              512 * (half + 1)],
                                        start=(p == 0), stop=(p == 1))
                            ot = osbp.tile([128, 1024], dt.bfloat16,
                                           tag="otw", name=f"otw{tt}")
                            if tt % 2 == 0:
                                nc.scalar.copy(ot[:], po[:])
                            else:
                                nc.vector.tensor_copy(ot[:], po[:])
                            nc.sync.dma_start(
                                out_d[128 * tt:128 * (tt + 1), :], ot[:])
                            return
                        po = alloc_po(tt, cc)
                        for p in range(2):
                            nc.tensor.matmul(
                                po, yt[:, p, 128 * tt:128 * (tt + 1)],
                                wp_sb[:, p, 512 * cc:512 * (cc + 1)],
                                start=(p == 0), stop=(p == 1))
                        ot = osbp.tile([128, 512], dt.bfloat16,
                                       tag="ot", name=f"ot{tt}{cc}")
                        nc.vector.tensor_copy(ot[:], po)
                        nc.sync.dma_start(
                            out_d[128 * tt:128 * (tt + 1),
                                  512 * cc:512 * (cc + 1)], ot[:])
                    return proj

                class HeadRun:
                    """Incremental emitter for one (head, query-block)."""

                    def __init__(self, h, j, on_act=False):
                        self.h, self.j = h, j
                        self.n_u = 2 * j + 2
                        self.units = []
                        self.np_ = 0
                        self.yn = None
                        self.on_act = on_act
                        self.done = False

                    def sexp(self, k=1):
                        for _ in range(k):
                            if len(self.units) >= self.n_u:
                                return
                            self.units.append(
                                emit_sexp(self.h, self.j, len(self.units)))

                    def pv(self, k=1):
                        ns = len(self.units)
                        lim = ns if ns == self.n_u else ns - 1
                        for _ in range(k):
                            if self.np_ >= lim:
                                break
                            if self.yn is None:
                                self.yn = alloc_yn(self.h, self.j)
                            u = self.np_
                            pt, mem, dmap = self.units[u]
                            for mi, (i, qoff, off, w) in enumerate(mem):
                                qbs = list(range(
                                    max(qoff // 128, i - 4 * self.j), 4))
                                for qi, qb in enumerate(qbs):
                                    if qb == i - 4 * self.j:
                                        lhs = dmap[i][:]
                                    else:
                                        col = off + 128 * qb - qoff
                                        lhs = pt[:, col:col + 128]
                                    nc.tensor.matmul(
                                        self.yn[:, qb, :], lhs,
                                        vn[:, self.h, i, :],
                                        start=(u == 0 and mi == 0
                                               and qi == 0),
                                        stop=(u == self.n_u - 1
                                              and mi == len(mem) - 1
                                              and qi == len(qbs) - 1))
                            self.units[u] = None
                            self.np_ += 1
                        if self.np_ == self.n_u and not self.done:
                            self.done = True
                            pending.append(make_norm(self.h, self.j, self.yn,
                                                     self.on_act))

                    def step(self):
                        self.sexp(1)
                        drip(1)
                        self.pv(1)

                    def run_all(self):
                        while not self.done:
                            self.step()


                def qk_unit(hh, mg):
                    ps = psA.tile([128, 4, 128], dt.float32, tag="psqk")
                    for mi in range(4):
                        m = 4 * mg + mi
                        for k in range(8):
                            nc.tensor.matmul(
                                ps[:, mi, :], wqall[:, m, k, :],
                                xt[:, k, 128 * hh:128 * (hh + 1)],
                                start=(mi == 0 and k == 0),
                                stop=(mi == 3 and k == 7))
                    dest = qt_all if mg < 2 else kt_all
                    dv = dest[:].rearrange("d (h rh g) -> d h rh g",
                                           rh=128, g=16)
                    for par in range(2):
                        gb = 8 * (mg % 2) + par
                        nc.vector.tensor_tensor(
                            dv[:, hh, :, gb:gb + 7:2],
                            ps[64 * par:64 * par + 64].rearrange(
                                "d mi rh -> d rh mi"),
                            bqk_sb[64 * par:64 * par + 64,
                                   4 * mg:4 * mg + 4][:, None, :]
                            .broadcast_to([64, 128, 4]),
                            op=OP.add)

                def v_head(h):
                    vsb = vst.tile([128, 2, 512], dt.bfloat16, tag="vsb",
                                   name=f"vsb{h}")
                    for jv in range(2):
                        ps = psB.tile([128, 512], dt.float32,
                                      tag=("pvA", "pvB")[jv], bufs=1)
                        for k in range(8):
                            nc.tensor.matmul(
                                ps[:], xt[:, k, 128 * h:128 * (h + 1)],
                                wvt[jv][:, k, :],
                                start=(k == 0), stop=False)
                        nc.tensor.matmul(
                            ps[:], onesr[:],
                            bvn_sb[:, 512 * jv:512 * (jv + 1)],
                            start=False, stop=True)
                        nc.vector.tensor_copy(vsb[:, jv, :], ps[:])
                    # re-partition to [s, hd] via a DRAM bounce (the gather's
                    # source AP mixes partition bits into free dims):
                    # s = 16*rr + g, g = 8*jv + g2 -> partition (r jv g2)
                    nc.sync.dma_start(vscr[h][:], vsb[:])
                    src = vscr[h][:].rearrange(
                        "(i r) jv (g2 d) -> (r jv g2) i d", r=8, d=64)
                    nc.sync.dma_start(vn[:, h, :, 0:64], src)

                # ---------------- the pipelined schedule ----------------
                runs = {}
                for h in range(HPC):
                    for j in range(4):
                        runs[(h, j)] = HeadRun(h, j, on_act=(j == 0))

                for mg in range(4):           # head 0 projection
                    qk_unit(0, mg)
                v_head(0)
                runs[(0, 3)].sexp(2)          # h0's exp starts during V
                v_head(1)
                runs[(0, 3)].sexp(2)
                v_head(2)
                runs[(0, 3)].sexp(2)
                v_head(3)
                runs[(0, 3)].sexp(2)

                for hh in range(1, HPC):      # heads 1-3 projection,
                    pr = runs[(hh - 1, 3)]    # interleaved with attn(hh-1)
                    for mg in range(4):
                        qk_unit(hh, mg)
                        pr.sexp(2)
                        drip(1)
                        pr.pv(3)
                        # keep ACT fed: pre-emit j=2 S/exp of done heads
                        runs[(hh - 1, 2)].sexp(1)
                    pr.pv(3)                  # finish the head
                    runs[(hh - 1, 2)].sexp(1)

                runs[(3, 3)].run_all()        # last head's big block

                for tt in range(12, 16):
                    pending.append(make_proj_one(3, tt, 0))
                    pending.append(make_proj_one(3, tt, 1))

                for jx, j in enumerate([2, 1, 0]):
                    for h in range(HPC):
                        runs[(h, j)].run_all()
                    for tt in range(4 * j, 4 * j + 4):
                        if j == 0:
                            pending.append(make_proj_one(j, tt, 0, last=True))
                        else:
                            for cc in range(2):
                                pending.append(make_proj_one(j, tt, cc))
                for fn in pending[:]:
                    pending.pop(0)()
    nc.compile()
    return nc


_NC_CACHE = None


def _get_program():
    global _NC_CACHE
    if _NC_CACHE is None:
        _NC_CACHE = build_program()
    return _NC_CACHE


def _prep_core_inputs(x, Wqkv, bqkv, Wproj, bproj):
    """Build the 8 per-core input dicts (host-side shard + layout prep)."""
    x = np.asarray(x, dtype=np.float32)
    Wqkv = np.ascontiguousarray(np.asarray(Wqkv, dtype=np.float32))
    bqkv = np.asarray(bqkv, dtype=np.float32)
    Wproj = np.asarray(Wproj, dtype=np.float32)

    wq_np = np.ascontiguousarray(
        Wqkv[:, :2048].reshape(8, 128, 16, 128).transpose(2, 1, 0, 3)
        .reshape(16, 128, 8 * 128)).astype(BF16)
    wv_np = np.ascontiguousarray(
        Wqkv[:, 2048:].reshape(8, 128, 2, 512).transpose(2, 1, 0, 3)
        .reshape(2, 128, 8 * 512)).astype(BF16)
    bqk_np = np.ascontiguousarray(bqkv[:2048].reshape(16, 128).T)
    bvn_np = np.ascontiguousarray(bqkv[2048:].reshape(1, 1024)).astype(BF16)

    in_maps = []
    for c in range(N_CORES):
        b, q = divmod(c, 4)
        xT_np = np.ascontiguousarray(
            x[b, RPC * q:RPC * (q + 1), :].reshape(RPC, 8, 128)
            .transpose(2, 1, 0)).astype(BF16)
        wp_np = np.ascontiguousarray(
            Wproj[256 * q:256 * (q + 1), :].reshape(2, 128, 1024)
            .transpose(1, 0, 2).reshape(128, 2048)).astype(BF16)
        in_maps.append({
            "xT": xT_np, "wq": wq_np, "wv": wv_np, "bqk": bqk_np,
            "bvn": bvn_np, "wp": wp_np,
        })
    return in_maps


def kernel(x, Wqkv, bqkv, Wproj, bproj):
    nc = _get_program()
    in_maps = _prep_core_inputs(x, Wqkv, bqkv, Wproj, bproj)
    res = run_bass_kernel_spmd(nc, in_maps, list(range(N_CORES)))
    out = np.zeros((B, T, C), dtype=np.float32)
    for c in range(N_CORES):
        out[c // 4] += res.results[c]["out"].astype(np.float32)
    out += np.asarray(bproj, dtype=np.float32)
    return out


# revision 43
# speedup vs baseline: 1.4462x; 1.0181x over previous
"""Causal self-attention (dense transformer) on 8 trn2 NeuronCores.

Reference semantics (note the headless reshape):
  x_proj = x @ Wqkv + bqkv                     # [B, T, 3C]
  q = x_proj[:, :, :C].reshape(B, H, T, hd)    # direct reshape, no transpose!
Because of the direct reshape, head h consumes the contiguous row block
x_proj[b, h*128:(h+1)*128, :] reinterpreted as [T, hd].  So sharding by
(batch, head-group) makes QKV projection + attention fully core-local;
only the output projection is a row-parallel partial sum, reduced on host.

Shapes (hardcoded): B=2, T=2048, C=1024, n_head=16, hd=64, 8 cores.
Core c: batch b=c//4, quarter q=c%4 -> x rows [512q, 512q+512), heads 4q..4q+3.

v4 design notes (cost model: matmul cost = out-free-size x cycles/row; K and
partition count are free; bf16 is 1 cyc/row at ANY width, f32r only >=256):
- bf16 everywhere (PSUM fp32).  Total error ~4e-3 vs the 2e-2 gate.
- QK projection is HEAD-BLOCKED (per (head, m-group-of-4) unit, 32 matmuls
  of 128-wide bf16 into one psum bank / one accumulation group), and the
  whole schedule is SOFTWARE-PIPELINED BY HEAD: head h's attention
  (S -> exp -> P@V) is emitted interleaved with head h+1's projection
  units, so the ACT engine (exp, the #2 load at ~70us) starts ~18us in and
  never waits for the full projection.
- order: QK(h0) | V(all heads, + S/exp of h0 woven) | QK(h1)+attn(h0) |
  QK(h2)+attn(h1) | QK(h3)+attn(h2) | attn(h3) | remaining query blocks
  j=2,1,0 with drip-fed normalization + projection closures.
- PSUM bank lifetimes telescope: QK pool (2) and V pool (2) + S pairs (4)
  early; V pool is then traded for the Y^T-transpose bank and the j=3 yn
  bank; the QK pool is traded for the j<=2 yn pool; the first yn bank is
  traded for the projection bank.  Always exactly 8 banks.
- eviction of q^T/k^T: DVE tensor_tensor add with a stride-0-broadcast
  per-(partition, m) bias AP; stride-16 shuffled dest APs.
- V natural with ones-row bias matmul (the V bias varies with s%16 via the
  headless reshape, so it must be added on x_proj columns), bf16-evicted,
  re-partitioned to [s, hd] tiles via a DRAM bounce.
- S^T tiles at causally-minimal widths (512/384/256/128); exp straight off
  2-bank psum with scale=1/8; only the exact-diagonal 128x128 block is
  masked, out-of-place into a small ptd tile (mask off the critical path).
- P@V natural-Y: yn[q=128, 4, 65] accumulates over s-tiles with lhsT =
  pt/ptd 128-col slices: 65 cycles per (s-tile, q-block) instead of 512.
  Above-diagonal blocks skipped; ones-column gives the denominator.
- normalization: batched reciprocal, one broadcast tensor_tensor eviction,
  PE-transpose (identity matmul) to Y^T.
- projection: 512-wide dripped units; the last query block runs 1024-wide
  units on the freed S-psum banks with ACT/DVE alternating evictions.
- host: 4-way partial reduction + bproj in fp32.
"""

import os

import numpy as np
import ml_dtypes

os.environ.setdefault("NEURON_RT_RESET_CORES", "1")

import concourse.bacc as bacc
import concourse.mybir as mybir
import concourse.tile as tile
from concourse import masks
from concourse.bass_utils import run_bass_kernel_spmd

dt = mybir.dt
AF = mybir.ActivationFunctionType
OP = mybir.AluOpType
BF16 = np.dtype(ml_dtypes.bfloat16)

B, T, C = 2, 2048, 1024
NH, HD = 16, 64
N_CORES = 8
HPC = 4          # heads per core
RPC = 512        # x rows per core
SCALE = 1.0 / 8.0   # 1/sqrt(hd), folded into the exp activation


def _pair_members(j, u):
    """s-tile pair u of query block j: list of (i, qoff, psum_off, width)."""
    if u < 2 * j:
        return [(2 * u, 0, 0, 512), (2 * u + 1, 0, 512, 512)]
    if u == 2 * j:
        return [(4 * j, 0, 0, 512), (4 * j + 1, 128, 512, 384)]
    return [(4 * j + 2, 256, 0, 256), (4 * j + 3, 384, 256, 128)]


def build_program():
    nc = bacc.Bacc("TRN2", target_bir_lowering=False, debug=False,
                   num_devices=N_CORES)

    # ---- DRAM I/O (per core) ----
    xT = nc.dram_tensor("xT", [128, 8, RPC], dt.bfloat16, kind="ExternalInput")
    wq = nc.dram_tensor("wq", [16, 128, 8 * 128], dt.bfloat16, kind="ExternalInput")
    wv = nc.dram_tensor("wv", [2, 128, 8 * 512], dt.bfloat16, kind="ExternalInput")
    bqk = nc.dram_tensor("bqk", [128, 16], dt.float32, kind="ExternalInput")
    bvn = nc.dram_tensor("bvn", [1, 1024], dt.bfloat16, kind="ExternalInput")
    wp = nc.dram_tensor("wp", [128, 2 * 1024], dt.bfloat16, kind="ExternalInput")
    out_d = nc.dram_tensor("out", [T, C], dt.bfloat16, kind="ExternalOutput")

    with tile.TileContext(nc) as tc:
        with tc.tile_pool(name="persist", bufs=1) as pp, \
             tc.tile_pool(name="drampool", bufs=1, space="DRAM") as dp:
            vscr = [dp.tile([128, 2, 512], dt.bfloat16, tag=f"vscr{h}",
                            name=f"vscr{h}") for h in range(HPC)]
            xt = pp.tile([128, 8, RPC], dt.bfloat16, tag="xt")
            bqk_sb = pp.tile([128, 16], dt.float32, tag="bqk")
            bvn_sb = pp.tile([1, 1024], dt.bfloat16, tag="bvn")
            onesr = pp.tile([1, 128], dt.bfloat16, tag="onesr")
            wp_sb = pp.tile([128, 2, 1024], dt.bfloat16, tag="wp")
            ident = pp.tile([128, 128], dt.bfloat16, tag="ident")

            qt_all = pp.tile([64, HPC * T], dt.bfloat16, tag="qt_all")
            kt_all = pp.tile([64, HPC * T], dt.bfloat16, tag="kt_all")
            vn = pp.tile([128, HPC, 16, 65], dt.bfloat16, tag="vn")
            yt = pp.tile([128, 2, T], dt.bfloat16, tag="yt")

            masks.make_identity(nc, ident[:])
            nc.gpsimd.memset(onesr[:], 1.0)
            nc.gpsimd.memset(vn[:, :, :, 64], 1.0)

            # p-state warmup: the PE clock ramp keys off the FIRST busy
            # time; burn it on the identity tile while input DMAs land
            with tc.tile_pool(name="warm", bufs=1, space="PSUM") as pw:
                wps = pw.tile([128, 128], dt.float32, tag="w")
                for _ in range(18):
                    nc.tensor.matmul(wps[:], ident[:], ident[:],
                                     start=True, stop=True)

            with tc.tile_pool(name="ptpool", bufs=30) as ptp, \
                 tc.tile_pool(name="ptdpool", bufs=14) as ptdp, \
                 tc.tile_pool(name="misc", bufs=3) as mp, \
                 tc.tile_pool(name="osb", bufs=3) as osbp, \
                 tc.tile_pool(name="wstream", bufs=1) as ws, \
                 tc.tile_pool(name="vstage", bufs=3) as vst, \
                 tc.tile_pool(name="ps2", bufs=2, space="PSUM") as ps2, \
                 tc.tile_pool(name="psA", bufs=2, space="PSUM") as psA, \
                 tc.tile_pool(name="psB", bufs=2, space="PSUM") as psB:

                # bank-reuse view allocators: psA's 2 banks serve the QK
                # units and later the j<=2 yn accumulators; psB's 2 banks
                # serve the V units and later the j=3 yn / Y^T-transpose /
                # projection tiles.  Always 8 banks total.
                def alloc_yn(h, j):
                    if j == 3 and h < 3:
                        tl = psB.tile([128, 512], dt.float32, tag="pvB",
                                      bufs=1, name=f"ynb{h}{j}")
                        return tl[:].rearrange("p (a b) -> p a b",
                                               a=4)[:, :, 0:65]
                    tl = psA.tile([128, 4, 128], dt.float32, tag="psqk",
                                  name=f"yna{h}{j}")
                    return tl[:, :, 0:65]

                def alloc_ytr(h, j):
                    tl = psB.tile([128, 512], dt.float32, tag="pvA",
                                  bufs=1, name=f"ytrt{h}{j}")
                    return tl[:].bitcast(dt.bfloat16)[0:64, 0:512]

                def alloc_po(tt, cc):
                    tl = psB.tile([128, 512], dt.float32, tag="pvB",
                                  bufs=1, name=f"pot{tt}{cc}")
                    return tl[:]

                # ---------------- input DMAs ----------------
                wqall = ws.tile([128, 16, 8, 128], dt.bfloat16, tag="wqall")
                nc.sync.dma_start(xt[:, 0:2, :], xT[:, 0:2, :])
                for c4 in range(4):
                    if c4 == 1:
                        nc.sync.dma_start(xt[:, 2:8, :], xT[:, 2:8, :])
                    if c4 == 0:
                        nc.sync.dma_start(
                            wqall[:, 0:2], wq[0:2].rearrange(
                                "m p (k j) -> p m k j", k=8))
                        nc.sync.dma_start(
                            wqall[:, 2:4], wq[2:4].rearrange(
                                "m p (k j) -> p m k j", k=8))
                        continue
                    nc.sync.dma_start(
                        wqall[:, 4 * c4:4 * c4 + 4],
                        wq[4 * c4:4 * c4 + 4].rearrange(
                            "m p (k j) -> p m k j", k=8))
                    if c4 == 1:
                        nc.sync.dma_start(bqk_sb[:], bqk[:])
                        nc.sync.dma_start(bvn_sb[:], bvn[:])
                # V/proj weights: virtual-time delayed so the wq stream owns
                # the DMA engine while it feeds PE; wv0 lands right as the
                # first V unit needs it
                wvt = [ws.tile([128, 8, 512], dt.bfloat16, tag="wvt",
                               bufs=2, name=f"wvt{jv}") for jv in range(2)]
                for jv in range(2):
                    tc.tile_set_cur_wait(0.0145 + 0.0035 * jv)
                    nc.sync.dma_start(wvt[jv][:], wv[jv].rearrange(
                        "p (k j) -> p k j", k=8))
                tc.tile_set_cur_wait(0.021)
                nc.sync.dma_start(wp_sb[:], wp.rearrange(
                    "p (t c) -> p t c", t=2))
                tc.tile_set_cur_wait(0.0)

                # ---------------- emission helpers ----------------
                def emit_sexp(h, j, u):
                    """S^T matmuls for pair u + exp + exact-diagonal mask."""
                    mem = _pair_members(j, u)
                    tot = mem[-1][2] + mem[-1][3]
                    same_bank = tot <= 512   # pair B: one group, start/stop split
                    ssp = ps2.tile([128, 1024], dt.float32, tag="ssp",
                                   name=f"ssp{h}{j}{u}")
                    for mi, (i, qoff, off, w) in enumerate(mem):
                        nc.tensor.matmul(
                            ssp[:, off:off + w],
                            kt_all[:, T * h + 128 * i:T * h + 128 * (i + 1)],
                            qt_all[:, T * h + 512 * j + qoff:
                                   T * h + 512 * j + qoff + w],
                            start=(not same_bank) or mi == 0,
                            stop=(not same_bank) or mi == len(mem) - 1)
                    pt = ptp.tile([128, 1024], dt.bfloat16, tag="pt",
                                  name=f"pt{h}{j}{u}")
                    nc.scalar.activation(pt[:, 0:tot], ssp[:, 0:tot],
                                         AF.Exp, scale=SCALE)
                    dmap = {}
                    if u >= 2 * j:  # mask the exact diagonal, out-of-place
                        for (i, qoff, off, w) in mem:
                            dcol = off + 128 * (i - 4 * j) - qoff
                            ptd = ptdp.tile([128, 128], dt.bfloat16,
                                            tag="ptd", name=f"ptd{h}{j}{u}{i}")
                            nc.gpsimd.affine_select(
                                out=ptd[:], in_=pt[:, dcol:dcol + 128],
                                compare_op=OP.is_ge, fill=0.0,
                                base=0, channel_multiplier=-1,
                                pattern=[[1, 128]])
                            dmap[i] = ptd
                    return pt, mem, dmap

                pending = []   # deferred (norm / proj) closures, drip-fed

                def drip(n=1):
                    npop = min(n + (len(pending) > 4) + (len(pending) > 8),
                               len(pending))
                    for _ in range(npop):
                        pending.pop(0)()

                def make_norm(h, j, yn, on_act=False):
                    def norm():
                        rin = mp.tile([128, 4], dt.float32, tag="rin",
                                      name=f"rin{h}{j}")
                        nc.vector.tensor_copy(rin[:], yn[:, :, 64])
                        rcp = mp.tile([128, 4], dt.float32, tag="rcp",
                                      name=f"rcp{h}{j}")
                        with nc.allow_low_precision(reason="softmax recip"):
                            nc.vector.reciprocal(rcp[:], rin[:])
                        yb = mp.tile([128, 4, 64], dt.bfloat16, tag="yb",
                                     name=f"yb{h}{j}")
                        nc.vector.tensor_tensor(
                            yb[:], yn[:, :, 0:64],
                            rcp[:, :, None].broadcast_to([128, 4, 64]),
                            op=OP.mult)
                        ytr = alloc_ytr(h, j)
                        for qb in range(4):
                            nc.tensor.transpose(
                                ytr[:, 128 * qb:128 * (qb + 1)],
                                yb[:, qb, :], ident[:])
                        dst = yt[64 * (h % 2):64 * (h % 2) + 64,
                                 h // 2, 512 * j:512 * (j + 1)]
                        if on_act:
                            nc.scalar.copy(dst, ytr)
                        else:
                            nc.vector.tensor_copy(dst, ytr)
                    return norm

                def make_proj_one(j, tt, cc, last=False):
                    def proj():
                        if last:
                            po = ps2.tile([128, 1024], dt.float32,
                                          tag="ssp", name=f"pol{tt}")
                            # one matmul per psum bank (a matmul must not
                            # cross a bank boundary)
                            for half in range(2):
                                for p in range(2):
                                    nc.tensor.matmul(
                                        po[:, 512 * half:512 * (half + 1)],
                                        yt[:, p, 128 * tt:128 * (tt + 1)],
                                        wp_sb[:, p, 512 * half:
                                              512 * (half + 1)],
                                        start=(p == 0), stop=(p == 1))
                            ot = osbp.tile([128, 1024], dt.bfloat16,
                                           tag="otw", name=f"otw{tt}")
                            if tt % 2 == 0:
                                nc.scalar.copy(ot[:], po[:])
                            else:
                                nc.vector.tensor_copy(ot[:], po[:])
                            nc.sync.dma_start(
                                out_d[128 * tt:128 * (tt + 1), :], ot[:])
                            return
                        po = alloc_po(tt, cc)
                        for p in range(2):
                            nc.tensor.matmul(
                                po, yt[:, p, 128 * tt:128 * (tt + 1)],
                                wp_sb[:, p, 512 * cc:512 * (cc + 1)],
                                start=(p == 0), stop=(p == 1))
                        ot = osbp.tile([128, 512], dt.bfloat16,
                                       tag="ot", name=f"ot{tt}{cc}")
                        nc.vector.tensor_copy(ot[:], po)
                        nc.sync.dma_start(
                            out_d[128 * tt:128 * (tt + 1),
                                  512 * cc:512 * (cc + 1)], ot[:])
                    return proj

                class HeadRun:
                    """Incremental emitter for one (head, query-block)."""

                    def __init__(self, h, j, on_act=False):
                        self.h, self.j = h, j
                        self.n_u = 2 * j + 2
                        self.units = []
                        self.np_ = 0
                        self.yn = None
                        self.on_act = on_act
                        self.done = False

                    def sexp(self, k=1):
                        for _ in range(k):
                            if len(self.units) >= self.n_u:
                                return
                            self.units.append(
                                emit_sexp(self.h, self.j, len(self.units)))

                    def pv(self, k=1):
                        ns = len(self.units)
                        la = 2
                        lim = ns if ns == self.n_u else max(ns - la, 0)
                        for _ in range(k):
                            if self.np_ >= lim:
                                break
                            if self.yn is None:
                                self.yn = alloc_yn(self.h, self.j)
                            u = self.np_
                            pt, mem, dmap = self.units[u]
                            for mi, (i, qoff, off, w) in enumerate(mem):
                                qbs = list(range(
                                    max(qoff // 128, i - 4 * self.j), 4))
                                for qi, qb in enumerate(qbs):
                                    if qb == i - 4 * self.j:
                                        lhs = dmap[i][:]
                                    else:
                                        col = off + 128 * qb - qoff
                                        lhs = pt[:, col:col + 128]
                                    nc.tensor.matmul(
                                        self.yn[:, qb, :], lhs,
                                        vn[:, self.h, i, :],
                                        start=(u == 0 and mi == 0
                                               and qi == 0),
                                        stop=(u == self.n_u - 1
                                              and mi == len(mem) - 1
                                              and qi == len(qbs) - 1))
                            self.units[u] = None
                            self.np_ += 1
                        if self.np_ == self.n_u and not self.done:
                            self.done = True
                            pending.append(make_norm(self.h, self.j, self.yn,
                                                     self.on_act))

                    def step(self):
                        self.sexp(1)
                        drip(1)
                        self.pv(1)

                    def run_all(self):
                        while not self.done:
                            self.step()


                def qk_unit(hh, mg):
                    ps = psA.tile([128, 4, 128], dt.float32, tag="psqk")
                    for mi in range(4):
                        m = 4 * mg + mi
                        for k in range(8):
                            nc.tensor.matmul(
                                ps[:, mi, :], wqall[:, m, k, :],
                                xt[:, k, 128 * hh:128 * (hh + 1)],
                                start=(mi == 0 and k == 0),
                                stop=(mi == 3 and k == 7))
                    dest = qt_all if mg < 2 else kt_all
                    dv = dest[:].rearrange("d (h rh g) -> d h rh g",
                                           rh=128, g=16)
                    for par in range(2):
                        gb = 8 * (mg % 2) + par
                        nc.vector.tensor_tensor(
                            dv[:, hh, :, gb:gb + 7:2],
                            ps[64 * par:64 * par + 64].rearrange(
                                "d mi rh -> d rh mi"),
                            bqk_sb[64 * par:64 * par + 64,
                                   4 * mg:4 * mg + 4][:, None, :]
                            .broadcast_to([64, 128, 4]),
                            op=OP.add)

                def v_head(h):
                    vsb = vst.tile([128, 2, 512], dt.bfloat16, tag="vsb",
                                   name=f"vsb{h}")
                    for jv in range(2):
                        ps = psB.tile([128, 512], dt.float32,
                                      tag=("pvA", "pvB")[jv], bufs=1)
                        for k in range(8):
                            nc.tensor.matmul(
                                ps[:], xt[:, k, 128 * h:128 * (h + 1)],
                                wvt[jv][:, k, :],
                                start=(k == 0), stop=(k == 7))
                        nc.vector.tensor_copy(vsb[:, jv, :], ps[:])
                    # V bias via a broadcast accumulate-DMA (frees PE of the
                    # ones-row bias matmuls; bias varies along columns)
                    nc.gpsimd.dma_start(
                        vsb[:], bvn[0:1, :].rearrange(
                            "o (jv c) -> o jv c", jv=2)
                        .broadcast_to([128, 2, 512]),
                        accum_op=OP.add)
                    # re-partition to [s, hd] via a DRAM bounce (the gather's
                    # source AP mixes partition bits into free dims):
                    # s = 16*rr + g, g = 8*jv + g2 -> partition (r jv g2)
                    nc.sync.dma_start(vscr[h][:], vsb[:])
                    src = vscr[h][:].rearrange(
                        "(i r) jv (g2 d) -> (r jv g2) i d", r=8, d=64)
                    nc.sync.dma_start(vn[:, h, :, 0:64], src)

                # ---------------- the pipelined schedule ----------------
                runs = {}
                for h in range(HPC):
                    for j in range(4):
                        runs[(h, j)] = HeadRun(h, j, on_act=(j == 0))

                for mg in range(4):           # head 0 projection
                    qk_unit(0, mg)
                v_head(0)
                runs[(0, 3)].sexp(2)          # h0's exp starts during V
                v_head(1)
                runs[(0, 3)].sexp(2)
                v_head(2)
                runs[(0, 3)].sexp(2)
                v_head(3)
                runs[(0, 3)].sexp(2)

                for hh in range(1, HPC):      # heads 1-3 projection,
                    pr = runs[(hh - 1, 3)]    # interleaved with attn(hh-1)
                    for mg in range(4):
                        qk_unit(hh, mg)
                        pr.sexp(2)
                        drip(1)
                        pr.pv(3)
                        # keep ACT fed: pre-emit j=2 S/exp of done heads
                        runs[(hh - 1, 2)].sexp(1)
                    pr.pv(3)                  # finish the head
                    runs[(hh - 1, 2)].sexp(1)

                runs[(3, 3)].run_all()        # last head's big block

                for tt in range(12, 16):
                    pending.append(make_proj_one(3, tt, 0))
                    pending.append(make_proj_one(3, tt, 1))

                for jx, j in enumerate([2, 1, 0]):
                    for h in range(HPC):
                        runs[(h, j)].run_all()
                    for tt in range(4 * j, 4 * j + 4):
                        if j == 0:
                            pending.append(make_proj_one(j, tt, 0, last=True))
                        else:
                            for cc in range(2):
                                pending.append(make_proj_one(j, tt, cc))
                for fn in pending[:]:
                    pending.pop(0)()
    nc.compile()
    return nc


_NC_CACHE = None


def _get_program():
    global _NC_CACHE
    if _NC_CACHE is None:
        _NC_CACHE = build_program()
    return _NC_CACHE


def _prep_core_inputs(x, Wqkv, bqkv, Wproj, bproj):
    """Build the 8 per-core input dicts (host-side shard + layout prep)."""
    x = np.asarray(x, dtype=np.float32)
    Wqkv = np.ascontiguousarray(np.asarray(Wqkv, dtype=np.float32))
    bqkv = np.asarray(bqkv, dtype=np.float32)
    Wproj = np.asarray(Wproj, dtype=np.float32)

    wq_np = np.ascontiguousarray(
        Wqkv[:, :2048].reshape(8, 128, 16, 128).transpose(2, 1, 0, 3)
        .reshape(16, 128, 8 * 128)).astype(BF16)
    wv_np = np.ascontiguousarray(
        Wqkv[:, 2048:].reshape(8, 128, 2, 512).transpose(2, 1, 0, 3)
        .reshape(2, 128, 8 * 512)).astype(BF16)
    bqk_np = np.ascontiguousarray(bqkv[:2048].reshape(16, 128).T)
    bvn_np = np.ascontiguousarray(bqkv[2048:].reshape(1, 1024)).astype(BF16)

    in_maps = []
    for c in range(N_CORES):
        b, q = divmod(c, 4)
        xT_np = np.ascontiguousarray(
            x[b, RPC * q:RPC * (q + 1), :].reshape(RPC, 8, 128)
            .transpose(2, 1, 0)).astype(BF16)
        wp_np = np.ascontiguousarray(
            Wproj[256 * q:256 * (q + 1), :].reshape(2, 128, 1024)
            .transpose(1, 0, 2).reshape(128, 2048)).astype(BF16)
        in_maps.append({
            "xT": xT_np, "wq": wq_np, "wv": wv_np, "bqk": bqk_np,
            "bvn": bvn_np, "wp": wp_np,
        })
    return in_maps


def kernel(x, Wqkv, bqkv, Wproj, bproj):
    nc = _get_program()
    in_maps = _prep_core_inputs(x, Wqkv, bqkv, Wproj, bproj)
    res = run_bass_kernel_spmd(nc, in_maps, list(range(N_CORES)))
    out = np.zeros((B, T, C), dtype=np.float32)
    for c in range(N_CORES):
        out[c // 4] += res.results[c]["out"].astype(np.float32)
    out += np.asarray(bproj, dtype=np.float32)
    return out


# revision 46
# speedup vs baseline: 1.4549x; 1.0060x over previous
"""Causal self-attention (dense transformer) on 8 trn2 NeuronCores.

Reference semantics (note the headless reshape):
  x_proj = x @ Wqkv + bqkv                     # [B, T, 3C]
  q = x_proj[:, :, :C].reshape(B, H, T, hd)    # direct reshape, no transpose!
Because of the direct reshape, head h consumes the contiguous row block
x_proj[b, h*128:(h+1)*128, :] reinterpreted as [T, hd].  So sharding by
(batch, head-group) makes QKV projection + attention fully core-local;
only the output projection is a row-parallel partial sum, reduced on host.

Shapes (hardcoded): B=2, T=2048, C=1024, n_head=16, hd=64, 8 cores.
Core c: batch b=c//4, quarter q=c%4 -> x rows [512q, 512q+512), heads 4q..4q+3.

v4 design notes (cost model: matmul cost = out-free-size x cycles/row; K and
partition count are free; bf16 is 1 cyc/row at ANY width, f32r only >=256):
- bf16 everywhere (PSUM fp32).  Total error ~4e-3 vs the 2e-2 gate.
- QK projection is HEAD-BLOCKED (per (head, m-group-of-4) unit, 32 matmuls
  of 128-wide bf16 into one psum bank / one accumulation group), and the
  whole schedule is SOFTWARE-PIPELINED BY HEAD: head h's attention
  (S -> exp -> P@V) is emitted interleaved with head h+1's projection
  units, so the ACT engine (exp, the #2 load at ~70us) starts ~18us in and
  never waits for the full projection.
- order: QK(h0) | V(all heads, + S/exp of h0 woven) | QK(h1)+attn(h0) |
  QK(h2)+attn(h1) | QK(h3)+attn(h2) | attn(h3) | remaining query blocks
  j=2,1,0 with drip-fed normalization + projection closures.
- PSUM bank lifetimes telescope: QK pool (2) and V pool (2) + S pairs (4)
  early; V pool is then traded for the Y^T-transpose bank and the j=3 yn
  bank; the QK pool is traded for the j<=2 yn pool; the first yn bank is
  traded for the projection bank.  Always exactly 8 banks.
- eviction of q^T/k^T: DVE tensor_tensor add with a stride-0-broadcast
  per-(partition, m) bias AP; stride-16 shuffled dest APs.
- V natural with ones-row bias matmul (the V bias varies with s%16 via the
  headless reshape, so it must be added on x_proj columns), bf16-evicted,
  re-partitioned to [s, hd] tiles via a DRAM bounce.
- S^T tiles at causally-minimal widths (512/384/256/128); exp straight off
  2-bank psum with scale=1/8; only the exact-diagonal 128x128 block is
  masked, out-of-place into a small ptd tile (mask off the critical path).
- P@V natural-Y: yn[q=128, 4, 65] accumulates over s-tiles with lhsT =
  pt/ptd 128-col slices: 65 cycles per (s-tile, q-block) instead of 512.
  Above-diagonal blocks skipped; ones-column gives the denominator.
- normalization: batched reciprocal, one broadcast tensor_tensor eviction,
  PE-transpose (identity matmul) to Y^T.
- projection: 512-wide dripped units; the last query block runs 1024-wide
  units on the freed S-psum banks with ACT/DVE alternating evictions.
- host: 4-way partial reduction + bproj in fp32.
"""

import os

import numpy as np
import ml_dtypes

os.environ.setdefault("NEURON_RT_RESET_CORES", "1")

import concourse.bacc as bacc
import concourse.mybir as mybir
import concourse.tile as tile
from concourse import masks
from concourse.bass_utils import run_bass_kernel_spmd

dt = mybir.dt
AF = mybir.ActivationFunctionType
OP = mybir.AluOpType
BF16 = np.dtype(ml_dtypes.bfloat16)

B, T, C = 2, 2048, 1024
NH, HD = 16, 64
N_CORES = 8
HPC = 4          # heads per core
RPC = 512        # x rows per core
SCALE = 1.0 / 8.0   # 1/sqrt(hd), folded into the exp activation


def _pair_members(j, u):
    """s-tile pair u of query block j: list of (i, qoff, psum_off, width)."""
    if u < 2 * j:
        return [(2 * u, 0, 0, 512), (2 * u + 1, 0, 512, 512)]
    if u == 2 * j:
        return [(4 * j, 0, 0, 512), (4 * j + 1, 128, 512, 384)]
    return [(4 * j + 2, 256, 0, 256), (4 * j + 3, 384, 256, 128)]


def build_program():
    nc = bacc.Bacc("TRN2", target_bir_lowering=False, debug=False,
                   num_devices=N_CORES)

    # ---- DRAM I/O (per core) ----
    xT = nc.dram_tensor("xT", [128, 8, RPC], dt.bfloat16, kind="ExternalInput")
    wq = nc.dram_tensor("wq", [16, 128, 8 * 128], dt.bfloat16, kind="ExternalInput")
    wv = nc.dram_tensor("wv", [2, 128, 8 * 512], dt.bfloat16, kind="ExternalInput")
    bqk = nc.dram_tensor("bqk", [128, 16], dt.float32, kind="ExternalInput")
    bvn = nc.dram_tensor("bvn", [1, 1024], dt.bfloat16, kind="ExternalInput")
    wp = nc.dram_tensor("wp", [128, 2 * 1024], dt.bfloat16, kind="ExternalInput")
    out_d = nc.dram_tensor("out", [T, C], dt.bfloat16, kind="ExternalOutput")

    with tile.TileContext(nc) as tc:
        with tc.tile_pool(name="persist", bufs=1) as pp, \
             tc.tile_pool(name="drampool", bufs=1, space="DRAM") as dp:
            vscr = [dp.tile([128, 2, 512], dt.bfloat16, tag=f"vscr{h}",
                            name=f"vscr{h}") for h in range(HPC)]
            xt = pp.tile([128, 8, RPC], dt.bfloat16, tag="xt")
            bqk_sb = pp.tile([128, 16], dt.float32, tag="bqk")
            bvn_sb = pp.tile([1, 1024], dt.bfloat16, tag="bvn")
            onesr = pp.tile([1, 128], dt.bfloat16, tag="onesr")
            wp_sb = pp.tile([128, 2, 1024], dt.bfloat16, tag="wp")
            ident = pp.tile([128, 128], dt.bfloat16, tag="ident")

            qt_all = pp.tile([64, HPC * T], dt.bfloat16, tag="qt_all")
            kt_all = pp.tile([64, HPC * T], dt.bfloat16, tag="kt_all")
            vn = pp.tile([128, HPC, 16, 65], dt.bfloat16, tag="vn")
            yt = pp.tile([128, 2, T], dt.bfloat16, tag="yt")

            masks.make_identity(nc, ident[:])
            nc.gpsimd.memset(onesr[:], 1.0)
            nc.gpsimd.memset(vn[:, :, :, 64], 1.0)

            # p-state warmup: the PE clock ramp keys off the FIRST busy
            # time; burn it on the identity tile while input DMAs land
            with tc.tile_pool(name="warm", bufs=1, space="PSUM") as pw:
                wps = pw.tile([128, 128], dt.float32, tag="w")
                for _ in range(18):
                    nc.tensor.matmul(wps[:], ident[:], ident[:],
                                     start=True, stop=True)

            with tc.tile_pool(name="ptpool", bufs=30) as ptp, \
                 tc.tile_pool(name="ptdpool", bufs=14) as ptdp, \
                 tc.tile_pool(name="misc", bufs=3) as mp, \
                 tc.tile_pool(name="osb", bufs=3) as osbp, \
                 tc.tile_pool(name="wstream", bufs=1) as ws, \
                 tc.tile_pool(name="vstage", bufs=3) as vst, \
                 tc.tile_pool(name="ps2", bufs=2, space="PSUM") as ps2, \
                 tc.tile_pool(name="psA", bufs=2, space="PSUM") as psA, \
                 tc.tile_pool(name="psB", bufs=2, space="PSUM") as psB:

                # bank-reuse view allocators: psA's 2 banks serve the QK
                # units and later the j<=2 yn accumulators; psB's 2 banks
                # serve the V units and later the j=3 yn / Y^T-transpose /
                # projection tiles.  Always 8 banks total.
                def alloc_yn(h, j):
                    if j == 3 and h < 3:
                        tl = psB.tile([128, 512], dt.float32, tag="pvB",
                                      bufs=1, name=f"ynb{h}{j}")
                        return tl[:].rearrange("p (a b) -> p a b",
                                               a=4)[:, :, 0:65]
                    tl = psA.tile([128, 4, 128], dt.float32, tag="psqk",
                                  name=f"yna{h}{j}")
                    return tl[:, :, 0:65]

                def alloc_ytr(h, j):
                    tl = psB.tile([128, 512], dt.float32, tag="pvA",
                                  bufs=1, name=f"ytrt{h}{j}")
                    return tl[:].bitcast(dt.bfloat16)[0:64, 0:512]

                def alloc_po(tt, cc):
                    tl = psB.tile([128, 512], dt.float32, tag="pvB",
                                  bufs=1, name=f"pot{tt}{cc}")
                    return tl[:]

                # ---------------- input DMAs ----------------
                wqall = ws.tile([128, 16, 8, 128], dt.bfloat16, tag="wqall")
                nc.sync.dma_start(xt[:, 0:2, :], xT[:, 0:2, :])
                for c4 in range(4):
                    if c4 == 1:
                        nc.sync.dma_start(xt[:, 2:8, :], xT[:, 2:8, :])
                    if c4 == 0:
                        nc.sync.dma_start(
                            wqall[:, 0:2], wq[0:2].rearrange(
                                "m p (k j) -> p m k j", k=8))
                        nc.sync.dma_start(
                            wqall[:, 2:4], wq[2:4].rearrange(
                                "m p (k j) -> p m k j", k=8))
                        continue
                    nc.sync.dma_start(
                        wqall[:, 4 * c4:4 * c4 + 4],
                        wq[4 * c4:4 * c4 + 4].rearrange(
                            "m p (k j) -> p m k j", k=8))
                    if c4 == 1:
                        nc.sync.dma_start(bqk_sb[:], bqk[:])
                        nc.sync.dma_start(bvn_sb[:], bvn[:])
                # V/proj weights: virtual-time delayed so the wq stream owns
                # the DMA engine while it feeds PE; wv0 lands right as the
                # first V unit needs it
                wvt = [ws.tile([128, 8, 512], dt.bfloat16, tag="wvt",
                               bufs=2, name=f"wvt{jv}") for jv in range(2)]
                for jv in range(2):
                    tc.tile_set_cur_wait(0.0145 + 0.0035 * jv)
                    nc.sync.dma_start(wvt[jv][:], wv[jv].rearrange(
                        "p (k j) -> p k j", k=8))
                tc.tile_set_cur_wait(0.021)
                nc.sync.dma_start(wp_sb[:], wp.rearrange(
                    "p (t c) -> p t c", t=2))
                tc.tile_set_cur_wait(0.0)

                # ---------------- emission helpers ----------------
                def emit_sexp(h, j, u):
                    """S^T matmuls for pair u + exp + exact-diagonal mask."""
                    mem = _pair_members(j, u)
                    tot = mem[-1][2] + mem[-1][3]
                    same_bank = tot <= 512   # pair B: one group, start/stop split
                    ssp = ps2.tile([128, 1024], dt.float32, tag="ssp",
                                   name=f"ssp{h}{j}{u}")
                    for mi, (i, qoff, off, w) in enumerate(mem):
                        nc.tensor.matmul(
                            ssp[:, off:off + w],
                            kt_all[:, T * h + 128 * i:T * h + 128 * (i + 1)],
                            qt_all[:, T * h + 512 * j + qoff:
                                   T * h + 512 * j + qoff + w],
                            start=(not same_bank) or mi == 0,
                            stop=(not same_bank) or mi == len(mem) - 1)
                    pt = ptp.tile([128, 1024], dt.bfloat16, tag="pt",
                                  name=f"pt{h}{j}{u}")
                    nc.scalar.activation(pt[:, 0:tot], ssp[:, 0:tot],
                                         AF.Exp, scale=SCALE)
                    dmap = {}
                    if u >= 2 * j:  # mask the exact diagonal, out-of-place
                        for (i, qoff, off, w) in mem:
                            dcol = off + 128 * (i - 4 * j) - qoff
                            ptd = ptdp.tile([128, 128], dt.bfloat16,
                                            tag="ptd", name=f"ptd{h}{j}{u}{i}")
                            nc.gpsimd.affine_select(
                                out=ptd[:], in_=pt[:, dcol:dcol + 128],
                                compare_op=OP.is_ge, fill=0.0,
                                base=0, channel_multiplier=-1,
                                pattern=[[1, 128]])
                            dmap[i] = ptd
                    return pt, mem, dmap

                pending = []   # deferred (norm / proj) closures, drip-fed

                def drip(n=1):
                    npop = min(n + (len(pending) > 4) + (len(pending) > 8),
                               len(pending))
                    for _ in range(npop):
                        pending.pop(0)()

                def make_norm(h, j, yn, on_act=False):
                    def norm():
                        rin = mp.tile([128, 4], dt.float32, tag="rin",
                                      name=f"rin{h}{j}")
                        nc.vector.tensor_copy(rin[:], yn[:, :, 64])
                        rcp = mp.tile([128, 4], dt.float32, tag="rcp",
                                      name=f"rcp{h}{j}")
                        with nc.allow_low_precision(reason="softmax recip"):
                            nc.vector.reciprocal(rcp[:], rin[:])
                        yb = mp.tile([128, 4, 64], dt.bfloat16, tag="yb",
                                     name=f"yb{h}{j}")
                        nc.vector.tensor_tensor(
                            yb[:], yn[:, :, 0:64],
                            rcp[:, :, None].broadcast_to([128, 4, 64]),
                            op=OP.mult)
                        ytr = alloc_ytr(h, j)
                        for qb in range(4):
                            nc.tensor.transpose(
                                ytr[:, 128 * qb:128 * (qb + 1)],
                                yb[:, qb, :], ident[:])
                        dst = yt[64 * (h % 2):64 * (h % 2) + 64,
                                 h // 2, 512 * j:512 * (j + 1)]
                        if on_act:
                            nc.scalar.copy(dst, ytr)
                        else:
                            nc.vector.tensor_copy(dst, ytr)
                    return norm

                def make_proj_one(j, tt, cc, last=False):
                    def proj():
                        if last:
                            po = ps2.tile([128, 1024], dt.float32,
                                          tag="ssp", name=f"pol{tt}")
                            # one matmul per psum bank (a matmul must not
                            # cross a bank boundary)
                            for half in range(2):
                                for p in range(2):
                                    nc.tensor.matmul(
                                        po[:, 512 * half:512 * (half + 1)],
                                        yt[:, p, 128 * tt:128 * (tt + 1)],
                                        wp_sb[:, p, 512 * half:
                                              512 * (half + 1)],
                                        start=(p == 0), stop=(p == 1))
                            ot = osbp.tile([128, 1024], dt.bfloat16,
                                           tag="otw", name=f"otw{tt}")
                            if tt % 2 == 0:
                                nc.scalar.copy(ot[:], po[:])
                            else:
                                nc.vector.tensor_copy(ot[:], po[:])
                            nc.sync.dma_start(
                                out_d[128 * tt:128 * (tt + 1), :], ot[:])
                            return
                        po = alloc_po(tt, cc)
                        for p in range(2):
                            nc.tensor.matmul(
                                po, yt[:, p, 128 * tt:128 * (tt + 1)],
                                wp_sb[:, p, 512 * cc:512 * (cc + 1)],
                                start=(p == 0), stop=(p == 1))
                        ot = osbp.tile([128, 512], dt.bfloat16,
                                       tag="ot", name=f"ot{tt}{cc}")
                        nc.vector.tensor_copy(ot[:], po)
                        nc.sync.dma_start(
                            out_d[128 * tt:128 * (tt + 1),
                                  512 * cc:512 * (cc + 1)], ot[:])
                    return proj

                class HeadRun:
                    """Incremental emitter for one (head, query-block)."""

                    def __init__(self, h, j, on_act=False):
                        self.h, self.j = h, j
                        self.n_u = 2 * j + 2
                        self.units = []
                        self.np_ = 0
                        self.yn = None
                        self.on_act = on_act
                        self.done = False

                    def sexp(self, k=1):
                        for _ in range(k):
                            if len(self.units) >= self.n_u:
                                return
                            self.units.append(
                                emit_sexp(self.h, self.j, len(self.units)))

                    def pv(self, k=1):
                        ns = len(self.units)
                        la = 2
                        lim = ns if ns == self.n_u else max(ns - la, 0)
                        for _ in range(k):
                            if self.np_ >= lim:
                                break
                            if self.yn is None:
                                self.yn = alloc_yn(self.h, self.j)
                            u = self.np_
                            pt, mem, dmap = self.units[u]
                            for mi, (i, qoff, off, w) in enumerate(mem):
                                qbs = list(range(
                                    max(qoff // 128, i - 4 * self.j), 4))
                                for qi, qb in enumerate(qbs):
                                    if qb == i - 4 * self.j:
                                        lhs = dmap[i][:]
                                    else:
                                        col = off + 128 * qb - qoff
                                        lhs = pt[:, col:col + 128]
                                    nc.tensor.matmul(
                                        self.yn[:, qb, :], lhs,
                                        vn[:, self.h, i, :],
                                        start=(u == 0 and mi == 0
                                               and qi == 0),
                                        stop=(u == self.n_u - 1
                                              and mi == len(mem) - 1
                                              and qi == len(qbs) - 1))
                            self.units[u] = None
                            self.np_ += 1
                        if self.np_ == self.n_u and not self.done:
                            self.done = True
                            pending.append(make_norm(self.h, self.j, self.yn,
                                                     self.on_act))

                    def step(self):
                        self.sexp(1)
                        drip(1)
                        self.pv(1)

                    def run_all(self):
                        while not self.done:
                            self.step()


                def qk_unit(hh, mg):
                    ps = psA.tile([128, 4, 128], dt.float32, tag="psqk")
                    for mi in range(4):
                        m = 4 * mg + mi
                        for k in range(8):
                            nc.tensor.matmul(
                                ps[:, mi, :], wqall[:, m, k, :],
                                xt[:, k, 128 * hh:128 * (hh + 1)],
                                start=(mi == 0 and k == 0),
                                stop=(mi == 3 and k == 7))
                    dest = qt_all if mg < 2 else kt_all
                    dv = dest[:].rearrange("d (h rh g) -> d h rh g",
                                           rh=128, g=16)
                    for par in range(2):
                        gb = 8 * (mg % 2) + par
                        nc.vector.tensor_tensor(
                            dv[:, hh, :, gb:gb + 7:2],
                            ps[64 * par:64 * par + 64].rearrange(
                                "d mi rh -> d rh mi"),
                            bqk_sb[64 * par:64 * par + 64,
                                   4 * mg:4 * mg + 4][:, None, :]
                            .broadcast_to([64, 128, 4]),
                            op=OP.add)

                def v_head(h):
                    vsb = vst.tile([128, 2, 512], dt.bfloat16, tag="vsb",
                                   name=f"vsb{h}")
                    for jv in range(2):
                        ps = psB.tile([128, 512], dt.float32,
                                      tag=("pvA", "pvB")[jv], bufs=1)
                        for k in range(8):
                            nc.tensor.matmul(
                                ps[:], xt[:, k, 128 * h:128 * (h + 1)],
                                wvt[jv][:, k, :],
                                start=(k == 0), stop=(k == 7))
                        nc.vector.tensor_copy(vsb[:, jv, :], ps[:])
                    # V bias via a broadcast accumulate-DMA (frees PE of the
                    # ones-row bias matmuls; bias varies along columns)
                    nc.gpsimd.dma_start(
                        vsb[:], bvn[0:1, :].rearrange(
                            "o (jv c) -> o jv c", jv=2)
                        .broadcast_to([128, 2, 512]),
                        accum_op=OP.add)
                    # re-partition to [s, hd] via a DRAM bounce (the gather's
                    # source AP mixes partition bits into free dims):
                    # s = 16*rr + g, g = 8*jv + g2 -> partition (r jv g2)
                    nc.sync.dma_start(vscr[h][:], vsb[:])
                    src = vscr[h][:].rearrange(
                        "(i r) jv (g2 d) -> (r jv g2) i d", r=8, d=64)
                    nc.sync.dma_start(vn[:, h, :, 0:64], src)

                # ---------------- the pipelined schedule ----------------
                runs = {}
                for h in range(HPC):
                    for j in range(4):
                        runs[(h, j)] = HeadRun(h, j, on_act=(j == 0))

                for mg in range(4):           # head 0 projection
                    qk_unit(0, mg)
                v_head(0)
                runs[(0, 3)].sexp(2)          # h0's exp starts during V
                v_head(1)
                runs[(0, 3)].sexp(2)
                v_head(2)
                runs[(0, 3)].sexp(2)
                v_head(3)
                runs[(0, 3)].sexp(2)

                for hh in range(1, HPC):      # heads 1-3 projection,
                    pr = runs[(hh - 1, 3)]    # interleaved with attn(hh-1)
                    for mg in range(4):
                        qk_unit(hh, mg)
                        pr.sexp(3)
                        drip(1)
                        pr.pv(3)
                        # keep ACT fed: pre-emit j=2 S/exp of done heads
                        runs[(hh - 1, 2)].sexp(1)
                    pr.pv(3)                  # finish the head
                    runs[(hh - 1, 2)].sexp(1)

                runs[(3, 3)].run_all()        # last head's big block

                for tt in range(12, 16):
                    pending.append(make_proj_one(3, tt, 0))
                    pending.append(make_proj_one(3, tt, 1))

                for jx, j in enumerate([2, 1, 0]):
                    for h in range(HPC):
                        runs[(h, j)].run_all()
                    for tt in range(4 * j, 4 * j + 4):
                        if j == 0:
                            pending.append(make_proj_one(j, tt, 0, last=True))
                        else:
                            for cc in range(2):
                                pending.append(make_proj_one(j, tt, cc))
                for fn in pending[:]:
                    pending.pop(0)()
    nc.compile()
    return nc


_NC_CACHE = None


def _get_program():
    global _NC_CACHE
    if _NC_CACHE is None:
        _NC_CACHE = build_program()
    return _NC_CACHE


def _prep_core_inputs(x, Wqkv, bqkv, Wproj, bproj):
    """Build the 8 per-core input dicts (host-side shard + layout prep)."""
    x = np.asarray(x, dtype=np.float32)
    Wqkv = np.ascontiguousarray(np.asarray(Wqkv, dtype=np.float32))
    bqkv = np.asarray(bqkv, dtype=np.float32)
    Wproj = np.asarray(Wproj, dtype=np.float32)

    wq_np = np.ascontiguousarray(
        Wqkv[:, :2048].reshape(8, 128, 16, 128).transpose(2, 1, 0, 3)
        .reshape(16, 128, 8 * 128)).astype(BF16)
    wv_np = np.ascontiguousarray(
        Wqkv[:, 2048:].reshape(8, 128, 2, 512).transpose(2, 1, 0, 3)
        .reshape(2, 128, 8 * 512)).astype(BF16)
    bqk_np = np.ascontiguousarray(bqkv[:2048].reshape(16, 128).T)
    bvn_np = np.ascontiguousarray(bqkv[2048:].reshape(1, 1024)).astype(BF16)

    in_maps = []
    for c in range(N_CORES):
        b, q = divmod(c, 4)
        xT_np = np.ascontiguousarray(
            x[b, RPC * q:RPC * (q + 1), :].reshape(RPC, 8, 128)
            .transpose(2, 1, 0)).astype(BF16)
        wp_np = np.ascontiguousarray(
            Wproj[256 * q:256 * (q + 1), :].reshape(2, 128, 1024)
            .transpose(1, 0, 2).reshape(128, 2048)).astype(BF16)
        in_maps.append({
            "xT": xT_np, "wq": wq_np, "wv": wv_np, "bqk": bqk_np,
            "bvn": bvn_np, "wp": wp_np,
        })
    return in_maps


def kernel(x, Wqkv, bqkv, Wproj, bproj):
    nc = _get_program()
    in_maps = _prep_core_inputs(x, Wqkv, bqkv, Wproj, bproj)
    res = run_bass_kernel_spmd(nc, in_maps, list(range(N_CORES)))
    out = np.zeros((B, T, C), dtype=np.float32)
    for c in range(N_CORES):
        out[c // 4] += res.results[c]["out"].astype(np.float32)
    out += np.asarray(bproj, dtype=np.float32)
    return out


# revision 48
# speedup vs baseline: 1.4610x; 1.0042x over previous
"""Causal self-attention (dense transformer) on 8 trn2 NeuronCores.

Reference semantics (note the headless reshape):
  x_proj = x @ Wqkv + bqkv                     # [B, T, 3C]
  q = x_proj[:, :, :C].reshape(B, H, T, hd)    # direct reshape, no transpose!
Because of the direct reshape, head h consumes the contiguous row block
x_proj[b, h*128:(h+1)*128, :] reinterpreted as [T, hd].  So sharding by
(batch, head-group) makes QKV projection + attention fully core-local;
only the output projection is a row-parallel partial sum, reduced on host.

Shapes (hardcoded): B=2, T=2048, C=1024, n_head=16, hd=64, 8 cores.
Core c: batch b=c//4, quarter q=c%4 -> x rows [512q, 512q+512), heads 4q..4q+3.

v4 design notes (cost model: matmul cost = out-free-size x cycles/row; K and
partition count are free; bf16 is 1 cyc/row at ANY width, f32r only >=256):
- bf16 everywhere (PSUM fp32).  Total error ~4e-3 vs the 2e-2 gate.
- QK projection is HEAD-BLOCKED (per (head, m-group-of-4) unit, 32 matmuls
  of 128-wide bf16 into one psum bank / one accumulation group), and the
  whole schedule is SOFTWARE-PIPELINED BY HEAD: head h's attention
  (S -> exp -> P@V) is emitted interleaved with head h+1's projection
  units, so the ACT engine (exp, the #2 load at ~70us) starts ~18us in and
  never waits for the full projection.
- order: QK(h0) | V(all heads, + S/exp of h0 woven) | QK(h1)+attn(h0) |
  QK(h2)+attn(h1) | QK(h3)+attn(h2) | attn(h3) | remaining query blocks
  j=2,1,0 with drip-fed normalization + projection closures.
- PSUM bank lifetimes telescope: QK pool (2) and V pool (2) + S pairs (4)
  early; V pool is then traded for the Y^T-transpose bank and the j=3 yn
  bank; the QK pool is traded for the j<=2 yn pool; the first yn bank is
  traded for the projection bank.  Always exactly 8 banks.
- eviction of q^T/k^T: DVE tensor_tensor add with a stride-0-broadcast
  per-(partition, m) bias AP; stride-16 shuffled dest APs.
- V natural with ones-row bias matmul (the V bias varies with s%16 via the
  headless reshape, so it must be added on x_proj columns), bf16-evicted,
  re-partitioned to [s, hd] tiles via a DRAM bounce.
- S^T tiles at causally-minimal widths (512/384/256/128); exp straight off
  2-bank psum with scale=1/8; only the exact-diagonal 128x128 block is
  masked, out-of-place into a small ptd tile (mask off the critical path).
- P@V natural-Y: yn[q=128, 4, 65] accumulates over s-tiles with lhsT =
  pt/ptd 128-col slices: 65 cycles per (s-tile, q-block) instead of 512.
  Above-diagonal blocks skipped; ones-column gives the denominator.
- normalization: batched reciprocal, one broadcast tensor_tensor eviction,
  PE-transpose (identity matmul) to Y^T.
- projection: 512-wide dripped units; the last query block runs 1024-wide
  units on the freed S-psum banks with ACT/DVE alternating evictions.
- host: 4-way partial reduction + bproj in fp32.
"""

import os

import numpy as np
import ml_dtypes

os.environ.setdefault("NEURON_RT_RESET_CORES", "1")

import concourse.bacc as bacc
import concourse.mybir as mybir
import concourse.tile as tile
from concourse import masks
from concourse.bass_utils import run_bass_kernel_spmd

dt = mybir.dt
AF = mybir.ActivationFunctionType
OP = mybir.AluOpType
BF16 = np.dtype(ml_dtypes.bfloat16)

B, T, C = 2, 2048, 1024
NH, HD = 16, 64
N_CORES = 8
HPC = 4          # heads per core
RPC = 512        # x rows per core
SCALE = 1.0 / 8.0   # 1/sqrt(hd), folded into the exp activation


def _pair_members(j, u):
    """s-tile pair u of query block j: list of (i, qoff, psum_off, width)."""
    if u < 2 * j:
        return [(2 * u, 0, 0, 512), (2 * u + 1, 0, 512, 512)]
    if u == 2 * j:
        return [(4 * j, 0, 0, 512), (4 * j + 1, 128, 512, 384)]
    return [(4 * j + 2, 256, 0, 256), (4 * j + 3, 384, 256, 128)]


def build_program():
    nc = bacc.Bacc("TRN2", target_bir_lowering=False, debug=False,
                   num_devices=N_CORES)

    # ---- DRAM I/O (per core) ----
    xT = nc.dram_tensor("xT", [128, 8, RPC], dt.bfloat16, kind="ExternalInput")
    wq = nc.dram_tensor("wq", [16, 128, 8 * 128], dt.bfloat16, kind="ExternalInput")
    wv = nc.dram_tensor("wv", [2, 128, 8 * 512], dt.bfloat16, kind="ExternalInput")
    bqk = nc.dram_tensor("bqk", [128, 16], dt.float32, kind="ExternalInput")
    bvn = nc.dram_tensor("bvn", [1, 1024], dt.bfloat16, kind="ExternalInput")
    wp = nc.dram_tensor("wp", [128, 2 * 1024], dt.bfloat16, kind="ExternalInput")
    out_d = nc.dram_tensor("out", [T, C], dt.bfloat16, kind="ExternalOutput")

    with tile.TileContext(nc) as tc:
        with tc.tile_pool(name="persist", bufs=1) as pp, \
             tc.tile_pool(name="drampool", bufs=1, space="DRAM") as dp:
            vscr = [dp.tile([128, 2, 512], dt.bfloat16, tag=f"vscr{h}",
                            name=f"vscr{h}") for h in range(HPC)]
            xt = pp.tile([128, 8, RPC], dt.bfloat16, tag="xt")
            bqk_sb = pp.tile([128, 16], dt.float32, tag="bqk")
            bvn_sb = pp.tile([1, 1024], dt.bfloat16, tag="bvn")
            onesr = pp.tile([1, 128], dt.bfloat16, tag="onesr")
            wp_sb = pp.tile([128, 2, 1024], dt.bfloat16, tag="wp")
            ident = pp.tile([128, 128], dt.bfloat16, tag="ident")

            qt_all = pp.tile([64, HPC * T], dt.bfloat16, tag="qt_all")
            kt_all = pp.tile([64, HPC * T], dt.bfloat16, tag="kt_all")
            vn = pp.tile([128, HPC, 16, 65], dt.bfloat16, tag="vn")
            yt = pp.tile([128, 2, T], dt.bfloat16, tag="yt")

            masks.make_identity(nc, ident[:])
            nc.gpsimd.memset(onesr[:], 1.0)
            nc.gpsimd.memset(vn[:, :, :, 64], 1.0)

            # p-state warmup: the PE clock ramp keys off the FIRST busy
            # time; burn it on the identity tile while input DMAs land
            with tc.tile_pool(name="warm", bufs=1, space="PSUM") as pw:
                wps = pw.tile([128, 128], dt.float32, tag="w")
                for _ in range(18):
                    nc.tensor.matmul(wps[:], ident[:], ident[:],
                                     start=True, stop=True)

            with tc.tile_pool(name="ptpool", bufs=30) as ptp, \
                 tc.tile_pool(name="ptdpool", bufs=14) as ptdp, \
                 tc.tile_pool(name="misc", bufs=3) as mp, \
                 tc.tile_pool(name="osb", bufs=3) as osbp, \
                 tc.tile_pool(name="wstream", bufs=1) as ws, \
                 tc.tile_pool(name="vstage", bufs=3) as vst, \
                 tc.tile_pool(name="ps2", bufs=2, space="PSUM") as ps2, \
                 tc.tile_pool(name="psA", bufs=2, space="PSUM") as psA, \
                 tc.tile_pool(name="psB", bufs=2, space="PSUM") as psB:

                # bank-reuse view allocators: psA's 2 banks serve the QK
                # units and later the j<=2 yn accumulators; psB's 2 banks
                # serve the V units and later the j=3 yn / Y^T-transpose /
                # projection tiles.  Always 8 banks total.
                def alloc_yn(h, j):
                    if j == 3 and h < 3:
                        tl = psB.tile([128, 512], dt.float32, tag="pvB",
                                      bufs=1, name=f"ynb{h}{j}")
                        return tl[:].rearrange("p (a b) -> p a b",
                                               a=4)[:, :, 0:65]
                    tl = psA.tile([128, 4, 128], dt.float32, tag="psqk",
                                  name=f"yna{h}{j}")
                    return tl[:, :, 0:65]

                def alloc_ytr(h, j):
                    tl = psB.tile([128, 512], dt.float32, tag="pvA",
                                  bufs=1, name=f"ytrt{h}{j}")
                    return tl[:].bitcast(dt.bfloat16)[0:64, 0:512]

                def alloc_po(tt, cc):
                    tl = psB.tile([128, 512], dt.float32, tag="pvB",
                                  bufs=1, name=f"pot{tt}{cc}")
                    return tl[:]

                # ---------------- input DMAs ----------------
                wqall = ws.tile([128, 16, 8, 128], dt.bfloat16, tag="wqall")
                nc.sync.dma_start(xt[:, 0:2, :], xT[:, 0:2, :])
                for c4 in range(4):
                    if c4 == 1:
                        nc.sync.dma_start(xt[:, 2:8, :], xT[:, 2:8, :])
                    if c4 == 0:
                        nc.sync.dma_start(
                            wqall[:, 0:2], wq[0:2].rearrange(
                                "m p (k j) -> p m k j", k=8))
                        nc.sync.dma_start(
                            wqall[:, 2:4], wq[2:4].rearrange(
                                "m p (k j) -> p m k j", k=8))
                        continue
                    nc.sync.dma_start(
                        wqall[:, 4 * c4:4 * c4 + 4],
                        wq[4 * c4:4 * c4 + 4].rearrange(
                            "m p (k j) -> p m k j", k=8))
                    if c4 == 1:
                        nc.sync.dma_start(bqk_sb[:], bqk[:])
                        nc.sync.dma_start(bvn_sb[:], bvn[:])
                # V/proj weights: virtual-time delayed so the wq stream owns
                # the DMA engine while it feeds PE; wv0 lands right as the
                # first V unit needs it
                wvt = [ws.tile([128, 8, 512], dt.bfloat16, tag="wvt",
                               bufs=2, name=f"wvt{jv}") for jv in range(2)]
                for jv in range(2):
                    tc.tile_set_cur_wait(0.0145 + 0.0035 * jv)
                    nc.sync.dma_start(wvt[jv][:], wv[jv].rearrange(
                        "p (k j) -> p k j", k=8))
                tc.tile_set_cur_wait(0.021)
                nc.sync.dma_start(wp_sb[:], wp.rearrange(
                    "p (t c) -> p t c", t=2))
                tc.tile_set_cur_wait(0.0)

                # ---------------- emission helpers ----------------
                def emit_sexp(h, j, u):
                    """S^T matmuls for pair u + exp + exact-diagonal mask."""
                    mem = _pair_members(j, u)
                    tot = mem[-1][2] + mem[-1][3]
                    same_bank = tot <= 512   # pair B: one group, start/stop split
                    ssp = ps2.tile([128, 1024], dt.float32, tag="ssp",
                                   name=f"ssp{h}{j}{u}")
                    for mi, (i, qoff, off, w) in enumerate(mem):
                        nc.tensor.matmul(
                            ssp[:, off:off + w],
                            kt_all[:, T * h + 128 * i:T * h + 128 * (i + 1)],
                            qt_all[:, T * h + 512 * j + qoff:
                                   T * h + 512 * j + qoff + w],
                            start=(not same_bank) or mi == 0,
                            stop=(not same_bank) or mi == len(mem) - 1)
                    pt = ptp.tile([128, 1024], dt.bfloat16, tag="pt",
                                  name=f"pt{h}{j}{u}")
                    nc.scalar.activation(pt[:, 0:tot], ssp[:, 0:tot],
                                         AF.Exp, scale=SCALE)
                    dmap = {}
                    if u >= 2 * j:  # mask the exact diagonal, out-of-place
                        for (i, qoff, off, w) in mem:
                            dcol = off + 128 * (i - 4 * j) - qoff
                            ptd = ptdp.tile([128, 128], dt.bfloat16,
                                            tag="ptd", name=f"ptd{h}{j}{u}{i}")
                            nc.gpsimd.affine_select(
                                out=ptd[:], in_=pt[:, dcol:dcol + 128],
                                compare_op=OP.is_ge, fill=0.0,
                                base=0, channel_multiplier=-1,
                                pattern=[[1, 128]])
                            dmap[i] = ptd
                    return pt, mem, dmap

                pending = []   # deferred (norm / proj) closures, drip-fed

                def drip(n=1):
                    npop = min(n + (len(pending) > 4) + (len(pending) > 8),
                               len(pending))
                    for _ in range(npop):
                        pending.pop(0)()

                def make_norm(h, j, yn, on_act=False):
                    def norm():
                        rin = mp.tile([128, 4], dt.float32, tag="rin",
                                      name=f"rin{h}{j}")
                        nc.vector.tensor_copy(rin[:], yn[:, :, 64])
                        rcp = mp.tile([128, 4], dt.float32, tag="rcp",
                                      name=f"rcp{h}{j}")
                        with nc.allow_low_precision(reason="softmax recip"):
                            nc.vector.reciprocal(rcp[:], rin[:])
                        yb = mp.tile([128, 4, 64], dt.bfloat16, tag="yb",
                                     name=f"yb{h}{j}")
                        nc.vector.tensor_tensor(
                            yb[:], yn[:, :, 0:64],
                            rcp[:, :, None].broadcast_to([128, 4, 64]),
                            op=OP.mult)
                        ytr = alloc_ytr(h, j)
                        for qb in range(4):
                            nc.tensor.transpose(
                                ytr[:, 128 * qb:128 * (qb + 1)],
                                yb[:, qb, :], ident[:])
                        dst = yt[64 * (h % 2):64 * (h % 2) + 64,
                                 h // 2, 512 * j:512 * (j + 1)]
                        if on_act:
                            nc.scalar.copy(dst, ytr)
                        else:
                            nc.vector.tensor_copy(dst, ytr)
                    return norm

                def make_proj_one(j, tt, cc, last=False):
                    def proj():
                        if last:
                            po = ps2.tile([128, 1024], dt.float32,
                                          tag="ssp", name=f"pol{tt}")
                            # one matmul per psum bank (a matmul must not
                            # cross a bank boundary)
                            for half in range(2):
                                for p in range(2):
                                    nc.tensor.matmul(
                                        po[:, 512 * half:512 * (half + 1)],
                                        yt[:, p, 128 * tt:128 * (tt + 1)],
                                        wp_sb[:, p, 512 * half:
                                              512 * (half + 1)],
                                        start=(p == 0), stop=(p == 1))
                            ot = osbp.tile([128, 1024], dt.bfloat16,
                                           tag="otw", name=f"otw{tt}")
                            if tt % 2 == 0:
                                nc.scalar.copy(ot[:], po[:])
                            else:
                                nc.vector.tensor_copy(ot[:], po[:])
                            nc.sync.dma_start(
                                out_d[128 * tt:128 * (tt + 1), :], ot[:])
                            return
                        po = alloc_po(tt, cc)
                        for p in range(2):
                            nc.tensor.matmul(
                                po, yt[:, p, 128 * tt:128 * (tt + 1)],
                                wp_sb[:, p, 512 * cc:512 * (cc + 1)],
                                start=(p == 0), stop=(p == 1))
                        ot = osbp.tile([128, 512], dt.bfloat16,
                                       tag="ot", name=f"ot{tt}{cc}")
                        nc.vector.tensor_copy(ot[:], po)
                        nc.sync.dma_start(
                            out_d[128 * tt:128 * (tt + 1),
                                  512 * cc:512 * (cc + 1)], ot[:])
                    return proj

                class HeadRun:
                    """Incremental emitter for one (head, query-block)."""

                    def __init__(self, h, j, on_act=False):
                        self.h, self.j = h, j
                        self.n_u = 2 * j + 2
                        self.units = []
                        self.np_ = 0
                        self.yn = None
                        self.on_act = on_act
                        self.done = False

                    def sexp(self, k=1):
                        for _ in range(k):
                            if len(self.units) >= self.n_u:
                                return
                            self.units.append(
                                emit_sexp(self.h, self.j, len(self.units)))

                    def pv(self, k=1):
                        ns = len(self.units)
                        la = 3
                        lim = ns if ns == self.n_u else max(ns - la, 0)
                        for _ in range(k):
                            if self.np_ >= lim:
                                break
                            if self.yn is None:
                                self.yn = alloc_yn(self.h, self.j)
                            u = self.np_
                            pt, mem, dmap = self.units[u]
                            for mi, (i, qoff, off, w) in enumerate(mem):
                                qbs = list(range(
                                    max(qoff // 128, i - 4 * self.j), 4))
                                for qi, qb in enumerate(qbs):
                                    if qb == i - 4 * self.j:
                                        lhs = dmap[i][:]
                                    else:
                                        col = off + 128 * qb - qoff
                                        lhs = pt[:, col:col + 128]
                                    nc.tensor.matmul(
                                        self.yn[:, qb, :], lhs,
                                        vn[:, self.h, i, :],
                                        start=(u == 0 and mi == 0
                                               and qi == 0),
                                        stop=(u == self.n_u - 1
                                              and mi == len(mem) - 1
                                              and qi == len(qbs) - 1))
                            self.units[u] = None
                            self.np_ += 1
                        if self.np_ == self.n_u and not self.done:
                            self.done = True
                            pending.append(make_norm(self.h, self.j, self.yn,
                                                     self.on_act))

                    def step(self):
                        self.sexp(1)
                        drip(1)
                        self.pv(1)

                    def run_all(self):
                        while not self.done:
                            self.step()


                def qk_unit(hh, mg):
                    ps = psA.tile([128, 4, 128], dt.float32, tag="psqk")
                    for mi in range(4):
                        m = 4 * mg + mi
                        for k in range(8):
                            nc.tensor.matmul(
                                ps[:, mi, :], wqall[:, m, k, :],
                                xt[:, k, 128 * hh:128 * (hh + 1)],
                                start=(mi == 0 and k == 0),
                                stop=(mi == 3 and k == 7))
                    dest = qt_all if mg < 2 else kt_all
                    dv = dest[:].rearrange("d (h rh g) -> d h rh g",
                                           rh=128, g=16)
                    for par in range(2):
                        gb = 8 * (mg % 2) + par
                        nc.vector.tensor_tensor(
                            dv[:, hh, :, gb:gb + 7:2],
                            ps[64 * par:64 * par + 64].rearrange(
                                "d mi rh -> d rh mi"),
                            bqk_sb[64 * par:64 * par + 64,
                                   4 * mg:4 * mg + 4][:, None, :]
                            .broadcast_to([64, 128, 4]),
                            op=OP.add)

                def v_head(h):
                    vsb = vst.tile([128, 2, 512], dt.bfloat16, tag="vsb",
                                   name=f"vsb{h}")
                    for jv in range(2):
                        ps = psB.tile([128, 512], dt.float32,
                                      tag=("pvA", "pvB")[jv], bufs=1)
                        for k in range(8):
                            nc.tensor.matmul(
                                ps[:], xt[:, k, 128 * h:128 * (h + 1)],
                                wvt[jv][:, k, :],
                                start=(k == 0), stop=(k == 7))
                        nc.vector.tensor_copy(vsb[:, jv, :], ps[:])
                    # V bias via a broadcast accumulate-DMA (frees PE of the
                    # ones-row bias matmuls; bias varies along columns)
                    nc.gpsimd.dma_start(
                        vsb[:], bvn[0:1, :].rearrange(
                            "o (jv c) -> o jv c", jv=2)
                        .broadcast_to([128, 2, 512]),
                        accum_op=OP.add)
                    # re-partition to [s, hd] via a DRAM bounce (the gather's
                    # source AP mixes partition bits into free dims):
                    # s = 16*rr + g, g = 8*jv + g2 -> partition (r jv g2)
                    nc.sync.dma_start(vscr[h][:], vsb[:])
                    src = vscr[h][:].rearrange(
                        "(i r) jv (g2 d) -> (r jv g2) i d", r=8, d=64)
                    nc.sync.dma_start(vn[:, h, :, 0:64], src)

                # ---------------- the pipelined schedule ----------------
                runs = {}
                for h in range(HPC):
                    for j in range(4):
                        runs[(h, j)] = HeadRun(h, j, on_act=(j == 0))

                for mg in range(4):           # head 0 projection
                    qk_unit(0, mg)
                v_head(0)
                runs[(0, 3)].sexp(2)          # h0's exp starts during V
                v_head(1)
                runs[(0, 3)].sexp(2)
                v_head(2)
                runs[(0, 3)].sexp(2)
                v_head(3)
                runs[(0, 3)].sexp(2)

                for hh in range(1, HPC):      # heads 1-3 projection,
                    pr = runs[(hh - 1, 3)]    # interleaved with attn(hh-1)
                    for mg in range(4):
                        qk_unit(hh, mg)
                        pr.sexp(3)
                        drip(1)
                        pr.pv(4)
                        # keep ACT fed: pre-emit j=2 S/exp of done heads
                        runs[(hh - 1, 2)].sexp(2)
                    pr.pv(3)                  # finish the head
                    runs[(hh - 1, 2)].sexp(1)

                runs[(3, 3)].run_all()        # last head's big block

                for tt in range(12, 16):
                    pending.append(make_proj_one(3, tt, 0))
                    pending.append(make_proj_one(3, tt, 1))

                for jx, j in enumerate([2, 1, 0]):
                    for h in range(HPC):
                        runs[(h, j)].run_all()
                    for tt in range(4 * j, 4 * j + 4):
                        if j == 0:
                            pending.append(make_proj_one(j, tt, 0, last=True))
                        else:
                            for cc in range(2):
                                pending.append(make_proj_one(j, tt, cc))
                for fn in pending[:]:
                    pending.pop(0)()
    nc.compile()
    return nc


_NC_CACHE = None


def _get_program():
    global _NC_CACHE
    if _NC_CACHE is None:
        _NC_CACHE = build_program()
    return _NC_CACHE


def _prep_core_inputs(x, Wqkv, bqkv, Wproj, bproj):
    """Build the 8 per-core input dicts (host-side shard + layout prep)."""
    x = np.asarray(x, dtype=np.float32)
    Wqkv = np.ascontiguousarray(np.asarray(Wqkv, dtype=np.float32))
    bqkv = np.asarray(bqkv, dtype=np.float32)
    Wproj = np.asarray(Wproj, dtype=np.float32)

    wq_np = np.ascontiguousarray(
        Wqkv[:, :2048].reshape(8, 128, 16, 128).transpose(2, 1, 0, 3)
        .reshape(16, 128, 8 * 128)).astype(BF16)
    wv_np = np.ascontiguousarray(
        Wqkv[:, 2048:].reshape(8, 128, 2, 512).transpose(2, 1, 0, 3)
        .reshape(2, 128, 8 * 512)).astype(BF16)
    bqk_np = np.ascontiguousarray(bqkv[:2048].reshape(16, 128).T)
    bvn_np = np.ascontiguousarray(bqkv[2048:].reshape(1, 1024)).astype(BF16)

    in_maps = []
    for c in range(N_CORES):
        b, q = divmod(c, 4)
        xT_np = np.ascontiguousarray(
            x[b, RPC * q:RPC * (q + 1), :].reshape(RPC, 8, 128)
            .transpose(2, 1, 0)).astype(BF16)
        wp_np = np.ascontiguousarray(
            Wproj[256 * q:256 * (q + 1), :].reshape(2, 128, 1024)
            .transpose(1, 0, 2).reshape(128, 2048)).astype(BF16)
        in_maps.append({
            "xT": xT_np, "wq": wq_np, "wv": wv_np, "bqk": bqk_np,
            "bvn": bvn_np, "wp": wp_np,
        })
    return in_maps


def kernel(x, Wqkv, bqkv, Wproj, bproj):
    nc = _get_program()
    in_maps = _prep_core_inputs(x, Wqkv, bqkv, Wproj, bproj)
    res = run_bass_kernel_spmd(nc, in_maps, list(range(N_CORES)))
    out = np.zeros((B, T, C), dtype=np.float32)
    for c in range(N_CORES):
        out[c // 4] += res.results[c]["out"].astype(np.float32)
    out += np.asarray(bproj, dtype=np.float32)
    return out


# revision 51
# speedup vs baseline: 1.4778x; 1.0115x over previous
"""Causal self-attention (dense transformer) on 8 trn2 NeuronCores.

Reference semantics (note the headless reshape):
  x_proj = x @ Wqkv + bqkv                     # [B, T, 3C]
  q = x_proj[:, :, :C].reshape(B, H, T, hd)    # direct reshape, no transpose!
Because of the direct reshape, head h consumes the contiguous row block
x_proj[b, h*128:(h+1)*128, :] reinterpreted as [T, hd].  So sharding by
(batch, head-group) makes QKV projection + attention fully core-local;
only the output projection is a row-parallel partial sum, reduced on host.

Shapes (hardcoded): B=2, T=2048, C=1024, n_head=16, hd=64, 8 cores.
Core c: batch b=c//4, quarter q=c%4 -> x rows [512q, 512q+512), heads 4q..4q+3.

v4 design notes (cost model: matmul cost = out-free-size x cycles/row; K and
partition count are free; bf16 is 1 cyc/row at ANY width, f32r only >=256):
- bf16 everywhere (PSUM fp32).  Total error ~4e-3 vs the 2e-2 gate.
- QK projection is HEAD-BLOCKED (per (head, m-group-of-4) unit, 32 matmuls
  of 128-wide bf16 into one psum bank / one accumulation group), and the
  whole schedule is SOFTWARE-PIPELINED BY HEAD: head h's attention
  (S -> exp -> P@V) is emitted interleaved with head h+1's projection
  units, so the ACT engine (exp, the #2 load at ~70us) starts ~18us in and
  never waits for the full projection.
- order: QK(h0) | V(all heads, + S/exp of h0 woven) | QK(h1)+attn(h0) |
  QK(h2)+attn(h1) | QK(h3)+attn(h2) | attn(h3) | remaining query blocks
  j=2,1,0 with drip-fed normalization + projection closures.
- PSUM bank lifetimes telescope: QK pool (2) and V pool (2) + S pairs (4)
  early; V pool is then traded for the Y^T-transpose bank and the j=3 yn
  bank; the QK pool is traded for the j<=2 yn pool; the first yn bank is
  traded for the projection bank.  Always exactly 8 banks.
- eviction of q^T/k^T: DVE tensor_tensor add with a stride-0-broadcast
  per-(partition, m) bias AP; stride-16 shuffled dest APs.
- V natural with ones-row bias matmul (the V bias varies with s%16 via the
  headless reshape, so it must be added on x_proj columns), bf16-evicted,
  re-partitioned to [s, hd] tiles via a DRAM bounce.
- S^T tiles at causally-minimal widths (512/384/256/128); exp straight off
  2-bank psum with scale=1/8; only the exact-diagonal 128x128 block is
  masked, out-of-place into a small ptd tile (mask off the critical path).
- P@V natural-Y: yn[q=128, 4, 65] accumulates over s-tiles with lhsT =
  pt/ptd 128-col slices: 65 cycles per (s-tile, q-block) instead of 512.
  Above-diagonal blocks skipped; ones-column gives the denominator.
- normalization: batched reciprocal, one broadcast tensor_tensor eviction,
  PE-transpose (identity matmul) to Y^T.
- projection: 512-wide dripped units; the last query block runs 1024-wide
  units on the freed S-psum banks with ACT/DVE alternating evictions.
- host: 4-way partial reduction + bproj in fp32.
"""

import os

import numpy as np
import ml_dtypes

os.environ.setdefault("NEURON_RT_RESET_CORES", "1")

import concourse.bacc as bacc
import concourse.mybir as mybir
import concourse.tile as tile
from concourse import masks
from concourse.bass_utils import run_bass_kernel_spmd

dt = mybir.dt
AF = mybir.ActivationFunctionType
OP = mybir.AluOpType
BF16 = np.dtype(ml_dtypes.bfloat16)

B, T, C = 2, 2048, 1024
NH, HD = 16, 64
N_CORES = 8
HPC = 4          # heads per core
RPC = 512        # x rows per core
SCALE = 1.0 / 8.0   # 1/sqrt(hd), folded into the exp activation


def _pair_members(j, u):
    """s-tile pair u of query block j: list of (i, qoff, psum_off, width)."""
    if u < 2 * j:
        return [(2 * u, 0, 0, 512), (2 * u + 1, 0, 512, 512)]
    if u == 2 * j:
        return [(4 * j, 0, 0, 512), (4 * j + 1, 128, 512, 384)]
    return [(4 * j + 2, 256, 0, 256), (4 * j + 3, 384, 256, 128)]


def build_program():
    nc = bacc.Bacc("TRN2", target_bir_lowering=False, debug=False,
                   num_devices=N_CORES)

    # ---- DRAM I/O (per core) ----
    xT = nc.dram_tensor("xT", [128, 8, RPC], dt.bfloat16, kind="ExternalInput")
    wq = nc.dram_tensor("wq", [16, 128, 8 * 128], dt.bfloat16, kind="ExternalInput")
    wv = nc.dram_tensor("wv", [2, 128, 8 * 512], dt.bfloat16, kind="ExternalInput")
    bqk = nc.dram_tensor("bqk", [128, 16], dt.float32, kind="ExternalInput")
    bvn = nc.dram_tensor("bvn", [1, 1024], dt.bfloat16, kind="ExternalInput")
    wp = nc.dram_tensor("wp", [128, 2 * 1024], dt.bfloat16, kind="ExternalInput")
    out_d = nc.dram_tensor("out", [T, C], dt.bfloat16, kind="ExternalOutput")

    with tile.TileContext(nc) as tc:
        with tc.tile_pool(name="persist", bufs=1) as pp, \
             tc.tile_pool(name="drampool", bufs=1, space="DRAM") as dp:
            vscr = [dp.tile([128, 2, 512], dt.bfloat16, tag=f"vscr{h}",
                            name=f"vscr{h}") for h in range(HPC)]
            xt = pp.tile([128, 8, RPC], dt.bfloat16, tag="xt")
            bqk_sb = pp.tile([128, 16], dt.float32, tag="bqk")
            bvn_sb = pp.tile([1, 1024], dt.bfloat16, tag="bvn")
            onesr = pp.tile([1, 128], dt.bfloat16, tag="onesr")
            wp_sb = pp.tile([128, 2, 1024], dt.bfloat16, tag="wp")
            ident = pp.tile([128, 128], dt.bfloat16, tag="ident")

            qt_all = pp.tile([64, HPC * T], dt.bfloat16, tag="qt_all")
            kt_all = pp.tile([64, HPC * T], dt.bfloat16, tag="kt_all")
            vn = pp.tile([128, HPC, 16, 65], dt.bfloat16, tag="vn")
            yt = pp.tile([128, 2, T], dt.bfloat16, tag="yt")

            masks.make_identity(nc, ident[:])
            nc.gpsimd.memset(onesr[:], 1.0)
            nc.gpsimd.memset(vn[:, :, :, 64], 1.0)

            # p-state warmup: the PE clock ramp keys off the FIRST busy
            # time; burn it on the identity tile while input DMAs land
            with tc.tile_pool(name="warm", bufs=1, space="PSUM") as pw:
                wps = pw.tile([128, 128], dt.float32, tag="w")
                for _ in range(18):
                    nc.tensor.matmul(wps[:], ident[:], ident[:],
                                     start=True, stop=True)

            with tc.tile_pool(name="ptpool", bufs=30) as ptp, \
                 tc.tile_pool(name="ptdpool", bufs=14) as ptdp, \
                 tc.tile_pool(name="misc", bufs=3) as mp, \
                 tc.tile_pool(name="osb", bufs=3) as osbp, \
                 tc.tile_pool(name="wstream", bufs=1) as ws, \
                 tc.tile_pool(name="vstage", bufs=3) as vst, \
                 tc.tile_pool(name="ps2", bufs=2, space="PSUM") as ps2, \
                 tc.tile_pool(name="psA", bufs=2, space="PSUM") as psA, \
                 tc.tile_pool(name="psB", bufs=2, space="PSUM") as psB:

                # bank-reuse view allocators: psA's 2 banks serve the QK
                # units and later the j<=2 yn accumulators; psB's 2 banks
                # serve the V units and later the j=3 yn / Y^T-transpose /
                # projection tiles.  Always 8 banks total.
                def alloc_yn(h, j):
                    if j == 3 and h < 3:
                        tl = psB.tile([128, 512], dt.float32, tag="pvB",
                                      bufs=1, name=f"ynb{h}{j}")
                        return tl[:].rearrange("p (a b) -> p a b",
                                               a=4)[:, :, 0:65]
                    tl = psA.tile([128, 4, 128], dt.float32, tag="psqk",
                                  name=f"yna{h}{j}")
                    return tl[:, :, 0:65]

                def alloc_ytr(h, j):
                    tl = psB.tile([128, 512], dt.float32, tag="pvA",
                                  bufs=1, name=f"ytrt{h}{j}")
                    return tl[:].bitcast(dt.bfloat16)[0:64, 0:512]

                def alloc_po(tt, cc):
                    tl = psB.tile([128, 512], dt.float32, tag="pvB",
                                  bufs=1, name=f"pot{tt}{cc}")
                    return tl[:]

                # ---------------- input DMAs ----------------
                wqall = ws.tile([128, 16, 8, 128], dt.bfloat16, tag="wqall")
                nc.sync.dma_start(xt[:, 0:2, :], xT[:, 0:2, :])
                for c4 in range(4):
                    if c4 == 1:
                        nc.sync.dma_start(xt[:, 2:8, :], xT[:, 2:8, :])
                    if c4 == 0:
                        nc.sync.dma_start(
                            wqall[:, 0:2], wq[0:2].rearrange(
                                "m p (k j) -> p m k j", k=8))
                        nc.sync.dma_start(
                            wqall[:, 2:4], wq[2:4].rearrange(
                                "m p (k j) -> p m k j", k=8))
                        continue
                    nc.sync.dma_start(
                        wqall[:, 4 * c4:4 * c4 + 4],
                        wq[4 * c4:4 * c4 + 4].rearrange(
                            "m p (k j) -> p m k j", k=8))
                    if c4 == 1:
                        nc.sync.dma_start(bqk_sb[:], bqk[:])
                        nc.sync.dma_start(bvn_sb[:], bvn[:])
                # V/proj weights: virtual-time delayed so the wq stream owns
                # the DMA engine while it feeds PE; wv0 lands right as the
                # first V unit needs it
                wvt = [ws.tile([128, 8, 512], dt.bfloat16, tag="wvt",
                               bufs=2, name=f"wvt{jv}") for jv in range(2)]
                for jv in range(2):
                    tc.tile_set_cur_wait(0.0145 + 0.0035 * jv)
                    nc.sync.dma_start(wvt[jv][:], wv[jv].rearrange(
                        "p (k j) -> p k j", k=8))
                tc.tile_set_cur_wait(0.021)
                nc.sync.dma_start(wp_sb[:], wp.rearrange(
                    "p (t c) -> p t c", t=2))
                tc.tile_set_cur_wait(0.0)

                # ---------------- emission helpers ----------------
                def emit_sexp(h, j, u):
                    """S^T matmuls for pair u + exp + exact-diagonal mask."""
                    mem = _pair_members(j, u)
                    tot = mem[-1][2] + mem[-1][3]
                    same_bank = tot <= 512   # pair B: one group, start/stop split
                    ssp = ps2.tile([128, 1024], dt.float32, tag="ssp",
                                   name=f"ssp{h}{j}{u}")
                    for mi, (i, qoff, off, w) in enumerate(mem):
                        nc.tensor.matmul(
                            ssp[:, off:off + w],
                            kt_all[:, T * h + 128 * i:T * h + 128 * (i + 1)],
                            qt_all[:, T * h + 512 * j + qoff:
                                   T * h + 512 * j + qoff + w],
                            start=(not same_bank) or mi == 0,
                            stop=(not same_bank) or mi == len(mem) - 1)
                    pt = ptp.tile([128, 1024], dt.bfloat16, tag="pt",
                                  name=f"pt{h}{j}{u}")
                    nc.scalar.activation(pt[:, 0:tot], ssp[:, 0:tot],
                                         AF.Exp, scale=SCALE)
                    dmap = {}
                    if u >= 2 * j:  # mask the exact diagonal, out-of-place
                        for (i, qoff, off, w) in mem:
                            dcol = off + 128 * (i - 4 * j) - qoff
                            ptd = ptdp.tile([128, 128], dt.bfloat16,
                                            tag="ptd", name=f"ptd{h}{j}{u}{i}")
                            nc.gpsimd.affine_select(
                                out=ptd[:], in_=pt[:, dcol:dcol + 128],
                                compare_op=OP.is_ge, fill=0.0,
                                base=0, channel_multiplier=-1,
                                pattern=[[1, 128]])
                            dmap[i] = ptd
                    return pt, mem, dmap

                pending = []   # deferred (norm / proj) closures, drip-fed

                def drip(n=1):
                    npop = min(n + (len(pending) > 4) + (len(pending) > 8),
                               len(pending))
                    for _ in range(npop):
                        pending.pop(0)()

                def make_norm(h, j, yn, on_act=False):
                    def norm():
                        rin = mp.tile([128, 4], dt.float32, tag="rin",
                                      name=f"rin{h}{j}")
                        nc.vector.tensor_copy(rin[:], yn[:, :, 64])
                        rcp = mp.tile([128, 4], dt.float32, tag="rcp",
                                      name=f"rcp{h}{j}")
                        with nc.allow_low_precision(reason="softmax recip"):
                            nc.vector.reciprocal(rcp[:], rin[:])
                        yb = mp.tile([128, 4, 64], dt.bfloat16, tag="yb",
                                     name=f"yb{h}{j}")
                        nc.vector.tensor_tensor(
                            yb[:], yn[:, :, 0:64],
                            rcp[:, :, None].broadcast_to([128, 4, 64]),
                            op=OP.mult)
                        ytr = alloc_ytr(h, j)
                        for qb in range(4):
                            nc.tensor.transpose(
                                ytr[:, 128 * qb:128 * (qb + 1)],
                                yb[:, qb, :], ident[:])
                        dst = yt[64 * (h % 2):64 * (h % 2) + 64,
                                 h // 2, 512 * j:512 * (j + 1)]
                        if on_act:
                            nc.scalar.copy(dst, ytr)
                        else:
                            nc.vector.tensor_copy(dst, ytr)
                    return norm

                def make_proj_one(j, tt, cc, last=False):
                    def proj():
                        if last:
                            po = ps2.tile([128, 1024], dt.float32,
                                          tag="ssp", name=f"pol{tt}")
                            # one matmul per psum bank (a matmul must not
                            # cross a bank boundary)
                            for half in range(2):
                                for p in range(2):
                                    nc.tensor.matmul(
                                        po[:, 512 * half:512 * (half + 1)],
                                        yt[:, p, 128 * tt:128 * (tt + 1)],
                                        wp_sb[:, p, 512 * half:
                                              512 * (half + 1)],
                                        start=(p == 0), stop=(p == 1))
                            ot = osbp.tile([128, 1024], dt.bfloat16,
                                           tag="otw", name=f"otw{tt}")
                            if tt % 2 == 0:
                                nc.scalar.copy(ot[:], po[:])
                            else:
                                nc.vector.tensor_copy(ot[:], po[:])
                            nc.sync.dma_start(
                                out_d[128 * tt:128 * (tt + 1), :], ot[:])
                            return
                        po = alloc_po(tt, cc)
                        for p in range(2):
                            nc.tensor.matmul(
                                po, yt[:, p, 128 * tt:128 * (tt + 1)],
                                wp_sb[:, p, 512 * cc:512 * (cc + 1)],
                                start=(p == 0), stop=(p == 1))
                        ot = osbp.tile([128, 512], dt.bfloat16,
                                       tag="ot", name=f"ot{tt}{cc}")
                        nc.vector.tensor_copy(ot[:], po)
                        nc.sync.dma_start(
                            out_d[128 * tt:128 * (tt + 1),
                                  512 * cc:512 * (cc + 1)], ot[:])
                    return proj

                class HeadRun:
                    """Incremental emitter for one (head, query-block)."""

                    def __init__(self, h, j, on_act=False):
                        self.h, self.j = h, j
                        self.n_u = 2 * j + 2
                        self.units = []
                        self.np_ = 0
                        self.yn = None
                        self.on_act = on_act
                        self.done = False

                    def sexp(self, k=1):
                        for _ in range(k):
                            if len(self.units) >= self.n_u:
                                return
                            self.units.append(
                                emit_sexp(self.h, self.j, len(self.units)))

                    def pv(self, k=1):
                        ns = len(self.units)
                        la = 3
                        lim = ns if ns == self.n_u else max(ns - la, 0)
                        for _ in range(k):
                            if self.np_ >= lim:
                                break
                            if self.yn is None:
                                self.yn = alloc_yn(self.h, self.j)
                            u = self.np_
                            pt, mem, dmap = self.units[u]
                            for mi, (i, qoff, off, w) in enumerate(mem):
                                qbs = list(range(
                                    max(qoff // 128, i - 4 * self.j), 4))
                                for qi, qb in enumerate(qbs):
                                    if qb == i - 4 * self.j:
                                        lhs = dmap[i][:]
                                    else:
                                        col = off + 128 * qb - qoff
                                        lhs = pt[:, col:col + 128]
                                    nc.tensor.matmul(
                                        self.yn[:, qb, :], lhs,
                                        vn[:, self.h, i, :],
                                        start=(u == 0 and mi == 0
                                               and qi == 0),
                                        stop=(u == self.n_u - 1
                                              and mi == len(mem) - 1
                                              and qi == len(qbs) - 1))
                            self.units[u] = None
                            self.np_ += 1
                        if self.np_ == self.n_u and not self.done:
                            self.done = True
                            pending.append(make_norm(self.h, self.j, self.yn,
                                                     self.on_act))

                    def step(self):
                        self.sexp(1)
                        drip(1)
                        self.pv(1)

                    def run_all(self):
                        while not self.done:
                            self.step()


                def qk_unit(hh, mg):
                    ps = psA.tile([128, 4, 128], dt.float32, tag="psqk")
                    for mi in range(4):
                        m = 4 * mg + mi
                        for k in range(8):
                            nc.tensor.matmul(
                                ps[:, mi, :], wqall[:, m, k, :],
                                xt[:, k, 128 * hh:128 * (hh + 1)],
                                start=(mi == 0 and k == 0),
                                stop=(mi == 3 and k == 7))
                    dest = qt_all if mg < 2 else kt_all
                    dv = dest[:].rearrange("d (h rh g) -> d h rh g",
                                           rh=128, g=16)
                    for par in range(2):
                        gb = 8 * (mg % 2) + par
                        nc.vector.tensor_tensor(
                            dv[:, hh, :, gb:gb + 7:2],
                            ps[64 * par:64 * par + 64].rearrange(
                                "d mi rh -> d rh mi"),
                            bqk_sb[64 * par:64 * par + 64,
                                   4 * mg:4 * mg + 4][:, None, :]
                            .broadcast_to([64, 128, 4]),
                            op=OP.add)

                def v_head(h):
                    vsb = vst.tile([128, 2, 512], dt.bfloat16, tag="vsb",
                                   name=f"vsb{h}")
                    for jv in range(2):
                        ps = psB.tile([128, 512], dt.float32,
                                      tag=("pvA", "pvB")[jv], bufs=1)
                        for k in range(8):
                            nc.tensor.matmul(
                                ps[:], xt[:, k, 128 * h:128 * (h + 1)],
                                wvt[jv][:, k, :],
                                start=(k == 0), stop=(k == 7))
                        nc.vector.tensor_copy(vsb[:, jv, :], ps[:])
                    # V bias via a broadcast accumulate-DMA (frees PE of the
                    # ones-row bias matmuls; bias varies along columns)
                    nc.gpsimd.dma_start(
                        vsb[:], bvn[0:1, :].rearrange(
                            "o (jv c) -> o jv c", jv=2)
                        .broadcast_to([128, 2, 512]),
                        accum_op=OP.add)
                    # re-partition to [s, hd] via a DRAM bounce (the gather's
                    # source AP mixes partition bits into free dims):
                    # s = 16*rr + g, g = 8*jv + g2 -> partition (r jv g2)
                    nc.sync.dma_start(vscr[h][:], vsb[:])
                    src = vscr[h][:].rearrange(
                        "(i r) jv (g2 d) -> (r jv g2) i d", r=8, d=64)
                    nc.sync.dma_start(vn[:, h, :, 0:64], src)

                # ---------------- the pipelined schedule ----------------
                runs = {}
                for h in range(HPC):
                    for j in range(4):
                        runs[(h, j)] = HeadRun(h, j, on_act=(j == 0))

                for mg in range(4):           # head 0 projection
                    qk_unit(0, mg)
                v_head(0)
                runs[(0, 3)].sexp(2)          # h0's exp starts during V
                v_head(1)
                runs[(0, 3)].sexp(2)
                v_head(2)
                runs[(0, 3)].sexp(2)
                v_head(3)
                runs[(0, 3)].sexp(2)

                for hh in range(1, HPC):      # heads 1-3 projection,
                    pr = runs[(hh - 1, 3)]    # interleaved with attn(hh-1)
                    for mg in range(4):
                        qk_unit(hh, mg)
                        pr.sexp(3)
                        drip(1)
                        pr.pv(4)
                        # keep ACT fed: pre-emit j=2 S/exp of done heads
                        runs[(hh - 1, 2)].sexp(2)
                    pr.pv(3)                  # finish the head
                    runs[(hh - 1, 2)].sexp(1)

                runs[(3, 3)].run_all()        # last head's big block

                for tt in range(12, 16):
                    pending.append(make_proj_one(3, tt, 0))
                    pending.append(make_proj_one(3, tt, 1))

                for jx, j in enumerate([2, 1, 0]):
                    for h in range(HPC):
                        r = runs[(h, j)]
                        while not r.done:
                            r.step()
                            if j > 0 and len(r.units) >= r.n_u:
                                runs[(h, j - 1)].sexp(1)
                    for tt in range(4 * j, 4 * j + 4):
                        if j == 0:
                            pending.append(make_proj_one(j, tt, 0, last=True))
                        else:
                            for cc in range(2):
                                pending.append(make_proj_one(j, tt, cc))
                for fn in pending[:]:
                    pending.pop(0)()
    nc.compile()
    return nc


_NC_CACHE = None


def _get_program():
    global _NC_CACHE
    if _NC_CACHE is None:
        _NC_CACHE = build_program()
    return _NC_CACHE


def _prep_core_inputs(x, Wqkv, bqkv, Wproj, bproj):
    """Build the 8 per-core input dicts (host-side shard + layout prep)."""
    x = np.asarray(x, dtype=np.float32)
    Wqkv = np.ascontiguousarray(np.asarray(Wqkv, dtype=np.float32))
    bqkv = np.asarray(bqkv, dtype=np.float32)
    Wproj = np.asarray(Wproj, dtype=np.float32)

    wq_np = np.ascontiguousarray(
        Wqkv[:, :2048].reshape(8, 128, 16, 128).transpose(2, 1, 0, 3)
        .reshape(16, 128, 8 * 128)).astype(BF16)
    wv_np = np.ascontiguousarray(
        Wqkv[:, 2048:].reshape(8, 128, 2, 512).transpose(2, 1, 0, 3)
        .reshape(2, 128, 8 * 512)).astype(BF16)
    bqk_np = np.ascontiguousarray(bqkv[:2048].reshape(16, 128).T)
    bvn_np = np.ascontiguousarray(bqkv[2048:].reshape(1, 1024)).astype(BF16)

    in_maps = []
    for c in range(N_CORES):
        b, q = divmod(c, 4)
        xT_np = np.ascontiguousarray(
            x[b, RPC * q:RPC * (q + 1), :].reshape(RPC, 8, 128)
            .transpose(2, 1, 0)).astype(BF16)
        wp_np = np.ascontiguousarray(
            Wproj[256 * q:256 * (q + 1), :].reshape(2, 128, 1024)
            .transpose(1, 0, 2).reshape(128, 2048)).astype(BF16)
        in_maps.append({
            "xT": xT_np, "wq": wq_np, "wv": wv_np, "bqk": bqk_np,
            "bvn": bvn_np, "wp": wp_np,
        })
    return in_maps


def kernel(x, Wqkv, bqkv, Wproj, bproj):
    nc = _get_program()
    in_maps = _prep_core_inputs(x, Wqkv, bqkv, Wproj, bproj)
    res = run_bass_kernel_spmd(nc, in_maps, list(range(N_CORES)))
    out = np.zeros((B, T, C), dtype=np.float32)
    for c in range(N_CORES):
        out[c // 4] += res.results[c]["out"].astype(np.float32)
    out += np.asarray(bproj, dtype=np.float32)
    return out


# revision 52
# speedup vs baseline: 1.4963x; 1.0125x over previous
"""Causal self-attention (dense transformer) on 8 trn2 NeuronCores.

Reference semantics (note the headless reshape):
  x_proj = x @ Wqkv + bqkv                     # [B, T, 3C]
  q = x_proj[:, :, :C].reshape(B, H, T, hd)    # direct reshape, no transpose!
Because of the direct reshape, head h consumes the contiguous row block
x_proj[b, h*128:(h+1)*128, :] reinterpreted as [T, hd].  So sharding by
(batch, head-group) makes QKV projection + attention fully core-local;
only the output projection is a row-parallel partial sum, reduced on host.

Shapes (hardcoded): B=2, T=2048, C=1024, n_head=16, hd=64, 8 cores.
Core c: batch b=c//4, quarter q=c%4 -> x rows [512q, 512q+512), heads 4q..4q+3.

v4 design notes (cost model: matmul cost = out-free-size x cycles/row; K and
partition count are free; bf16 is 1 cyc/row at ANY width, f32r only >=256):
- bf16 everywhere (PSUM fp32).  Total error ~4e-3 vs the 2e-2 gate.
- QK projection is HEAD-BLOCKED (per (head, m-group-of-4) unit, 32 matmuls
  of 128-wide bf16 into one psum bank / one accumulation group), and the
  whole schedule is SOFTWARE-PIPELINED BY HEAD: head h's attention
  (S -> exp -> P@V) is emitted interleaved with head h+1's projection
  units, so the ACT engine (exp, the #2 load at ~70us) starts ~18us in and
  never waits for the full projection.
- order: QK(h0) | V(all heads, + S/exp of h0 woven) | QK(h1)+attn(h0) |
  QK(h2)+attn(h1) | QK(h3)+attn(h2) | attn(h3) | remaining query blocks
  j=2,1,0 with drip-fed normalization + projection closures.
- PSUM bank lifetimes telescope: QK pool (2) and V pool (2) + S pairs (4)
  early; V pool is then traded for the Y^T-transpose bank and the j=3 yn
  bank; the QK pool is traded for the j<=2 yn pool; the first yn bank is
  traded for the projection bank.  Always exactly 8 banks.
- eviction of q^T/k^T: DVE tensor_tensor add with a stride-0-broadcast
  per-(partition, m) bias AP; stride-16 shuffled dest APs.
- V natural with ones-row bias matmul (the V bias varies with s%16 via the
  headless reshape, so it must be added on x_proj columns), bf16-evicted,
  re-partitioned to [s, hd] tiles via a DRAM bounce.
- S^T tiles at causally-minimal widths (512/384/256/128); exp straight off
  2-bank psum with scale=1/8; only the exact-diagonal 128x128 block is
  masked, out-of-place into a small ptd tile (mask off the critical path).
- P@V natural-Y: yn[q=128, 4, 65] accumulates over s-tiles with lhsT =
  pt/ptd 128-col slices: 65 cycles per (s-tile, q-block) instead of 512.
  Above-diagonal blocks skipped; ones-column gives the denominator.
- normalization: batched reciprocal, one broadcast tensor_tensor eviction,
  PE-transpose (identity matmul) to Y^T.
- projection: 512-wide dripped units; the last query block runs 1024-wide
  units on the freed S-psum banks with ACT/DVE alternating evictions.
- host: 4-way partial reduction + bproj in fp32.
"""

import os

import numpy as np
import ml_dtypes

os.environ.setdefault("NEURON_RT_RESET_CORES", "1")

import concourse.bacc as bacc
import concourse.mybir as mybir
import concourse.tile as tile
from concourse import masks
from concourse.bass_utils import run_bass_kernel_spmd

dt = mybir.dt
AF = mybir.ActivationFunctionType
OP = mybir.AluOpType
BF16 = np.dtype(ml_dtypes.bfloat16)

B, T, C = 2, 2048, 1024
NH, HD = 16, 64
N_CORES = 8
HPC = 4          # heads per core
RPC = 512        # x rows per core
SCALE = 1.0 / 8.0   # 1/sqrt(hd), folded into the exp activation


def _pair_members(j, u):
    """s-tile pair u of query block j: list of (i, qoff, psum_off, width)."""
    if u < 2 * j:
        return [(2 * u, 0, 0, 512), (2 * u + 1, 0, 512, 512)]
    if u == 2 * j:
        return [(4 * j, 0, 0, 512), (4 * j + 1, 128, 512, 384)]
    return [(4 * j + 2, 256, 0, 256), (4 * j + 3, 384, 256, 128)]


def build_program():
    nc = bacc.Bacc("TRN2", target_bir_lowering=False, debug=False,
                   num_devices=N_CORES)

    # ---- DRAM I/O (per core) ----
    xT = nc.dram_tensor("xT", [128, 8, RPC], dt.bfloat16, kind="ExternalInput")
    wq = nc.dram_tensor("wq", [16, 128, 8 * 128], dt.bfloat16, kind="ExternalInput")
    wv = nc.dram_tensor("wv", [2, 128, 8 * 512], dt.bfloat16, kind="ExternalInput")
    bqk = nc.dram_tensor("bqk", [128, 16], dt.float32, kind="ExternalInput")
    bvn = nc.dram_tensor("bvn", [1, 1024], dt.bfloat16, kind="ExternalInput")
    wp = nc.dram_tensor("wp", [128, 2 * 1024], dt.bfloat16, kind="ExternalInput")
    out_d = nc.dram_tensor("out", [T, C], dt.bfloat16, kind="ExternalOutput")

    with tile.TileContext(nc) as tc:
        with tc.tile_pool(name="persist", bufs=1) as pp, \
             tc.tile_pool(name="drampool", bufs=1, space="DRAM") as dp:
            vscr = [dp.tile([128, 2, 512], dt.bfloat16, tag=f"vscr{h}",
                            name=f"vscr{h}") for h in range(HPC)]
            xt = pp.tile([128, 8, RPC], dt.bfloat16, tag="xt")
            bqk_sb = pp.tile([128, 16], dt.float32, tag="bqk")
            bvn_sb = pp.tile([1, 1024], dt.bfloat16, tag="bvn")
            onesr = pp.tile([1, 128], dt.bfloat16, tag="onesr")
            wp_sb = pp.tile([128, 2, 1024], dt.bfloat16, tag="wp")
            ident = pp.tile([128, 128], dt.bfloat16, tag="ident")

            qt_all = pp.tile([64, HPC * T], dt.bfloat16, tag="qt_all")
            kt_all = pp.tile([64, HPC * T], dt.bfloat16, tag="kt_all")
            vn = pp.tile([128, HPC, 16, 65], dt.bfloat16, tag="vn")
            yt = pp.tile([128, 2, T], dt.bfloat16, tag="yt")

            masks.make_identity(nc, ident[:])
            nc.gpsimd.memset(onesr[:], 1.0)
            nc.gpsimd.memset(vn[:, :, :, 64], 1.0)

            # p-state warmup: the PE clock ramp keys off the FIRST busy
            # time; burn it on the identity tile while input DMAs land
            with tc.tile_pool(name="warm", bufs=1, space="PSUM") as pw:
                wps = pw.tile([128, 128], dt.float32, tag="w")
                for _ in range(18):
                    nc.tensor.matmul(wps[:], ident[:], ident[:],
                                     start=True, stop=True)

            with tc.tile_pool(name="ptpool", bufs=30) as ptp, \
                 tc.tile_pool(name="ptdpool", bufs=14) as ptdp, \
                 tc.tile_pool(name="misc", bufs=3) as mp, \
                 tc.tile_pool(name="osb", bufs=3) as osbp, \
                 tc.tile_pool(name="wstream", bufs=1) as ws, \
                 tc.tile_pool(name="vstage", bufs=3) as vst, \
                 tc.tile_pool(name="ps2", bufs=2, space="PSUM") as ps2, \
                 tc.tile_pool(name="psA", bufs=2, space="PSUM") as psA, \
                 tc.tile_pool(name="psB", bufs=2, space="PSUM") as psB:

                # bank-reuse view allocators: psA's 2 banks serve the QK
                # units and later the j<=2 yn accumulators; psB's 2 banks
                # serve the V units and later the j=3 yn / Y^T-transpose /
                # projection tiles.  Always 8 banks total.
                def alloc_yn(h, j):
                    if j == 3 and h < 3:
                        tl = psB.tile([128, 512], dt.float32, tag="pvB",
                                      bufs=1, name=f"ynb{h}{j}")
                        return tl[:].rearrange("p (a b) -> p a b",
                                               a=4)[:, :, 0:65]
                    tl = psA.tile([128, 4, 128], dt.float32, tag="psqk",
                                  name=f"yna{h}{j}")
                    return tl[:, :, 0:65]

                def alloc_ytr(h, j):
                    tl = psB.tile([128, 512], dt.float32, tag="pvA",
                                  bufs=1, name=f"ytrt{h}{j}")
                    return tl[:].bitcast(dt.bfloat16)[0:64, 0:512]

                def alloc_po(tt, cc):
                    tl = psB.tile([128, 512], dt.float32, tag="pvB",
                                  bufs=1, name=f"pot{tt}{cc}")
                    return tl[:]

                # ---------------- input DMAs ----------------
                wqall = ws.tile([128, 16, 8, 128], dt.bfloat16, tag="wqall")
                nc.sync.dma_start(xt[:, 0:2, :], xT[:, 0:2, :])
                for c4 in range(4):
                    if c4 == 1:
                        nc.sync.dma_start(xt[:, 2:8, :], xT[:, 2:8, :])
                    if c4 == 0:
                        nc.sync.dma_start(
                            wqall[:, 0:2], wq[0:2].rearrange(
                                "m p (k j) -> p m k j", k=8))
                        nc.sync.dma_start(
                            wqall[:, 2:4], wq[2:4].rearrange(
                                "m p (k j) -> p m k j", k=8))
                        continue
                    nc.sync.dma_start(
                        wqall[:, 4 * c4:4 * c4 + 4],
                        wq[4 * c4:4 * c4 + 4].rearrange(
                            "m p (k j) -> p m k j", k=8))
                    if c4 == 1:
                        nc.sync.dma_start(bqk_sb[:], bqk[:])
                        nc.sync.dma_start(bvn_sb[:], bvn[:])
                # V/proj weights: virtual-time delayed so the wq stream owns
                # the DMA engine while it feeds PE; wv0 lands right as the
                # first V unit needs it
                wvt = [ws.tile([128, 8, 512], dt.bfloat16, tag="wvt",
                               bufs=2, name=f"wvt{jv}") for jv in range(2)]
                for jv in range(2):
                    tc.tile_set_cur_wait(0.0145 + 0.0035 * jv)
                    nc.sync.dma_start(wvt[jv][:], wv[jv].rearrange(
                        "p (k j) -> p k j", k=8))
                tc.tile_set_cur_wait(0.021)
                nc.sync.dma_start(wp_sb[:], wp.rearrange(
                    "p (t c) -> p t c", t=2))
                tc.tile_set_cur_wait(0.0)

                # ---------------- emission helpers ----------------
                def emit_sexp(h, j, u):
                    """S^T matmuls for pair u + exp + exact-diagonal mask."""
                    mem = _pair_members(j, u)
                    tot = mem[-1][2] + mem[-1][3]
                    same_bank = tot <= 512   # pair B: one group, start/stop split
                    ssp = ps2.tile([128, 1024], dt.float32, tag="ssp",
                                   name=f"ssp{h}{j}{u}")
                    for mi, (i, qoff, off, w) in enumerate(mem):
                        nc.tensor.matmul(
                            ssp[:, off:off + w],
                            kt_all[:, T * h + 128 * i:T * h + 128 * (i + 1)],
                            qt_all[:, T * h + 512 * j + qoff:
                                   T * h + 512 * j + qoff + w],
                            start=(not same_bank) or mi == 0,
                            stop=(not same_bank) or mi == len(mem) - 1)
                    pt = ptp.tile([128, 1024], dt.bfloat16, tag="pt",
                                  name=f"pt{h}{j}{u}")
                    nc.scalar.activation(pt[:, 0:tot], ssp[:, 0:tot],
                                         AF.Exp, scale=SCALE)
                    dmap = {}
                    if u >= 2 * j:  # mask the exact diagonal, out-of-place
                        for (i, qoff, off, w) in mem:
                            dcol = off + 128 * (i - 4 * j) - qoff
                            ptd = ptdp.tile([128, 128], dt.bfloat16,
                                            tag="ptd", name=f"ptd{h}{j}{u}{i}")
                            nc.gpsimd.affine_select(
                                out=ptd[:], in_=pt[:, dcol:dcol + 128],
                                compare_op=OP.is_ge, fill=0.0,
                                base=0, channel_multiplier=-1,
                                pattern=[[1, 128]])
                            dmap[i] = ptd
                    return pt, mem, dmap

                pending = []   # deferred (norm / proj) closures, drip-fed

                def drip(n=1):
                    npop = min(n + (len(pending) > 4) + (len(pending) > 8),
                               len(pending))
                    for _ in range(npop):
                        pending.pop(0)()

                def make_norm(h, j, yn, on_act=False):
                    def norm():
                        rin = mp.tile([128, 4], dt.float32, tag="rin",
                                      name=f"rin{h}{j}")
                        nc.vector.tensor_copy(rin[:], yn[:, :, 64])
                        rcp = mp.tile([128, 4], dt.float32, tag="rcp",
                                      name=f"rcp{h}{j}")
                        with nc.allow_low_precision(reason="softmax recip"):
                            nc.vector.reciprocal(rcp[:], rin[:])
                        yb = mp.tile([128, 4, 64], dt.bfloat16, tag="yb",
                                     name=f"yb{h}{j}")
                        nc.vector.tensor_tensor(
                            yb[:], yn[:, :, 0:64],
                            rcp[:, :, None].broadcast_to([128, 4, 64]),
                            op=OP.mult)
                        ytr = alloc_ytr(h, j)
                        for qb in range(4):
                            nc.tensor.transpose(
                                ytr[:, 128 * qb:128 * (qb + 1)],
                                yb[:, qb, :], ident[:])
                        dst = yt[64 * (h % 2):64 * (h % 2) + 64,
                                 h // 2, 512 * j:512 * (j + 1)]
                        if on_act:
                            nc.scalar.copy(dst, ytr)
                        else:
                            nc.vector.tensor_copy(dst, ytr)
                    return norm

                def make_proj_one(j, tt, cc, last=False):
                    def proj():
                        if last:
                            po = ps2.tile([128, 1024], dt.float32,
                                          tag="ssp", name=f"pol{tt}")
                            # one matmul per psum bank (a matmul must not
                            # cross a bank boundary)
                            for half in range(2):
                                for p in range(2):
                                    nc.tensor.matmul(
                                        po[:, 512 * half:512 * (half + 1)],
                                        yt[:, p, 128 * tt:128 * (tt + 1)],
                                        wp_sb[:, p, 512 * half:
                                              512 * (half + 1)],
                                        start=(p == 0), stop=(p == 1))
                            ot = osbp.tile([128, 1024], dt.bfloat16,
                                           tag="otw", name=f"otw{tt}")
                            if tt % 2 == 0:
                                nc.scalar.copy(ot[:], po[:])
                            else:
                                nc.vector.tensor_copy(ot[:], po[:])
                            nc.sync.dma_start(
                                out_d[128 * tt:128 * (tt + 1), :], ot[:])
                            return
                        po = alloc_po(tt, cc)
                        for p in range(2):
                            nc.tensor.matmul(
                                po, yt[:, p, 128 * tt:128 * (tt + 1)],
                                wp_sb[:, p, 512 * cc:512 * (cc + 1)],
                                start=(p == 0), stop=(p == 1))
                        ot = osbp.tile([128, 512], dt.bfloat16,
                                       tag="ot", name=f"ot{tt}{cc}")
                        nc.vector.tensor_copy(ot[:], po)
                        nc.sync.dma_start(
                            out_d[128 * tt:128 * (tt + 1),
                                  512 * cc:512 * (cc + 1)], ot[:])
                    return proj

                class HeadRun:
                    """Incremental emitter for one (head, query-block)."""

                    def __init__(self, h, j, on_act=False):
                        self.h, self.j = h, j
                        self.n_u = 2 * j + 2
                        self.units = []
                        self.np_ = 0
                        self.yn = None
                        self.on_act = on_act
                        self.done = False

                    def sexp(self, k=1):
                        for _ in range(k):
                            if len(self.units) >= self.n_u:
                                return
                            self.units.append(
                                emit_sexp(self.h, self.j, len(self.units)))

                    def pv(self, k=1):
                        ns = len(self.units)
                        la = 3
                        lim = ns if ns == self.n_u else max(ns - la, 0)
                        for _ in range(k):
                            if self.np_ >= lim:
                                break
                            if self.yn is None:
                                self.yn = alloc_yn(self.h, self.j)
                            u = self.np_
                            pt, mem, dmap = self.units[u]
                            for mi, (i, qoff, off, w) in enumerate(mem):
                                qbs = list(range(
                                    max(qoff // 128, i - 4 * self.j), 4))
                                for qi, qb in enumerate(qbs):
                                    if qb == i - 4 * self.j:
                                        lhs = dmap[i][:]
                                    else:
                                        col = off + 128 * qb - qoff
                                        lhs = pt[:, col:col + 128]
                                    nc.tensor.matmul(
                                        self.yn[:, qb, :], lhs,
                                        vn[:, self.h, i, :],
                                        start=(u == 0 and mi == 0
                                               and qi == 0),
                                        stop=(u == self.n_u - 1
                                              and mi == len(mem) - 1
                                              and qi == len(qbs) - 1))
                            self.units[u] = None
                            self.np_ += 1
                        if self.np_ == self.n_u and not self.done:
                            self.done = True
                            pending.append(make_norm(self.h, self.j, self.yn,
                                                     self.on_act))

                    def step(self):
                        self.sexp(1)
                        drip(1)
                        self.pv(1)

                    def run_all(self):
                        while not self.done:
                            self.step()


                def qk_unit(hh, mg):
                    ps = psA.tile([128, 4, 128], dt.float32, tag="psqk")
                    for mi in range(4):
                        m = 4 * mg + mi
                        for k in range(8):
                            nc.tensor.matmul(
                                ps[:, mi, :], wqall[:, m, k, :],
                                xt[:, k, 128 * hh:128 * (hh + 1)],
                                start=(mi == 0 and k == 0),
                                stop=(mi == 3 and k == 7))
                    dest = qt_all if mg < 2 else kt_all
                    dv = dest[:].rearrange("d (h rh g) -> d h rh g",
                                           rh=128, g=16)
                    for par in range(2):
                        gb = 8 * (mg % 2) + par
                        nc.vector.tensor_tensor(
                            dv[:, hh, :, gb:gb + 7:2],
                            ps[64 * par:64 * par + 64].rearrange(
                                "d mi rh -> d rh mi"),
                            bqk_sb[64 * par:64 * par + 64,
                                   4 * mg:4 * mg + 4][:, None, :]
                            .broadcast_to([64, 128, 4]),
                            op=OP.add)

                def v_head(h):
                    vsb = vst.tile([128, 2, 512], dt.bfloat16, tag="vsb",
                                   name=f"vsb{h}")
                    for jv in range(2):
                        ps = psB.tile([128, 512], dt.float32,
                                      tag=("pvA", "pvB")[jv], bufs=1)
                        for k in range(8):
                            nc.tensor.matmul(
                                ps[:], xt[:, k, 128 * h:128 * (h + 1)],
                                wvt[jv][:, k, :],
                                start=(k == 0), stop=(k == 7))
                        nc.vector.tensor_copy(vsb[:, jv, :], ps[:])
                    # V bias via a broadcast accumulate-DMA (frees PE of the
                    # ones-row bias matmuls; bias varies along columns)
                    nc.gpsimd.dma_start(
                        vsb[:], bvn[0:1, :].rearrange(
                            "o (jv c) -> o jv c", jv=2)
                        .broadcast_to([128, 2, 512]),
                        accum_op=OP.add)
                    # re-partition to [s, hd] via a DRAM bounce (the gather's
                    # source AP mixes partition bits into free dims):
                    # s = 16*rr + g, g = 8*jv + g2 -> partition (r jv g2)
                    nc.sync.dma_start(vscr[h][:], vsb[:])
                    src = vscr[h][:].rearrange(
                        "(i r) jv (g2 d) -> (r jv g2) i d", r=8, d=64)
                    nc.sync.dma_start(vn[:, h, :, 0:64], src)

                # ---------------- the pipelined schedule ----------------
                runs = {}
                for h in range(HPC):
                    for j in range(4):
                        runs[(h, j)] = HeadRun(h, j, on_act=(j == 0))

                for mg in range(4):           # head 0 projection
                    qk_unit(0, mg)
                v_head(0)
                runs[(0, 3)].sexp(2)          # h0's exp starts during V
                v_head(1)
                runs[(0, 3)].sexp(2)
                v_head(2)
                runs[(0, 3)].sexp(2)
                v_head(3)
                runs[(0, 3)].sexp(2)

                for hh in range(1, HPC):      # heads 1-3 projection,
                    pr = runs[(hh - 1, 3)]    # interleaved with attn(hh-1)
                    for mg in range(4):
                        qk_unit(hh, mg)
                        pr.sexp(3)
                        drip(1)
                        pr.pv(4)
                        # keep ACT fed: pre-emit j=2 S/exp of done heads
                        runs[(hh - 1, 2)].sexp(2)
                    pr.pv(3)                  # finish the head
                    runs[(hh - 1, 2)].sexp(1)

                r33 = runs[(3, 3)]            # last head's big block
                while not r33.done:
                    r33.step()
                    if len(r33.units) >= r33.n_u:
                        runs[(3, 2)].sexp(1)

                for tt in range(12, 16):
                    pending.append(make_proj_one(3, tt, 0))
                    pending.append(make_proj_one(3, tt, 1))

                for jx, j in enumerate([2, 1, 0]):
                    for h in range(HPC):
                        r = runs[(h, j)]
                        while not r.done:
                            r.step()
                            if j > 0 and len(r.units) >= r.n_u:
                                runs[(h, j - 1)].sexp(1)
                    for tt in range(4 * j, 4 * j + 4):
                        if j == 0:
                            pending.append(make_proj_one(j, tt, 0, last=True))
                        else:
                            for cc in range(2):
                                pending.append(make_proj_one(j, tt, cc))
                for fn in pending[:]:
                    pending.pop(0)()
    nc.compile()
    return nc


_NC_CACHE = None


def _get_program():
    global _NC_CACHE
    if _NC_CACHE is None:
        _NC_CACHE = build_program()
    return _NC_CACHE


def _prep_core_inputs(x, Wqkv, bqkv, Wproj, bproj):
    """Build the 8 per-core input dicts (host-side shard + layout prep)."""
    x = np.asarray(x, dtype=np.float32)
    Wqkv = np.ascontiguousarray(np.asarray(Wqkv, dtype=np.float32))
    bqkv = np.asarray(bqkv, dtype=np.float32)
    Wproj = np.asarray(Wproj, dtype=np.float32)

    wq_np = np.ascontiguousarray(
        Wqkv[:, :2048].reshape(8, 128, 16, 128).transpose(2, 1, 0, 3)
        .reshape(16, 128, 8 * 128)).astype(BF16)
    wv_np = np.ascontiguousarray(
        Wqkv[:, 2048:].reshape(8, 128, 2, 512).transpose(2, 1, 0, 3)
        .reshape(2, 128, 8 * 512)).astype(BF16)
    bqk_np = np.ascontiguousarray(bqkv[:2048].reshape(16, 128).T)
    bvn_np = np.ascontiguousarray(bqkv[2048:].reshape(1, 1024)).astype(BF16)

    in_maps = []
    for c in range(N_CORES):
        b, q = divmod(c, 4)
        xT_np = np.ascontiguousarray(
            x[b, RPC * q:RPC * (q + 1), :].reshape(RPC, 8, 128)
            .transpose(2, 1, 0)).astype(BF16)
        wp_np = np.ascontiguousarray(
            Wproj[256 * q:256 * (q + 1), :].reshape(2, 128, 1024)
            .transpose(1, 0, 2).reshape(128, 2048)).astype(BF16)
        in_maps.append({
            "xT": xT_np, "wq": wq_np, "wv": wv_np, "bqk": bqk_np,
            "bvn": bvn_np, "wp": wp_np,
        })
    return in_maps


def kernel(x, Wqkv, bqkv, Wproj, bproj):
    nc = _get_program()
    in_maps = _prep_core_inputs(x, Wqkv, bqkv, Wproj, bproj)
    res = run_bass_kernel_spmd(nc, in_maps, list(range(N_CORES)))
    out = np.zeros((B, T, C), dtype=np.float32)
    for c in range(N_CORES):
        out[c // 4] += res.results[c]["out"].astype(np.float32)
    out += np.asarray(bproj, dtype=np.float32)
    return out
